# revision 1
# baseline (speedup 1.0000x reference)
"""DiT block Bass kernel for 8 TRN2 NeuronCores.

Core i -> (b = i//4, g = i%4): batch item b; head group 4g..4g+3; token
quarter [512g, 512g+512) of batch b.  Activations are hidden-major
("transposed", [hidden_chunk=128, tokens]) throughout; PE transposes at
entry (x) and exit (out).  Collectives: AllGather(4) for mod + h,
AllToAll(4) for ctx.  Matmuls bf16 with f32 PSUM accumulate; softmax is
computed without max-subtraction (scores are provably small) with the
relative bias applied multiplicatively post-exp from a host-precomputed
diagonal-shifted exp(bias) table.
"""
import contextlib
import time
import numpy as np
import ml_dtypes
import jax
from jax.sharding import Mesh, PartitionSpec
from jax.experimental.shard_map import shard_map

import concourse.bass as bass
import concourse.mybir as mybir
import concourse.tile as tile
from concourse import bacc
from concourse.bass2jax import _bass_exec_p, install_neuronx_cc_hook, partition_id_tensor

F32 = mybir.dt.float32
BF16 = mybir.dt.bfloat16
AF = mybir.ActivationFunctionType
OP = mybir.AluOpType
ts = bass.ts

B, N, HID = 2, 2048, 1024
NH, HD = 16, 64
MLPH = 4 * HID
NB, MAXD = 32, 128
P = 128
TT = 512
KC = HID // P          # 8
NBLK = N // P          # 16
EB_A = 1920
EB_J = 3968
RG4 = [[0, 1, 2, 3], [4, 5, 6, 7]]


# ---------------------------------------------------------------- host prep
def rel_bucket_np(d):
    nb = NB // 2
    buckets = np.where(d > 0, nb, 0).astype(np.int64)
    rp = np.abs(d)
    max_exact = nb // 2
    is_small = rp < max_exact
    log_ratio = np.log(np.maximum(rp, 1).astype(np.float32) / np.float32(max_exact))
    rpl = max_exact + (
        log_ratio / np.float32(np.log(MAXD / max_exact)) * (nb - max_exact)
    ).astype(np.int32)
    rpl = np.minimum(rpl, nb - 1)
    return buckets + np.where(is_small, rp, rpl)


def make_eb_tables(rel_table):
    d = np.arange(-(N - 1), N)
    buck = rel_bucket_np(d)
    p = np.arange(P)[:, None]
    j = np.arange(EB_J)[None, :]
    dd = p + EB_A - j
    valid = (dd >= -(N - 1)) & (dd <= N - 1)
    idx = np.clip(dd + (N - 1), 0, 2 * N - 2)
    ebs = np.zeros((NH, P, EB_J), dtype=np.float32)
    for h in range(NH):
        bvec = rel_table[buck, h].astype(np.float32)
        tab = np.exp(bvec)[idx]
        tab[~valid] = 1.0
        ebs[h] = tab
    return ebs.astype(ml_dtypes.bfloat16)


def make_in_maps(inputs):
    x = np.asarray(inputs["x"], np.float32)
    c = np.asarray(inputs["c"], np.float32)
    w_ada = np.asarray(inputs["w_ada"], np.float32)
    b_ada = np.asarray(inputs["b_ada"], np.float32)
    w_qkv = np.asarray(inputs["w_qkv"], np.float32)
    b_qkv = np.asarray(inputs["b_qkv"], np.float32)
    w_out = np.asarray(inputs["w_out"], np.float32)
    b_out = np.asarray(inputs["b_out"], np.float32)
    rel_table = np.asarray(inputs["rel_table"], np.float32)
    w_mlp1 = np.asarray(inputs["w_mlp1"], np.float32)
    b_mlp1 = np.asarray(inputs["b_mlp1"], np.float32)
    w_mlp2 = np.asarray(inputs["w_mlp2"], np.float32)
    b_mlp2 = np.asarray(inputs["b_mlp2"], np.float32)

    eb_all = make_eb_tables(rel_table)
    ident = np.eye(P, dtype=np.float32)
    ones_col = np.ones((P, 1), np.float32)
    ones_row = np.ones((1, P), np.float32)

    maps = []
    for i in range(8):
        b, g = divmod(i, 4)
        qs, ks, vs = 256 * g, HID + 256 * g, 2 * HID + 256 * g
        w_qkv_s = np.concatenate(
            [w_qkv[:, qs:qs + 256], w_qkv[:, ks:ks + 256], w_qkv[:, vs:vs + 256]], 1)
        b_qk = np.concatenate([b_qkv[qs:qs + 256], b_qkv[ks:ks + 256]])
        bv = b_qkv[vs:vs + 256]
        maps.append({
            "x_own": np.ascontiguousarray(x[b, 512 * g:512 * (g + 1), :]),
            "c_own": np.ascontiguousarray(c[b][:, None]),
            "w_ada_s": np.ascontiguousarray(
                w_ada[:, 1536 * g:1536 * (g + 1)].reshape(KC, P, 12, P)
                .transpose(2, 1, 0, 3)),
            "b_ada_s": np.ascontiguousarray(
                b_ada[1536 * g:1536 * (g + 1)].reshape(12, P).T),
            "w_qk_r": np.ascontiguousarray(
                w_qkv_s[:, :512].reshape(KC, P, 4, P).transpose(2, 1, 0, 3)),
            "w_v_r": np.ascontiguousarray(
                w_qkv_s[:, 512:].reshape(KC, P, 256).transpose(1, 0, 2)),
            "b_qk_s": np.ascontiguousarray(b_qk.reshape(4, P).T),
            "b_v_bcast": np.ascontiguousarray(
                np.broadcast_to(bv[None, :], (P, 256)).astype(ml_dtypes.bfloat16)),
            "w_out_s": np.ascontiguousarray(
                w_out[256 * g:256 * (g + 1), :].reshape(2, P, HID)
                .transpose(1, 0, 2)),
            "b_out_r": np.ascontiguousarray(b_out.reshape(KC, P).T),
            "w_mlp1": np.ascontiguousarray(
                w_mlp1.reshape(KC, P, MLPH // P, P).transpose(2, 1, 0, 3)),
            "b_mlp1_r": np.ascontiguousarray(b_mlp1.reshape(MLPH // P, P).T),
            "w_mlp2": np.ascontiguousarray(
                w_mlp2.reshape(2, 16, P, KC, P).transpose(3, 0, 2, 1, 4)),
            "b_mlp2_r": np.ascontiguousarray(b_mlp2.reshape(KC, P).T),
            "eb": np.ascontiguousarray(eb_all[4 * g:4 * g + 4]),
            "ident": ident,
            "ones_col": ones_col,
            "ones_row": ones_row,
        })
    return maps


def assemble_output(results):
    out = np.zeros((B, N, HID), np.float32)
    for i in range(8):
        b, g = divmod(i, 4)
        out[b, 512 * g:512 * (g + 1), :] = results[i]["out"]
    return out


# ---------------------------------------------------------------- builder
def build_kernel(sim=False):
    nc = bacc.Bacc("TRN2", target_bir_lowering=False, debug=False, num_devices=8)

    din = lambda nm, sh, dt=F32: nc.dram_tensor(nm, sh, dt, kind="ExternalInput")
    x_own = din("x_own", [TT, HID])
    c_own = din("c_own", [HID, 1])
    w_ada_s = din("w_ada_s", [12, P, KC, P])
    b_ada_s = din("b_ada_s", [P, 12])
    w_qk_r = din("w_qk_r", [4, P, KC, P])
    w_v_r = din("w_v_r", [P, KC, 256])
    b_qk_s = din("b_qk_s", [P, 4])
    b_v_bcast = din("b_v_bcast", [P, 256], BF16)
    w_out_s = din("w_out_s", [P, 2, HID])
    b_out_r = din("b_out_r", [P, KC])
    w_mlp1 = din("w_mlp1", [MLPH // P, P, KC, P])
    b_mlp1_r = din("b_mlp1_r", [P, MLPH // P])
    w_mlp2 = din("w_mlp2", [KC, 2, P, 16, P])
    b_mlp2_r = din("b_mlp2_r", [P, KC])
    eb_in = din("eb", [4, P, EB_J], BF16)
    ident_in = din("ident", [P, P])
    ones_col_in = din("ones_col", [P, 1])
    ones_row_in = din("ones_row", [1, P])

    out_t = nc.dram_tensor("out", [TT, HID], F32, kind="ExternalOutput")

    with tile.TileContext(nc) as tc, contextlib.ExitStack() as ctx:
        const = ctx.enter_context(tc.tile_pool(name="const", bufs=1))
        pers = ctx.enter_context(tc.tile_pool(name="pers", bufs=1))
        big = ctx.enter_context(tc.tile_pool(name="big", bufs=1))
        work = ctx.enter_context(tc.tile_pool(name="work", bufs=3))
        wst = ctx.enter_context(tc.tile_pool(name="wst", bufs=2))
        dram = ctx.enter_context(tc.tile_pool(name="dram", bufs=1, space="DRAM"))
        ebp = ctx.enter_context(tc.tile_pool(name="ebp", bufs=2))
        ps_acc = ctx.enter_context(tc.tile_pool(name="ps_acc", bufs=4, space="PSUM"))
        ps_bc = ctx.enter_context(tc.tile_pool(name="ps_bc", bufs=2, space="PSUM"))
        ps_ctx = ctx.enter_context(tc.tile_pool(name="ps_ctx", bufs=2, space="PSUM"))

        # ---------------- constants
        ident = const.tile([P, P], F32)
        nc.sync.dma_start(ident[:], ident_in.ap())
        ones_col = const.tile([P, 1], F32)
        nc.sync.dma_start(ones_col[:], ones_col_in.ap())
        ones_row = const.tile([1, P], F32)
        nc.sync.dma_start(ones_row[:], ones_row_in.ap())
        b_qk_sb = const.tile([P, 4], F32)
        nc.sync.dma_start(b_qk_sb[:], b_qk_s.ap())
        b_v_sb = const.tile([P, 256], BF16)
        nc.sync.dma_start(b_v_sb[:], b_v_bcast.ap())
        b_out_sb = const.tile([P, KC], F32)
        nc.sync.dma_start(b_out_sb[:], b_out_r.ap())
        b_mlp1_sb = const.tile([P, MLPH // P], F32)
        nc.sync.dma_start(b_mlp1_sb[:], b_mlp1_r.ap())
        b_mlp2_sb = const.tile([P, KC], F32)
        nc.sync.dma_start(b_mlp2_sb[:], b_mlp2_r.ap())
        b_ada_sb = const.tile([P, 12], F32)
        nc.sync.dma_start(b_ada_sb[:], b_ada_s.ap())
        eps_sb = const.tile([1, 1], F32)
        nc.vector.memset(eps_sb[:], 1e-6)

        # ---------------- phase 0: mod shard (this core: w_ada cols 1536g..)
        cT_sb = pers.tile([P, KC], F32)
        nc.sync.dma_start(cT_sb[:], c_own.ap().rearrange("(c p) o -> p (c o)", p=P))
        silu_sb = pers.tile([P, KC], F32)
        nc.scalar.activation(silu_sb[:], cT_sb[:], AF.Silu)
        mod_sh_sb = pers.tile([P, 12], F32)
        for mu in range(12):
            wa = wst.tile([P, KC, P], F32, tag="wf")
            nc.sync.dma_start(wa[:], w_ada_s.ap()[mu])
            mps = ps_acc.tile([P, 1], F32, tag="acc")
            for kc in range(KC):
                nc.tensor.matmul(mps[:], wa[:, kc, :], silu_sb[:, kc:kc + 1],
                                 start=(kc == 0), stop=(kc == KC - 1))
            nc.vector.tensor_scalar_add(
                mod_sh_sb[:, mu:mu + 1], mps[:], b_ada_sb[:, mu:mu + 1])
        mod_bounce_in = dram.tile([P, 12], F32)
        nc.sync.dma_start(mod_bounce_in[:], mod_sh_sb[:])
        mod_bounce_out = dram.tile([4 * P, 12], F32)
        if sim:
            nc.sync.dma_start(mod_bounce_out[:][0:P, :], mod_bounce_in[:])
        else:
            nc.gpsimd.collective_compute(
                "AllGather", OP.bypass, replica_groups=RG4,
                ins=[mod_bounce_in.opt()], outs=[mod_bounce_out.opt()])
        mod_sb = pers.tile([P, 4, 12], F32)
        nc.sync.dma_start(
            mod_sb[:], mod_bounce_out[:].rearrange("(g p) j -> p g j", p=P))

        def mod_chunk(vec_idx, kc):
            gc = 8 * vec_idx + kc
            return mod_sb[:, gc // 12, gc % 12:gc % 12 + 1]

        sc1p_msa = pers.tile([P, KC], F32)
        sc1p_mlp = pers.tile([P, KC], F32)
        for kc in range(KC):
            nc.vector.tensor_scalar_add(sc1p_msa[:, kc:kc + 1], mod_chunk(1, kc), 1.0)
            nc.vector.tensor_scalar_add(sc1p_mlp[:, kc:kc + 1], mod_chunk(4, kc), 1.0)

        # ---------------- phase 1: xT via PE transpose
        xT = pers.tile([P, KC, TT], F32)
        for r in range(TT // P):
            x_sb = work.tile([P, HID], F32, tag="xrow", bufs=3)
            nc.sync.dma_start(x_sb[:], x_own.ap()[ts(r, P), :])
            for kc in range(KC):
                tps = ps_acc.tile([P, P], F32, tag="acc")
                nc.tensor.transpose(tps[:], x_sb[:, ts(kc, P)], ident[:])
                nc.vector.tensor_copy(xT[:, kc, ts(r, P)], tps[:])

        def ln_stats(src, tag):
            sum_ps = ps_acc.tile([1, TT], F32, tag="acc")
            for kc in range(KC):
                nc.tensor.matmul(sum_ps[:], ones_col[:], src[:, kc, :],
                                 start=(kc == 0), stop=(kc == KC - 1))
            sumsq_ps = ps_acc.tile([1, TT], F32, tag="acc")
            for kc in range(KC):
                sq = work.tile([P, TT], F32, tag="wf32", bufs=5)
                nc.scalar.activation(sq[:], src[:, kc, :], AF.Square)
                nc.tensor.matmul(sumsq_ps[:], ones_col[:], sq[:],
                                 start=(kc == 0), stop=(kc == KC - 1))
            m_row = work.tile([1, TT], F32, tag="rowtmp", bufs=4)
            nc.vector.tensor_scalar_mul(m_row[:], sum_ps[:], 1.0 / HID)
            msq = work.tile([1, TT], F32, tag="rowtmp", bufs=4)
            nc.vector.tensor_tensor(msq[:], m_row[:], m_row[:], op=OP.mult)
            var_row = work.tile([1, TT], F32, tag="rowtmp", bufs=4)
            nc.vector.scalar_tensor_tensor(
                var_row[:], sumsq_ps[:], 1.0 / HID, msq[:],
                op0=OP.mult, op1=OP.subtract)
            sd_row = work.tile([1, TT], F32, tag="rowtmp", bufs=4)
            nc.scalar.activation(sd_row[:], var_row[:], AF.Sqrt, bias=eps_sb[:])
            r_row = work.tile([1, TT], F32, tag="rowtmp", bufs=4)
            nc.vector.reciprocal(r_row[:], sd_row[:])
            m_bc = ps_bc.tile([P, TT], F32, tag="bc")
            nc.tensor.matmul(m_bc[:], ones_row[:], m_row[:], start=True, stop=True)
            r_bc = ps_bc.tile([P, TT], F32, tag="bc")
            nc.tensor.matmul(r_bc[:], ones_row[:], r_row[:], start=True, stop=True)
            return m_bc, r_bc

        # ---------------- phase 2: hT own + AllGather
        m_bc, r_bc = ln_stats(xT, "ln1")
        hT_own = big.tile([P, KC, TT], BF16, tag="slot32")
        for kc in range(KC):
            t0 = work.tile([P, TT], F32, tag="wf32", bufs=5)
            nc.vector.tensor_sub(t0[:], xT[:, kc, :], m_bc[:])
            t1 = work.tile([P, TT], F32, tag="wf32", bufs=5)
            nc.vector.tensor_tensor(t1[:], t0[:], r_bc[:], op=OP.mult)
            nc.vector.tensor_scalar(
                hT_own[:, kc, :], t1[:], sc1p_msa[:, kc:kc + 1], mod_chunk(0, kc),
                op0=OP.mult, op1=OP.add)
        h_bounce_in_a = dram.tile([HID // 2, TT], BF16)
        h_bounce_in_b = dram.tile([HID // 2, TT], BF16)
        nc.sync.dma_start(
            h_bounce_in_a[:].rearrange("(c p) t -> p c t", p=P), hT_own[:, 0:4, :])
        nc.sync.dma_start(
            h_bounce_in_b[:].rearrange("(c p) t -> p c t", p=P), hT_own[:, 4:8, :])
        h_bounce_out_a = dram.tile([2 * HID, TT], BF16)
        h_bounce_out_b = dram.tile([2 * HID, TT], BF16)
        if sim:
            nc.sync.dma_start(h_bounce_out_a[:][0:HID // 2, :], h_bounce_in_a[:])
            nc.sync.dma_start(h_bounce_out_b[:][0:HID // 2, :], h_bounce_in_b[:])
        else:
            nc.gpsimd.collective_compute(
                "AllGather", OP.bypass, replica_groups=RG4,
                ins=[h_bounce_in_a.opt()], outs=[h_bounce_out_a.opt()])
            nc.gpsimd.collective_compute(
                "AllGather", OP.bypass, replica_groups=RG4,
                ins=[h_bounce_in_b.opt()], outs=[h_bounce_out_b.opt()])
        hT_full = big.tile([P, 32, TT], BF16, tag="slot32")
        for jq in range(4):
            nc.sync.dma_start(
                hT_full[:, KC * jq:KC * jq + 4, :],
                h_bounce_out_a[:][ts(jq, HID // 2), :].rearrange("(c p) t -> p c t", p=P))
            nc.sync.dma_start(
                hT_full[:, KC * jq + 4:KC * jq + 8, :],
                h_bounce_out_b[:][ts(jq, HID // 2), :].rearrange("(c p) t -> p c t", p=P))

        # ---------------- phase 3: qkv
        qT = pers.tile([P, 2, N], BF16)
        kT = pers.tile([P, 2, N], BF16)
        v_aug = pers.tile([P, NBLK, 260], BF16)
        nc.vector.memset(
            v_aug[:].rearrange("p b (h e) -> p b h e", h=4)[:, :, :, 64:65], 1.0)

        wvf = wst.tile([P, KC, 256], F32, tag="wf")
        nc.sync.dma_start(wvf[:], w_v_r.ap())
        wvb = wst.tile([P, KC, 256], BF16, tag="wb")
        nc.scalar.activation(wvb[:], wvf[:], AF.Copy)
        for blk in range(NBLK):
            ps = ps_acc.tile([P, 256], F32, tag="acc")
            for kc in range(KC):
                nc.tensor.matmul(
                    ps[:], hT_full[:, 8 * (blk // 4) + kc, ts(blk % 4, P)],
                    wvb[:, kc, :], start=(kc == 0), stop=(kc == KC - 1))
            vtmp = work.tile([P, 256], BF16, tag="wbf", bufs=6)
            nc.vector.tensor_copy(vtmp[:], ps[:])
            nc.vector.tensor_add(
                v_aug[:, blk, :].rearrange("p (h e) -> p h e", h=4)[:, :, 0:64],
                vtmp[:].rearrange("p (h e) -> p h e", h=4), b_v_sb[:].rearrange("p (h e) -> p h e", h=4))

        for mu in range(4):       # q chunks 0,1; k chunks 2,3
            wqf = wst.tile([P, KC, P], F32, tag="wf")
            nc.sync.dma_start(wqf[:], w_qk_r.ap()[mu])
            wqb = wst.tile([P, KC, P], BF16, tag="wb")
            nc.scalar.activation(wqb[:], wqf[:], AF.Copy)
            for tau in range(4):
                ps = ps_acc.tile([P, TT], F32, tag="acc")
                for kc in range(KC):
                    nc.tensor.matmul(
                        ps[:], wqb[:, kc, :], hT_full[:, 8 * tau + kc, :],
                        start=(kc == 0), stop=(kc == KC - 1))
                dst = qT if mu < 2 else kT
                nc.vector.tensor_scalar_add(
                    dst[:, mu % 2, ts(tau, TT)], ps[:], b_qk_sb[:, mu:mu + 1])
        # ---------------- phase 4: attention
        ctxT = pers.tile([P, 2, N], BF16)
        for a in range(2):
            eb_sb = ebp.tile([P, 2, EB_J], BF16, tag="eb")
            nc.sync.dma_start(
                eb_sb[:], eb_in.ap()[2 * a:2 * a + 2].rearrange("h p j -> p h j"))
            for tau in range(4):
                cps0 = ps_ctx.tile([65, TT], F32, tag="ctx")
                cps1 = ps_ctx.tile([65, TT], F32, tag="ctx")
                cps = [cps0, cps1]
                for blk in range(NBLK):
                    col0 = EB_A - P * (blk - 4 * tau)
                    sps = []
                    for o in range(2):
                        sp = ps_acc.tile([P, TT], F32, tag="acc")
                        nc.tensor.matmul(
                            sp[:],
                            kT[64 * o:64 * o + 64, a, ts(blk, P)],
                            qT[64 * o:64 * o + 64, a, ts(tau, TT)],
                            start=True, stop=True)
                        sps.append(sp)
                    for o in range(2):
                        h = 2 * a + o
                        tsb = work.tile([P, TT], BF16, tag="wbf", bufs=6)
                        nc.scalar.activation(tsb[:], sps[o][:], AF.Exp, scale=0.125)
                        esb = work.tile([P, TT], BF16, tag="wbf", bufs=6)
                        nc.vector.tensor_tensor(
                            esb[:], tsb[:], eb_sb[:, o, col0:col0 + TT], op=OP.mult)
                        nc.tensor.matmul(
                            cps[o][:], v_aug[:, blk, 65 * h:65 * h + 65], esb[:],
                            start=(blk == 0), stop=(blk == NBLK - 1))
                for o in range(2):
                    recip = work.tile([1, TT], F32, tag="rowtmp", bufs=4)
                    nc.vector.reciprocal(recip[:], cps[o][64:65, :])
                    bc = ps_bc.tile([64, TT], F32, tag="bc")
                    nc.tensor.matmul(bc[:], ones_row[:, 0:64], recip[:],
                                     start=True, stop=True)
                    csb = work.tile([64, TT], BF16, tag="wbf", bufs=6)
                    nc.scalar.activation(csb[:], cps[o][0:64, :], AF.Copy)
                    nc.vector.tensor_tensor(
                        ctxT[64 * o:64 * o + 64, a, ts(tau, TT)],
                        csb[:], bc[:], op=OP.mult)

        # ---------------- phase 5: head-sharded out-proj partials + RS(add)
        # partial attn_out^T over own 4 heads (ctx dims 256), ALL tokens
        wof = wst.tile([P, 2, HID], F32, tag="wf")
        nc.sync.dma_start(wof[:], w_out_s.ap())
        wob = wst.tile([P, 2, HID], BF16, tag="wb")
        nc.vector.tensor_copy(wob[:], wof[:])
        po_sb = big.tile([P, KC, N], BF16, tag="slot32")
        for tau in range(4):
            for mu in range(KC):
                ps = ps_acc.tile([P, TT], F32, tag="acc")
                for kc in range(2):
                    nc.tensor.matmul(
                        ps[:], wob[:, kc, ts(mu, P)],
                        ctxT[:, kc, ts(tau, TT)],
                        start=(kc == 0), stop=(kc == 1))
                nc.vector.tensor_copy(po_sb[:, mu, ts(tau, TT)], ps[:])
        rs_bounce_in = dram.tile([4 * HID, TT], BF16)
        for j in range(4):
            nc.sync.dma_start(
                rs_bounce_in[:][ts(j, HID), :].rearrange("(c p) t -> p c t", p=P),
                po_sb[:, :, ts(j, TT)])
        rs_bounce_out = dram.tile([HID, TT], BF16)
        if sim:
            nc.sync.dma_start(rs_bounce_out[:], rs_bounce_in[:][0:HID, :])
        else:
            nc.gpsimd.collective_compute(
                "ReduceScatter", OP.add, replica_groups=RG4,
                ins=[rs_bounce_in.opt()], outs=[rs_bounce_out.opt()])
        ao_sb = pers.tile([P, KC, TT], BF16)
        nc.sync.dma_start(
            ao_sb[:], rs_bounce_out[:].rearrange("(c p) t -> p c t", p=P))

        # ---------------- phase 6: residual + LN2
        x2T = pers.tile([P, KC, TT], F32)
        for mu in range(KC):
            tmp = work.tile([P, TT], F32, tag="wf32", bufs=5)
            nc.vector.tensor_scalar(
                tmp[:], ao_sb[:, mu, :], b_out_sb[:, mu:mu + 1], mod_chunk(2, mu),
                op0=OP.add, op1=OP.mult)
            nc.vector.tensor_add(x2T[:, mu, :], tmp[:], xT[:, mu, :])

        m2_bc, r2_bc = ln_stats(x2T, "ln2")
        h2T = pers.tile([P, KC, TT], BF16)
        for kc in range(KC):
            t0 = work.tile([P, TT], F32, tag="wf32", bufs=5)
            nc.vector.tensor_sub(t0[:], x2T[:, kc, :], m2_bc[:])
            t1 = work.tile([P, TT], F32, tag="wf32", bufs=5)
            nc.vector.tensor_tensor(t1[:], t0[:], r2_bc[:], op=OP.mult)
            nc.vector.tensor_scalar(
                h2T[:, kc, :], t1[:], sc1p_mlp[:, kc:kc + 1], mod_chunk(3, kc),
                op0=OP.mult, op1=OP.add)

        # ---------------- phase 7: MLP (token-sharded, weights streamed)
        gT = big.tile([P, MLPH // P, TT], BF16, tag="slot32")
        for nu in range(MLPH // P):
            w1f = wst.tile([P, KC, P], F32, tag="wf")
            nc.sync.dma_start(w1f[:], w_mlp1.ap()[nu])
            w1b = wst.tile([P, KC, P], BF16, tag="wb")
            nc.scalar.activation(w1b[:], w1f[:], AF.Copy)
            ps = ps_acc.tile([P, TT], F32, tag="acc")
            for kc in range(KC):
                nc.tensor.matmul(ps[:], w1b[:, kc, :], h2T[:, kc, :],
                                 start=(kc == 0), stop=(kc == KC - 1))
            nc.scalar.activation(
                gT[:, nu, :], ps[:], AF.Gelu_apprx_tanh, bias=b_mlp1_sb[:, nu:nu + 1])
        for mu in range(KC):
            ps = ps_acc.tile([P, TT], F32, tag="acc")
            for half in range(2):
                w2f = wst.tile([P, 16, P], F32, tag="wf")
                nc.sync.dma_start(w2f[:], w_mlp2.ap()[mu, half])
                w2b = wst.tile([P, 16, P], BF16, tag="wb")
                nc.vector.tensor_copy(w2b[:], w2f[:])
                for kc in range(16):
                    gkc = 16 * half + kc
                    nc.tensor.matmul(ps[:], w2b[:, kc, :], gT[:, gkc, :],
                                     start=(gkc == 0), stop=(gkc == MLPH // P - 1))
            tmp = work.tile([P, TT], F32, tag="wf32", bufs=5)
            nc.vector.tensor_scalar(
                tmp[:], ps[:], b_mlp2_sb[:, mu:mu + 1], mod_chunk(5, mu),
                op0=OP.add, op1=OP.mult)
            outT = work.tile([P, TT], F32, tag="wf32", bufs=5)
            nc.vector.tensor_add(outT[:], tmp[:], x2T[:, mu, :])
            for r in range(TT // P):
                tps = ps_acc.tile([P, P], F32, tag="acc")
                nc.tensor.transpose(tps[:], outT[:, ts(r, P)], ident[:])
                osb = work.tile([P, P], F32, tag="osb", bufs=4)
                nc.vector.tensor_copy(osb[:], tps[:])
                nc.sync.dma_start(out_t.ap()[ts(r, P), ts(mu, P)], osb[:])

    nc.compile()
    return nc


# ---------------------------------------------------------------- runner



class SpmdRunner:
    def __init__(self, nc, n_cores):
        install_neuronx_cc_hook()
        self.nc = nc
        self.n_cores = n_cores
        partition_name = nc.partition_id_tensor.name if nc.partition_id_tensor else None
        in_names, out_names, out_avals = [], [], []
        for alloc in nc.m.functions[0].allocations:
            if not isinstance(alloc, mybir.MemoryLocationSet):
                continue
            name = alloc.memorylocations[0].name
            if alloc.kind == "ExternalInput":
                if name != partition_name:
                    in_names.append(name)
            elif alloc.kind == "ExternalOutput":
                out_names.append(name)
                out_avals.append(
                    jax.core.ShapedArray(tuple(alloc.tensor_shape), mybir.dt.np(alloc.dtype))
                )
        self.in_names, self.out_names, self.out_avals = in_names, out_names, out_avals
        n_params = len(in_names)
        n_outs = len(out_avals)
        all_in_names = list(in_names) + list(out_names)
        if partition_name is not None:
            all_in_names.append(partition_name)

        def _body(*args):
            operands = list(args)
            if partition_name is not None:
                operands.append(partition_id_tensor())
            return tuple(
                _bass_exec_p.bind(
                    *operands,
                    out_avals=tuple(out_avals),
                    in_names=tuple(all_in_names),
                    out_names=tuple(out_names),
                    lowering_input_output_aliases=(),
                    sim_require_finite=True,
                    sim_require_nnan=True,
                    nc=nc,
                )
            )

        devices = jax.devices()[:n_cores]
        self.mesh = Mesh(np.asarray(devices), ("core",))
        donate = tuple(range(n_params, n_params + n_outs))
        self.fn = jax.jit(
            shard_map(
                _body,
                mesh=self.mesh,
                in_specs=(PartitionSpec("core"),) * (n_params + n_outs),
                out_specs=(PartitionSpec("core"),) * n_outs,
                check_rep=False,
            ),
            donate_argnums=donate,
            keep_unused=True,
        )
        self.n_params, self.n_outs = n_params, n_outs

    def _concat_inputs(self, in_maps):
        return [
            np.concatenate([np.asarray(in_maps[c][n]) for c in range(self.n_cores)], axis=0)
            for n in self.in_names
        ]

    def run(self, in_maps):
        sharding = jax.sharding.NamedSharding(self.mesh, PartitionSpec("core"))
        concat_in = [
            jax.device_put(x, sharding) for x in self._concat_inputs(in_maps)
        ]
        zeros = [
            jax.device_put(
                np.zeros((self.n_cores * a.shape[0], *a.shape[1:]), a.dtype), sharding)
            for a in self.out_avals
        ]
        outs = self.fn(*concat_in, *zeros)
        return self._split(outs)

    def _split(self, out_arrs):
        return [
            {
                n: np.asarray(out_arrs[i]).reshape(self.n_cores, *self.out_avals[i].shape)[c]
                for i, n in enumerate(self.out_names)
            }
            for c in range(self.n_cores)
        ]

    def bench(self, in_maps, iters=30, warmup=3):
        """Chained repeated execution: output buffers of call i are donated as
        the output operands of call i+1, serializing calls on-device."""
        sharding = jax.sharding.NamedSharding(self.mesh, PartitionSpec("core"))
        concat_in = [jax.device_put(x, sharding) for x in self._concat_inputs(in_maps)]
        outs = tuple(
            jax.device_put(
                np.zeros((self.n_cores * a.shape[0], *a.shape[1:]), a.dtype), sharding)
            for a in self.out_avals
        )
        for _ in range(warmup):
            outs = self.fn(*concat_in, *outs)
        jax.block_until_ready(outs)
        t0 = time.perf_counter()
        for _ in range(iters):
            outs = self.fn(*concat_in, *outs)
        jax.block_until_ready(outs)
        t1 = time.perf_counter()
        return (t1 - t0) / iters, self._split(outs)


_CACHE = {}


def kernel(**inputs):
    """Full-input DiT block on 8 NeuronCores; returns full [B, N, HID] f32."""
    if "nc" not in _CACHE:
        _CACHE["nc"] = build_kernel()
        _CACHE["runner"] = SpmdRunner(_CACHE["nc"], 8)
    maps = make_in_maps(inputs)
    results = _CACHE["runner"].run(maps)
    return assemble_output(results)



# revision 16
# speedup vs baseline: 1.1785x; 1.1785x over previous
"""DiT block Bass kernel for 8 TRN2 NeuronCores.

Core i -> (b = i//4, g = i%4): batch item b; head group 4g..4g+3; token
quarter [512g, 512g+512) of batch b.  Activations are hidden-major
([hidden_chunk=128, tokens]) throughout; PE transposes at entry (x) and
exit (out).  Collectives: AllGather(4) for mod + h, ReduceScatter(4)
for the out-projection partials.

Dtype strategy: residual stream bf16; weights host-quantized (w_qkv,
w_out, w_mlp1, w_mlp2 in fp8e4m3 prescaled by 32; w_ada bf16); all big
GEMMs except QK^T run in fp8 DoubleRow perf mode (2 k-subtiles of 128
per pass).  Relative attention bias: constant for |d| >= 91, so
off-band score tiles get their bias via the exp() bias operand; band
tiles get a PE matmul-add of pretransposed bias tiles into PSUM before
exp.  Softmax is computed without max-subtraction (scores provably
small); denominators come from an appended ones-row in the fp8 V tiles.
"""
import contextlib
import time
import numpy as np
import ml_dtypes
import jax
from jax.sharding import Mesh, PartitionSpec
from jax.experimental.shard_map import shard_map

import concourse.bass as bass
import concourse.mybir as mybir
import concourse.tile as tile
from concourse import bacc
from concourse.bass2jax import _bass_exec_p, install_neuronx_cc_hook, partition_id_tensor

F32 = mybir.dt.float32
BF16 = mybir.dt.bfloat16
FP8 = mybir.dt.float8e4
AF = mybir.ActivationFunctionType
OP = mybir.AluOpType
DR = mybir.MatmulPerfMode.DoubleRow
ts = bass.ts

NPBF16 = ml_dtypes.bfloat16
NPFP8 = ml_dtypes.float8_e4m3

B, N, HID = 2, 2048, 1024
NH, HD = 16, 64
MLPH = 4 * HID
NB, MAXD = 32, 128
P = 128
TT = 512
KC = HID // P          # 8
NBLK = N // P          # 16
WS = 32.0              # host weight prescale for fp8
CXS = 64.0             # ctx prescale for fp8
RG4 = [[0, 1, 2, 3], [4, 5, 6, 7]]


# ---------------------------------------------------------------- host prep
def rel_bucket_np(d):
    nb = NB // 2
    buckets = np.where(d > 0, nb, 0).astype(np.int64)
    rp = np.abs(d)
    max_exact = nb // 2
    is_small = rp < max_exact
    log_ratio = np.log(np.maximum(rp, 1).astype(np.float32) / np.float32(max_exact))
    rpl = max_exact + (
        log_ratio / np.float32(np.log(MAXD / max_exact)) * (nb - max_exact)
    ).astype(np.int32)
    rpl = np.minimum(rpl, nb - 1)
    return buckets + np.where(is_small, rp, rpl)


def make_bias_tables(rel_table, g):
    """Band bias tiles + deltas for local heads 4g..4g+3.

    bt[h, j][p, m] = 8 * badj(128*(j-1) + m - p)   (j = koff+1, koff in -1..1)
    badj(d) = bias(d) - bias_minus;  delta = bias_plus - bias_minus.
    The 8x prescale compensates the 0.125 exp scale (bias added in PSUM
    pre-scale, exp bias arg applied post-scale).
    """
    d = np.arange(-(N - 1), N)
    buck = rel_bucket_np(d)  # index by d + N-1
    bt = np.zeros((4, 3, P, P), np.float32)
    delta = np.zeros((4,), np.float32)
    for hl in range(4):
        hg = 4 * g + hl
        bvec = rel_table[:, hg].astype(np.float32)
        bmin = bvec[NB // 2 - 1]
        delta[hl] = bvec[NB - 1] - bmin
        diag = bvec[buck] - bmin  # badj over d in [-(N-1), N-1]
        p = np.arange(P)[:, None]
        m = np.arange(P)[None, :]
        for j, koff in enumerate((-1, 0, 1)):
            dd = 128 * koff + m - p
            bt[hl, j] = 8.0 * diag[dd + (N - 1)]
    return bt.astype(NPBF16), delta


def make_in_maps(inputs):
    x = np.asarray(inputs["x"], np.float32)
    c = np.asarray(inputs["c"], np.float32)
    w_ada = np.asarray(inputs["w_ada"], np.float32)
    b_ada = np.asarray(inputs["b_ada"], np.float32)
    w_qkv = np.asarray(inputs["w_qkv"], np.float32)
    b_qkv = np.asarray(inputs["b_qkv"], np.float32)
    w_out = np.asarray(inputs["w_out"], np.float32)
    b_out = np.asarray(inputs["b_out"], np.float32)
    rel_table = np.asarray(inputs["rel_table"], np.float32)
    w_mlp1 = np.asarray(inputs["w_mlp1"], np.float32)
    b_mlp1 = np.asarray(inputs["b_mlp1"], np.float32)
    w_mlp2 = np.asarray(inputs["w_mlp2"], np.float32)
    b_mlp2 = np.asarray(inputs["b_mlp2"], np.float32)

    ident = np.eye(P, dtype=np.float32)
    ident_b = np.eye(P, dtype=np.float32).astype(NPBF16)
    ones_col_b = np.ones((P, 1), np.float32).astype(NPBF16)
    ones_row_b = np.ones((1, P), np.float32).astype(NPBF16)
    ones64_row = np.full((1, 64), CXS, np.float32).astype(NPBF16)

    w_mlp1_b = np.ascontiguousarray(
        w_mlp1.reshape(KC, P, MLPH // P, P).transpose(2, 1, 0, 3)
        .astype(NPBF16))                      # [32, P, kc8, P]
    w_mlp2_b = np.ascontiguousarray(
        w_mlp2.reshape(MLPH // P, P, KC, P).transpose(2, 1, 0, 3)
        .astype(NPBF16))                      # [8, P, kc32, P]

    maps = []
    for i in range(8):
        b, g = divmod(i, 4)
        qs, ks, vs = 256 * g, HID + 256 * g, 2 * HID + 256 * g
        w_qk = np.concatenate([w_qkv[:, qs:qs + 256], w_qkv[:, ks:ks + 256]], 1)
        w_v = w_qkv[:, vs:vs + 256]
        b_qk = np.concatenate([b_qkv[qs:qs + 256], b_qkv[ks:ks + 256]])
        bv = b_qkv[vs:vs + 256]

        # [P, mu4, kp4, 2, P]: global k = 128*(2*kp+sub) + p, out chunk mu
        w_qk_q = (w_qk * WS).reshape(4, 2, P, 4, P).transpose(
            2, 3, 0, 1, 4).astype(NPFP8)
        # [P, kp4, 2, 256]
        w_v_q = (w_v * WS).reshape(4, 2, P, 256).transpose(2, 0, 1, 3).astype(NPFP8)
        # [P, 2, HID]: ctx chunk-major (2 chunks of own 256 ctx dims)
        w_out_q = (w_out[256 * g:256 * (g + 1), :] * WS).reshape(
            2, P, HID).transpose(1, 0, 2).astype(NPFP8)

        bt, delta = make_bias_tables(rel_table, g)
        maps.append({
            "x_own": np.ascontiguousarray(x[b, 512 * g:512 * (g + 1), :]),
            "c_own": np.ascontiguousarray(c[b][:, None]),
            "w_ada_s": np.ascontiguousarray(
                w_ada[:, 1536 * g:1536 * (g + 1)].reshape(KC, P, 12, P)
                .transpose(2, 1, 0, 3).astype(NPBF16)),
            "b_ada_s": np.ascontiguousarray(
                b_ada[1536 * g:1536 * (g + 1)].reshape(12, P).T),
            "w_qk_q": np.ascontiguousarray(w_qk_q),
            "w_v_q": np.ascontiguousarray(w_v_q),
            "b_qk_s": np.ascontiguousarray(b_qk.reshape(4, P).T),
            "b_v_bcast": np.ascontiguousarray(
                np.broadcast_to(bv[None, :], (P, 256)).astype(NPBF16)),
            "w_out_q": np.ascontiguousarray(w_out_q),
            "b_out_r": np.ascontiguousarray(b_out.reshape(KC, P).T),
            "w_mlp1_b": w_mlp1_b,
            "b_mlp1_r": np.ascontiguousarray(b_mlp1.reshape(MLPH // P, P).T),
            "w_mlp2_b": w_mlp2_b,
            "b_mlp2_r": np.ascontiguousarray(b_mlp2.reshape(KC, P).T),
            "bt": np.ascontiguousarray(bt.reshape(12, P, P)),
            "delta_row8": np.ascontiguousarray(
                np.broadcast_to((8.0 * delta)[None, :, None], (1, 4, P))
                .astype(NPBF16)),
            "delta_col": np.ascontiguousarray(
                np.broadcast_to(delta[None, :], (P, 4)).astype(np.float32)),
            "ident": ident,
            "ident_b": ident_b,
            "ones_col_b": ones_col_b,
            "ones_row_b": ones_row_b,
            "ones64_row": ones64_row,
        })
    return maps


def assemble_output(results):
    out = np.zeros((B, N, HID), np.float32)
    for i in range(8):
        b, g = divmod(i, 4)
        out[b, 512 * g:512 * (g + 1), :] = results[i]["out"]
    return out


# ---------------------------------------------------------------- builder
def build_kernel(sim=False):
    nc = bacc.Bacc("TRN2", target_bir_lowering=False, debug=False, num_devices=8)

    din = lambda nm, sh, dt=F32: nc.dram_tensor(nm, sh, dt, kind="ExternalInput")
    x_own = din("x_own", [TT, HID])
    c_own = din("c_own", [HID, 1])
    w_ada_s = din("w_ada_s", [12, P, KC, P], BF16)
    b_ada_s = din("b_ada_s", [P, 12])
    w_qk_q = din("w_qk_q", [P, 4, 4, 2, P], FP8)
    w_v_q = din("w_v_q", [P, 4, 2, 256], FP8)
    b_qk_s = din("b_qk_s", [P, 4])
    b_v_bcast = din("b_v_bcast", [P, 256], BF16)
    w_out_q = din("w_out_q", [P, 2, HID], FP8)
    b_out_r = din("b_out_r", [P, KC])
    w_mlp1_b = din("w_mlp1_b", [MLPH // P, P, KC, P], BF16)
    b_mlp1_r = din("b_mlp1_r", [P, MLPH // P])
    w_mlp2_b = din("w_mlp2_b", [KC, P, MLPH // P, P], BF16)
    b_mlp2_r = din("b_mlp2_r", [P, KC])
    bt_in = din("bt", [12, P, P], BF16)
    delta_row8_in = din("delta_row8", [1, 4, P], BF16)
    delta_col_in = din("delta_col", [P, 4])
    ident_in = din("ident", [P, P])
    ident_b_in = din("ident_b", [P, P], BF16)
    ones_col_b_in = din("ones_col_b", [P, 1], BF16)
    ones_row_b_in = din("ones_row_b", [1, P], BF16)
    ones64_row_in = din("ones64_row", [1, 64], BF16)

    out_t = nc.dram_tensor("out", [TT, HID], F32, kind="ExternalOutput")

    with tile.TileContext(nc) as tc, contextlib.ExitStack() as ctx:
        const = ctx.enter_context(tc.tile_pool(name="const", bufs=1))
        pers = ctx.enter_context(tc.tile_pool(name="pers", bufs=1))
        work = ctx.enter_context(tc.tile_pool(name="work", bufs=3))
        wst = ctx.enter_context(tc.tile_pool(name="wst", bufs=2))
        dram = ctx.enter_context(tc.tile_pool(name="dram", bufs=1, space="DRAM"))
        ps_a = ctx.enter_context(tc.tile_pool(name="ps_a", bufs=2, space="PSUM"))
        ps_c = ctx.enter_context(tc.tile_pool(name="ps_c", bufs=2, space="PSUM"))

        def pa():
            return ps_a.tile([P, 1024], F32, tag="A", name="pa")

        def pc():
            return ps_c.tile([72, 1024], F32, tag="C", name="pc")

        # ---------------- constants
        ident = const.tile([P, P], F32)
        nc.sync.dma_start(ident[:], ident_in.ap())
        ident_b = const.tile([P, P], BF16)
        nc.sync.dma_start(ident_b[:], ident_b_in.ap())
        ones_col_b = const.tile([P, 1], BF16)
        nc.sync.dma_start(ones_col_b[:], ones_col_b_in.ap())
        ones_row_b = const.tile([1, P], BF16)
        nc.sync.dma_start(ones_row_b[:], ones_row_b_in.ap())
        ones64_row = const.tile([1, 64], BF16)
        nc.sync.dma_start(ones64_row[:], ones64_row_in.ap())
        b_qk_sb = const.tile([P, 4], F32)
        nc.sync.dma_start(b_qk_sb[:], b_qk_s.ap())
        b_v_sb = const.tile([P, 256], BF16)
        nc.sync.dma_start(b_v_sb[:], b_v_bcast.ap())
        b_out_sb = const.tile([P, KC], F32)
        nc.sync.dma_start(b_out_sb[:], b_out_r.ap())
        b_mlp1_sb = const.tile([P, MLPH // P], F32)
        nc.sync.dma_start(b_mlp1_sb[:], b_mlp1_r.ap())
        b_mlp2_sb = const.tile([P, KC], F32)
        nc.sync.dma_start(b_mlp2_sb[:], b_mlp2_r.ap())
        b_ada_sb = const.tile([P, 12], F32)
        nc.sync.dma_start(b_ada_sb[:], b_ada_s.ap())
        bt_sb = const.tile([P, 12, P], BF16)
        nc.sync.dma_start(bt_sb[:], bt_in.ap().rearrange("j p m -> p j m"))
        delta_row8 = const.tile([1, 4, P], BF16)
        nc.sync.dma_start(delta_row8[:], delta_row8_in.ap())
        delta_col = const.tile([P, 4], F32)
        nc.sync.dma_start(delta_col[:], delta_col_in.ap())
        wqk_sb = const.tile([P, 4, 4, 2, P], FP8)
        nc.sync.dma_start(wqk_sb[:], w_qk_q.ap())
        wv_sb = const.tile([P, 4, 2, 256], FP8)
        nc.sync.dma_start(wv_sb[:], w_v_q.ap())
        wout_sb = const.tile([P, 2, HID], FP8)
        nc.sync.dma_start(wout_sb[:], w_out_q.ap())
        eps_sb = const.tile([1, 1], F32)
        nc.vector.memset(eps_sb[:], 1e-6)

        # ---------------- phase 0: mod shard (this core: w_ada cols 1536g..)
        cT_sb = pers.tile([P, KC], F32)
        nc.sync.dma_start(cT_sb[:], c_own.ap().rearrange("(c p) o -> p (c o)", p=P))
        silu_sb = pers.tile([P, KC], BF16)
        nc.scalar.activation(silu_sb[:], cT_sb[:], AF.Silu)
        mod_sh_sb = pers.tile([P, 12], F32)
        for mu in range(12):
            wa = wst.tile([P, KC, P], BF16, tag="wada")
            nc.sync.dma_start(wa[:], w_ada_s.ap()[mu])
            mps = pa()
            for kc in range(KC):
                nc.tensor.matmul(mps[:, 0:1], wa[:, kc, :], silu_sb[:, kc:kc + 1],
                                 start=(kc == 0), stop=(kc == KC - 1))
            nc.vector.tensor_scalar_add(
                mod_sh_sb[:, mu:mu + 1], mps[:, 0:1], b_ada_sb[:, mu:mu + 1])
        mod_bounce_in = dram.tile([P, 12], F32)
        nc.sync.dma_start(mod_bounce_in[:], mod_sh_sb[:])
        mod_bounce_out = dram.tile([4 * P, 12], F32)
        if sim:
            nc.sync.dma_start(mod_bounce_out[:][0:P, :], mod_bounce_in[:])
        else:
            nc.gpsimd.collective_compute(
                "AllGather", OP.bypass, replica_groups=RG4,
                ins=[mod_bounce_in.opt()], outs=[mod_bounce_out.opt()])
        mod_sb = pers.tile([P, 4, 12], F32)
        nc.sync.dma_start(
            mod_sb[:], mod_bounce_out[:].rearrange("(g p) j -> p g j", p=P))

        def mod_chunk(vec_idx, kc):
            gc = 8 * vec_idx + kc
            return mod_sb[:, gc // 12, gc % 12:gc % 12 + 1]

        sc1p_msa = pers.tile([P, KC], F32)
        sc1p_mlp = pers.tile([P, KC], F32)
        s1_msa = pers.tile([P, KC], F32)
        s2_msa = pers.tile([P, KC], F32)
        s1_mlp = pers.tile([P, KC], F32)
        s2_mlp = pers.tile([P, KC], F32)
        for kc in range(KC):
            nc.vector.tensor_scalar_add(sc1p_msa[:, kc:kc + 1], mod_chunk(1, kc), 1.0)
            nc.vector.tensor_scalar_add(sc1p_mlp[:, kc:kc + 1], mod_chunk(4, kc), 1.0)
            nc.vector.tensor_scalar_mul(
                s1_msa[:, kc:kc + 1], mod_chunk(2, kc), 1.0 / (WS * CXS))
            nc.vector.tensor_tensor(
                s2_msa[:, kc:kc + 1], mod_chunk(2, kc), b_out_sb[:, kc:kc + 1],
                op=OP.mult)
            nc.vector.tensor_scalar_mul(
                s1_mlp[:, kc:kc + 1], mod_chunk(5, kc), 1.0)
            nc.vector.tensor_tensor(
                s2_mlp[:, kc:kc + 1], mod_chunk(5, kc), b_mlp2_sb[:, kc:kc + 1],
                op=OP.mult)

        # ---------------- phase 1: xT (bf16) via PE transpose
        xT = pers.tile([P, KC, TT], BF16)
        for r in range(TT // P):
            x_sb = work.tile([P, HID], F32, tag="xrow", bufs=2)
            nc.sync.dma_start(x_sb[:], x_own.ap()[ts(r, P), :])
            for kc in range(KC):
                tps = pa()
                nc.tensor.transpose(tps[:, 0:P], x_sb[:, ts(kc, P)], ident[:])
                nc.vector.tensor_copy(xT[:, kc, ts(r, P)], tps[:, 0:P])

        def ln_stats(src):
            """src: [P, KC, TT] bf16 -> (m_bc, r_bc) [P, TT] f32 PSUM tiles."""
            sum_c = pc()
            for kc in range(KC):
                nc.tensor.matmul(sum_c[0:1, 0:TT], ones_col_b[:], src[:, kc, :],
                                 start=(kc == 0), stop=(kc == KC - 1))
            sumsq_c = pc()
            for kc in range(KC):
                sq = work.tile([P, TT], BF16, tag="sq", bufs=3)
                nc.gpsimd.tensor_tensor(sq[:], src[:, kc, :], src[:, kc, :],
                                        op=OP.mult)
                nc.tensor.matmul(sumsq_c[0:1, 0:TT], ones_col_b[:], sq[:],
                                 start=(kc == 0), stop=(kc == KC - 1))
            m_row = work.tile([1, TT], BF16, tag="rowtmp", bufs=4)
            nc.vector.tensor_scalar_mul(m_row[:], sum_c[0:1, 0:TT], 1.0 / HID)
            msq = work.tile([1, TT], BF16, tag="rowtmp", bufs=4)
            nc.vector.tensor_tensor(msq[:], m_row[:], m_row[:], op=OP.mult)
            var_row = work.tile([1, TT], F32, tag="rowtmp", bufs=4)
            nc.vector.scalar_tensor_tensor(
                var_row[:], sumsq_c[0:1, 0:TT], 1.0 / HID, msq[:],
                op0=OP.mult, op1=OP.subtract)
            sd_row = work.tile([1, TT], F32, tag="rowtmp", bufs=4)
            nc.scalar.activation(sd_row[:], var_row[:], AF.Sqrt, bias=eps_sb[:])
            r_row = work.tile([1, TT], BF16, tag="rowtmp", bufs=4)
            with nc.allow_low_precision(reason="rstd in bf16 is plenty for LN"):
                nc.vector.reciprocal(r_row[:], sd_row[:])
            m_ps = pa()
            nc.tensor.matmul(m_ps[:, 0:TT], ones_row_b[:], m_row[:],
                             start=True, stop=True)
            r_ps = pa()
            nc.tensor.matmul(r_ps[:, 0:TT], ones_row_b[:], r_row[:],
                             start=True, stop=True)
            return m_ps, r_ps

        # ---------------- phase 2: hT own (fp8) + AllGather
        m_ps, r_ps = ln_stats(xT)
        hT_own = pers.tile([P, KC, TT], FP8)
        for kc in range(KC):
            t0 = work.tile([P, TT], F32, tag="wf32", bufs=4)
            nc.vector.tensor_sub(t0[:], xT[:, kc, :], m_ps[:, 0:TT])
            t1 = work.tile([P, TT], F32, tag="wf32", bufs=4)
            nc.vector.tensor_tensor(t1[:], t0[:], r_ps[:, 0:TT], op=OP.mult)
            nc.vector.tensor_scalar(
                hT_own[:, kc, :], t1[:], sc1p_msa[:, kc:kc + 1], mod_chunk(0, kc),
                op0=OP.mult, op1=OP.add)
        h_bounce_in_a = dram.tile([HID // 2, TT], FP8)
        h_bounce_in_b = dram.tile([HID // 2, TT], FP8)
        nc.sync.dma_start(
            h_bounce_in_a[:].rearrange("(c p) t -> p c t", p=P), hT_own[:, 0:4, :])
        nc.sync.dma_start(
            h_bounce_in_b[:].rearrange("(c p) t -> p c t", p=P), hT_own[:, 4:8, :])
        h_bounce_out_a = dram.tile([2 * HID, TT], FP8)
        h_bounce_out_b = dram.tile([2 * HID, TT], FP8)
        if sim:
            nc.sync.dma_start(h_bounce_out_a[:][0:HID // 2, :], h_bounce_in_a[:])
            nc.sync.dma_start(h_bounce_out_b[:][0:HID // 2, :], h_bounce_in_b[:])
        else:
            nc.gpsimd.collective_compute(
                "AllGather", OP.bypass, replica_groups=RG4,
                ins=[h_bounce_in_a.opt()], outs=[h_bounce_out_a.opt()])
            nc.gpsimd.collective_compute(
                "AllGather", OP.bypass, replica_groups=RG4,
                ins=[h_bounce_in_b.opt()], outs=[h_bounce_out_b.opt()])
        hT_full = pers.tile([P, 32, TT], FP8)
        for jq in range(4):
            nc.sync.dma_start(
                hT_full[:, KC * jq:KC * jq + 4, :],
                h_bounce_out_a[:][ts(jq, HID // 2), :].rearrange("(c p) t -> p c t", p=P))
            nc.sync.dma_start(
                hT_full[:, KC * jq + 4:KC * jq + 8, :],
                h_bounce_out_b[:][ts(jq, HID // 2), :].rearrange("(c p) t -> p c t", p=P))

        # ---------------- phase 3: qkv (fp8 DoubleRow)
        qT = pers.tile([P, 2, N], BF16)
        kT = pers.tile([P, 2, N], BF16)
        v_aug = pers.tile([P, NBLK, 288], FP8)
        nc.vector.memset(v_aug[:], 0.0)
        nc.vector.memset(
            v_aug[:].rearrange("p b (h e) -> p b h e", h=4)[:, :, :, 64:65], 1.0)

        for blk in range(NBLK):
            jq, tb = blk // 4, blk % 4
            vps = pa()
            for kp in range(4):
                nc.tensor.matmul(
                    vps[:, 0:256],
                    hT_full[:, KC * jq + 2 * kp:KC * jq + 2 * kp + 2, ts(tb, P)],
                    wv_sb[:, kp, :, :],
                    start=(kp == 0), stop=(kp == 3), perf_mode=DR)
            nc.vector.scalar_tensor_tensor(
                v_aug[:, blk, :].rearrange("p (h e) -> p h e", h=4)[:, :, 0:64],
                vps[:, 0:256].rearrange("p (h e) -> p h e", h=4), 1.0 / WS,
                b_v_sb[:].rearrange("p (h e) -> p h e", h=4),
                op0=OP.mult, op1=OP.add)

        for mu in range(4):       # q chunks 0,1; k chunks 2,3
            for jq in range(4):
                qps = pa()
                for kp in range(4):
                    nc.tensor.matmul(
                        qps[:, 0:TT],
                        wqk_sb[:, mu, kp, :, :],
                        hT_full[:, KC * jq + 2 * kp:KC * jq + 2 * kp + 2, :],
                        start=(kp == 0), stop=(kp == 3), perf_mode=DR)
                dst = qT if mu < 2 else kT
                nc.vector.tensor_scalar(
                    dst[:, mu % 2, ts(jq, TT)], qps[:, 0:TT], 1.0 / WS,
                    b_qk_sb[:, mu:mu + 1], op0=OP.mult, op1=OP.add)

        # ---------------- phase 4: attention
        # heads: local h = 2a + o; scores [128 keys, 1024 queries] per
        # (a, o, qspan, blk); bias koff = blk - 8*qspan - s per 128-col
        # sub-block s.
        ctxT = pers.tile([P, 2, N], FP8)
        for a in range(2):
            for qspan in range(2):
                cps = [pc(), pc()]
                for bp in range(NBLK // 2):
                    esb = [
                        work.tile([P, 2, 1024], FP8, tag=f"esb{o}", bufs=2,
                                  name=f"esb{o}")
                        for o in range(2)
                    ]
                    for sub in range(2):
                        blk = 2 * bp + sub
                        koff0 = blk - 8 * qspan
                        for o in range(2):
                            h = 2 * a + o
                            sc = pa()
                            for half in range(2):
                                nc.tensor.matmul(
                                    sc[:, ts(half, TT)],
                                    kT[64 * o:64 * o + 64, a, ts(blk, P)],
                                    qT[64 * o:64 * o + 64, a,
                                       1024 * qspan + 512 * half:
                                       1024 * qspan + 512 * (half + 1)],
                                    start=True, stop=True)
                            bias_arg = 0.0
                            if koff0 >= 9:
                                bias_arg = delta_col[:, h:h + 1]
                            elif koff0 >= -1:
                                for s in range(8):
                                    koff = koff0 - s
                                    if -1 <= koff <= 1:
                                        nc.tensor.matmul(
                                            sc[:, ts(s, P)],
                                            bt_sb[:, 3 * h + koff + 1, :],
                                            ident_b[:],
                                            start=False, stop=True,
                                            skip_group_check=True)
                                    elif koff >= 2:
                                        nc.tensor.matmul(
                                            sc[:, ts(s, P)],
                                            delta_row8[0:1, h, :],
                                            ones_row_b[:],
                                            start=False, stop=True,
                                            skip_group_check=True)
                            nc.scalar.activation(
                                esb[o][:, sub, :], sc[:], AF.Exp,
                                scale=0.125, bias=bias_arg)
                    for o in range(2):
                        h = 2 * a + o
                        for half in range(2):
                            nc.tensor.matmul(
                                cps[o][:, ts(half, TT)],
                                v_aug[:, 2 * bp:2 * bp + 2, 72 * h:72 * h + 72],
                                esb[o][:, :, ts(half, TT)],
                                start=(bp == 0), stop=(bp == NBLK // 2 - 1),
                                perf_mode=DR)
                for o in range(2):
                    recip = work.tile([1, 1024], BF16, tag="recip", bufs=2)
                    with nc.allow_low_precision(reason="softmax denom recip bf16"):
                        nc.vector.reciprocal(recip[:], cps[o][64:65, :])
                    csb = work.tile([64, 1024], BF16, tag="csb", bufs=2)
                    nc.vector.tensor_copy(csb[:], cps[o][0:64, :])
                    bc = pc()
                    for half in range(2):
                        nc.tensor.matmul(
                            bc[0:64, ts(half, TT)], ones64_row[:],
                            recip[0:1, ts(half, TT)], start=True, stop=True)
                    nc.vector.tensor_tensor(
                        ctxT[64 * o:64 * o + 64, a, ts(qspan, 1024)],
                        csb[:], bc[0:64, :], op=OP.mult)

        # ---------------- phase 5: head-sharded out-proj partials + RS(add)
        rs_bounce_in = dram.tile([4 * HID, TT], BF16)
        for tau in range(4):
            for mu in range(KC):
                ops_ = pa()
                nc.tensor.matmul(
                    ops_[:, 0:TT], wout_sb[:, :, ts(mu, P)],
                    ctxT[:, :, ts(tau, TT)],
                    start=True, stop=True, perf_mode=DR)
                po = work.tile([P, TT], BF16, tag="po", bufs=3)
                nc.vector.tensor_copy(po[:], ops_[:, 0:TT])
                nc.sync.dma_start(
                    rs_bounce_in[:][tau * HID + mu * P:tau * HID + (mu + 1) * P, :],
                    po[:])
        rs_bounce_out = dram.tile([HID, TT], BF16)
        if sim:
            nc.sync.dma_start(rs_bounce_out[:], rs_bounce_in[:][0:HID, :])
        else:
            nc.gpsimd.collective_compute(
                "ReduceScatter", OP.add, replica_groups=RG4,
                ins=[rs_bounce_in.opt()], outs=[rs_bounce_out.opt()])
        ao_sb = pers.tile([P, KC, TT], BF16)
        nc.sync.dma_start(
            ao_sb[:], rs_bounce_out[:].rearrange("(c p) t -> p c t", p=P))

        # ---------------- phase 6: residual + LN2
        x2T = pers.tile([P, KC, TT], BF16)
        for mu in range(KC):
            tmp = work.tile([P, TT], BF16, tag="wbf", bufs=4)
            nc.vector.tensor_scalar(
                tmp[:], ao_sb[:, mu, :], s1_msa[:, mu:mu + 1], s2_msa[:, mu:mu + 1],
                op0=OP.mult, op1=OP.add)
            nc.vector.tensor_add(x2T[:, mu, :], tmp[:], xT[:, mu, :])

        m2_ps, r2_ps = ln_stats(x2T)
        h2T = pers.tile([P, KC, TT], BF16)
        for kc in range(KC):
            t0 = work.tile([P, TT], F32, tag="wf32", bufs=4)
            nc.vector.tensor_sub(t0[:], x2T[:, kc, :], m2_ps[:, 0:TT])
            t1 = work.tile([P, TT], F32, tag="wf32", bufs=4)
            nc.vector.tensor_tensor(t1[:], t0[:], r2_ps[:, 0:TT], op=OP.mult)
            nc.vector.tensor_scalar(
                h2T[:, kc, :], t1[:], sc1p_mlp[:, kc:kc + 1], mod_chunk(3, kc),
                op0=OP.mult, op1=OP.add)

        # ---------------- phase 7: MLP (token-sharded, fp8 weights streamed)
        gT = pers.tile([P, MLPH // P, TT], BF16)
        for nu in range(MLPH // P):
            w1 = wst.tile([P, KC, P], BF16, tag="w1", bufs=3)
            nc.sync.dma_start(w1[:], w_mlp1_b.ap()[nu])
            gps = pa()
            for kc in range(KC):
                nc.tensor.matmul(gps[:, 0:TT], w1[:, kc, :],
                                 h2T[:, kc, :],
                                 start=(kc == 0), stop=(kc == KC - 1))
            nc.scalar.activation(
                gT[:, nu, :], gps[:, 0:TT], AF.Gelu_apprx_tanh,
                bias=b_mlp1_sb[:, nu:nu + 1])
        for mu in range(KC):
            mps2 = pa()
            for hf in range(2):
                w2 = wst.tile([P, 16, P], BF16, tag="w2", bufs=2)
                nc.sync.dma_start(w2[:], w_mlp2_b.ap()[mu, :, ts(hf, 16), :])
                for kc in range(16):
                    gkc = 16 * hf + kc
                    nc.tensor.matmul(mps2[:, 0:TT], w2[:, kc, :],
                                     gT[:, gkc, :],
                                     start=(gkc == 0), stop=(gkc == MLPH // P - 1))
            tmp = work.tile([P, TT], BF16, tag="wbf", bufs=4)
            nc.vector.tensor_scalar(
                tmp[:], mps2[:, 0:TT], s1_mlp[:, mu:mu + 1], s2_mlp[:, mu:mu + 1],
                op0=OP.mult, op1=OP.add)
            outT = work.tile([P, TT], F32, tag="wf32", bufs=4)
            nc.vector.tensor_add(outT[:], tmp[:], x2T[:, mu, :])
            for r in range(TT // P):
                tps = pa()
                nc.tensor.transpose(tps[:, 0:P], outT[:, ts(r, P)], ident[:])
                osb = work.tile([P, P], F32, tag="osb", bufs=4)
                nc.vector.tensor_copy(osb[:], tps[:, 0:P])
                nc.sync.dma_start(out_t.ap()[ts(r, P), ts(mu, P)], osb[:])

    nc.compile()
    return nc


# ---------------------------------------------------------------- runner
class SpmdRunner:
    def __init__(self, nc, n_cores, donate=True):
        install_neuronx_cc_hook()
        self.nc = nc
        self.n_cores = n_cores
        partition_name = nc.partition_id_tensor.name if nc.partition_id_tensor else None
        in_names, out_names, out_avals = [], [], []
        for alloc in nc.m.functions[0].allocations:
            if not isinstance(alloc, mybir.MemoryLocationSet):
                continue
            name = alloc.memorylocations[0].name
            if alloc.kind == "ExternalInput":
                if name != partition_name:
                    in_names.append(name)
            elif alloc.kind == "ExternalOutput":
                out_names.append(name)
                out_avals.append(
                    jax.core.ShapedArray(tuple(alloc.tensor_shape), mybir.dt.np(alloc.dtype))
                )
        self.in_names, self.out_names, self.out_avals = in_names, out_names, out_avals
        n_params = len(in_names)
        n_outs = len(out_avals)
        all_in_names = list(in_names) + list(out_names)
        if partition_name is not None:
            all_in_names.append(partition_name)

        def _body(*args):
            operands = list(args)
            if partition_name is not None:
                operands.append(partition_id_tensor())
            return tuple(
                _bass_exec_p.bind(
                    *operands,
                    out_avals=tuple(out_avals),
                    in_names=tuple(all_in_names),
                    out_names=tuple(out_names),
                    lowering_input_output_aliases=(),
                    sim_require_finite=True,
                    sim_require_nnan=True,
                    nc=nc,
                )
            )

        devices = jax.devices()[:n_cores]
        self.mesh = Mesh(np.asarray(devices), ("core",))
        donate_idx = tuple(range(n_params, n_params + n_outs)) if donate else ()
        self.fn = jax.jit(
            shard_map(
                _body,
                mesh=self.mesh,
                in_specs=(PartitionSpec("core"),) * (n_params + n_outs),
                out_specs=(PartitionSpec("core"),) * n_outs,
                check_rep=False,
            ),
            donate_argnums=donate_idx,
            keep_unused=True,
        )
        self.n_params, self.n_outs = n_params, n_outs

    def _concat_inputs(self, in_maps):
        return [
            np.concatenate([np.asarray(in_maps[c][n]) for c in range(self.n_cores)], axis=0)
            for n in self.in_names
        ]

    def run(self, in_maps):
        sharding = jax.sharding.NamedSharding(self.mesh, PartitionSpec("core"))
        concat_in = [
            jax.device_put(x, sharding) for x in self._concat_inputs(in_maps)
        ]
        zeros = [
            jax.device_put(
                np.zeros((self.n_cores * a.shape[0], *a.shape[1:]), a.dtype), sharding)
            for a in self.out_avals
        ]
        outs = self.fn(*concat_in, *zeros)
        return self._split(outs)

    def _split(self, out_arrs):
        return [
            {
                n: np.asarray(out_arrs[i]).reshape(self.n_cores, *self.out_avals[i].shape)[c]
                for i, n in enumerate(self.out_names)
            }
            for c in range(self.n_cores)
        ]

    def bench(self, in_maps, iters=30, warmup=3):
        """Chained repeated execution: output buffers of call i are donated as
        the output operands of call i+1, serializing calls on-device."""
        sharding = jax.sharding.NamedSharding(self.mesh, PartitionSpec("core"))
        concat_in = [jax.device_put(x, sharding) for x in self._concat_inputs(in_maps)]
        outs = tuple(
            jax.device_put(
                np.zeros((self.n_cores * a.shape[0], *a.shape[1:]), a.dtype), sharding)
            for a in self.out_avals
        )
        for _ in range(warmup):
            outs = self.fn(*concat_in, *outs)
        jax.block_until_ready(outs)
        t0 = time.perf_counter()
        for _ in range(iters):
            outs = self.fn(*concat_in, *outs)
        jax.block_until_ready(outs)
        t1 = time.perf_counter()
        return (t1 - t0) / iters, self._split(outs)


_CACHE = {}


def kernel(**inputs):
    """Full-input DiT block on 8 NeuronCores; returns full [B, N, HID] f32."""
    if "nc" not in _CACHE:
        _CACHE["nc"] = build_kernel()
        _CACHE["runner"] = SpmdRunner(_CACHE["nc"], 8)
    maps = make_in_maps(inputs)
    results = _CACHE["runner"].run(maps)
    return assemble_output(results)


# revision 17
# speedup vs baseline: 1.1911x; 1.0107x over previous
"""DiT block Bass kernel for 8 TRN2 NeuronCores.

Core i -> (b = i//4, g = i%4): batch item b; head group 4g..4g+3; token
quarter [512g, 512g+512) of batch b.  Activations are hidden-major
([hidden_chunk=128, tokens]) throughout; PE transposes at entry (x) and
exit (out).  Collectives: AllGather(4) for mod + h, ReduceScatter(4)
for the out-projection partials.

Dtype strategy: residual stream bf16; weights host-quantized (w_qkv,
w_out, w_mlp1, w_mlp2 in fp8e4m3 prescaled by 32; w_ada bf16); all big
GEMMs except QK^T run in fp8 DoubleRow perf mode (2 k-subtiles of 128
per pass).  Relative attention bias: constant for |d| >= 91, so
off-band score tiles get their bias via the exp() bias operand; band
tiles get a PE matmul-add of pretransposed bias tiles into PSUM before
exp.  Softmax is computed without max-subtraction (scores provably
small); denominators come from an appended ones-row in the fp8 V tiles.
"""
import contextlib
import time
import numpy as np
import ml_dtypes
import jax
from jax.sharding import Mesh, PartitionSpec
from jax.experimental.shard_map import shard_map

import concourse.bass as bass
import concourse.mybir as mybir
import concourse.tile as tile
from concourse import bacc
from concourse.bass2jax import _bass_exec_p, install_neuronx_cc_hook, partition_id_tensor

F32 = mybir.dt.float32
BF16 = mybir.dt.bfloat16
FP8 = mybir.dt.float8e4
AF = mybir.ActivationFunctionType
OP = mybir.AluOpType
DR = mybir.MatmulPerfMode.DoubleRow
ts = bass.ts

NPBF16 = ml_dtypes.bfloat16
NPFP8 = ml_dtypes.float8_e4m3

B, N, HID = 2, 2048, 1024
NH, HD = 16, 64
MLPH = 4 * HID
NB, MAXD = 32, 128
P = 128
TT = 512
KC = HID // P          # 8
NBLK = N // P          # 16
WS = 32.0              # host weight prescale for fp8
CXS = 64.0             # ctx prescale for fp8
RG4 = [[0, 1, 2, 3], [4, 5, 6, 7]]


# ---------------------------------------------------------------- host prep
def rel_bucket_np(d):
    nb = NB // 2
    buckets = np.where(d > 0, nb, 0).astype(np.int64)
    rp = np.abs(d)
    max_exact = nb // 2
    is_small = rp < max_exact
    log_ratio = np.log(np.maximum(rp, 1).astype(np.float32) / np.float32(max_exact))
    rpl = max_exact + (
        log_ratio / np.float32(np.log(MAXD / max_exact)) * (nb - max_exact)
    ).astype(np.int32)
    rpl = np.minimum(rpl, nb - 1)
    return buckets + np.where(is_small, rp, rpl)


def make_bias_tables(rel_table, g):
    """Band bias tiles + deltas for local heads 4g..4g+3.

    bt[h, j][p, m] = 8 * badj(128*(j-1) + m - p)   (j = koff+1, koff in -1..1)
    badj(d) = bias(d) - bias_minus;  delta = bias_plus - bias_minus.
    The 8x prescale compensates the 0.125 exp scale (bias added in PSUM
    pre-scale, exp bias arg applied post-scale).
    """
    d = np.arange(-(N - 1), N)
    buck = rel_bucket_np(d)  # index by d + N-1
    bt = np.zeros((4, 3, P, P), np.float32)
    delta = np.zeros((4,), np.float32)
    for hl in range(4):
        hg = 4 * g + hl
        bvec = rel_table[:, hg].astype(np.float32)
        bmin = bvec[NB // 2 - 1]
        delta[hl] = bvec[NB - 1] - bmin
        diag = bvec[buck] - bmin  # badj over d in [-(N-1), N-1]
        p = np.arange(P)[:, None]
        m = np.arange(P)[None, :]
        for j, koff in enumerate((-1, 0, 1)):
            dd = 128 * koff + m - p
            bt[hl, j] = 8.0 * diag[dd + (N - 1)]
    return bt.astype(NPBF16), delta


def make_in_maps(inputs):
    x = np.asarray(inputs["x"], np.float32)
    c = np.asarray(inputs["c"], np.float32)
    w_ada = np.asarray(inputs["w_ada"], np.float32)
    b_ada = np.asarray(inputs["b_ada"], np.float32)
    w_qkv = np.asarray(inputs["w_qkv"], np.float32)
    b_qkv = np.asarray(inputs["b_qkv"], np.float32)
    w_out = np.asarray(inputs["w_out"], np.float32)
    b_out = np.asarray(inputs["b_out"], np.float32)
    rel_table = np.asarray(inputs["rel_table"], np.float32)
    w_mlp1 = np.asarray(inputs["w_mlp1"], np.float32)
    b_mlp1 = np.asarray(inputs["b_mlp1"], np.float32)
    w_mlp2 = np.asarray(inputs["w_mlp2"], np.float32)
    b_mlp2 = np.asarray(inputs["b_mlp2"], np.float32)

    ident = np.eye(P, dtype=np.float32)
    ident_b = np.eye(P, dtype=np.float32).astype(NPBF16)
    ones_col_b = np.ones((P, 1), np.float32).astype(NPBF16)
    ones_row_b = np.ones((1, P), np.float32).astype(NPBF16)
    ones64_row = np.full((1, 64), CXS, np.float32).astype(NPBF16)

    w_mlp1_b = np.ascontiguousarray(
        w_mlp1.reshape(KC, P, MLPH // P, P).transpose(2, 1, 0, 3)
        .astype(NPBF16))                      # [32, P, kc8, P]
    w_mlp2_b = np.ascontiguousarray(
        w_mlp2.reshape(MLPH // P, P, KC, P).transpose(2, 1, 0, 3)
        .astype(NPBF16))                      # [8, P, kc32, P]

    maps = []
    for i in range(8):
        b, g = divmod(i, 4)
        qs, ks, vs = 256 * g, HID + 256 * g, 2 * HID + 256 * g
        w_qk = np.concatenate([w_qkv[:, qs:qs + 256], w_qkv[:, ks:ks + 256]], 1)
        w_v = w_qkv[:, vs:vs + 256]
        b_qk = np.concatenate([b_qkv[qs:qs + 256], b_qkv[ks:ks + 256]])
        bv = b_qkv[vs:vs + 256]

        # [P, mu4, kp4, 2, P]: global k = 128*(2*kp+sub) + p, out chunk mu
        w_qk_q = (w_qk * WS).reshape(4, 2, P, 4, P).transpose(
            2, 3, 0, 1, 4).astype(NPFP8)
        # [P, kp4, 2, 256]
        w_v_q = (w_v * WS).reshape(4, 2, P, 256).transpose(2, 0, 1, 3).astype(NPFP8)
        # [P, 2, HID]: ctx chunk-major (2 chunks of own 256 ctx dims)
        w_out_q = (w_out[256 * g:256 * (g + 1), :] * WS).reshape(
            2, P, HID).transpose(1, 0, 2).astype(NPFP8)

        bt, delta = make_bias_tables(rel_table, g)
        maps.append({
            "x_own": np.ascontiguousarray(x[b, 512 * g:512 * (g + 1), :]),
            "c_own": np.ascontiguousarray(c[b][:, None]),
            "w_ada_s": np.ascontiguousarray(
                w_ada[:, 1536 * g:1536 * (g + 1)].reshape(KC, P, 12, P)
                .transpose(2, 1, 0, 3).astype(NPBF16)),
            "b_ada_s": np.ascontiguousarray(
                b_ada[1536 * g:1536 * (g + 1)].reshape(12, P).T),
            "w_qk_q": np.ascontiguousarray(w_qk_q),
            "w_v_q": np.ascontiguousarray(w_v_q),
            "b_qk_s": np.ascontiguousarray(b_qk.reshape(4, P).T),
            "b_v_bcast": np.ascontiguousarray(
                np.broadcast_to(bv[None, :], (P, 256)).astype(NPBF16)),
            "w_out_q": np.ascontiguousarray(w_out_q),
            "b_out_r": np.ascontiguousarray(b_out.reshape(KC, P).T),
            "w_mlp1_b": w_mlp1_b,
            "b_mlp1_r": np.ascontiguousarray(b_mlp1.reshape(MLPH // P, P).T),
            "w_mlp2_b": w_mlp2_b,
            "b_mlp2_r": np.ascontiguousarray(b_mlp2.reshape(KC, P).T),
            "bt": np.ascontiguousarray(bt.reshape(12, P, P)),
            "delta_row8": np.ascontiguousarray(
                np.broadcast_to((8.0 * delta)[None, :, None], (1, 4, P))
                .astype(NPBF16)),
            "delta_col": np.ascontiguousarray(
                np.broadcast_to(delta[None, :], (P, 4)).astype(np.float32)),
            "ident": ident,
            "ident_b": ident_b,
            "ones_col_b": ones_col_b,
            "ones_row_b": ones_row_b,
            "ones64_row": ones64_row,
        })
    return maps


def assemble_output(results):
    out = np.zeros((B, N, HID), np.float32)
    for i in range(8):
        b, g = divmod(i, 4)
        out[b, 512 * g:512 * (g + 1), :] = results[i]["out"]
    return out


# ---------------------------------------------------------------- builder
def build_kernel(sim=False):
    nc = bacc.Bacc("TRN2", target_bir_lowering=False, debug=False, num_devices=8)

    din = lambda nm, sh, dt=F32: nc.dram_tensor(nm, sh, dt, kind="ExternalInput")
    x_own = din("x_own", [TT, HID])
    c_own = din("c_own", [HID, 1])
    w_ada_s = din("w_ada_s", [12, P, KC, P], BF16)
    b_ada_s = din("b_ada_s", [P, 12])
    w_qk_q = din("w_qk_q", [P, 4, 4, 2, P], FP8)
    w_v_q = din("w_v_q", [P, 4, 2, 256], FP8)
    b_qk_s = din("b_qk_s", [P, 4])
    b_v_bcast = din("b_v_bcast", [P, 256], BF16)
    w_out_q = din("w_out_q", [P, 2, HID], FP8)
    b_out_r = din("b_out_r", [P, KC])
    w_mlp1_b = din("w_mlp1_b", [MLPH // P, P, KC, P], BF16)
    b_mlp1_r = din("b_mlp1_r", [P, MLPH // P])
    w_mlp2_b = din("w_mlp2_b", [KC, P, MLPH // P, P], BF16)
    b_mlp2_r = din("b_mlp2_r", [P, KC])
    bt_in = din("bt", [12, P, P], BF16)
    delta_row8_in = din("delta_row8", [1, 4, P], BF16)
    delta_col_in = din("delta_col", [P, 4])
    ident_in = din("ident", [P, P])
    ident_b_in = din("ident_b", [P, P], BF16)
    ones_col_b_in = din("ones_col_b", [P, 1], BF16)
    ones_row_b_in = din("ones_row_b", [1, P], BF16)
    ones64_row_in = din("ones64_row", [1, 64], BF16)

    out_t = nc.dram_tensor("out", [TT, HID], F32, kind="ExternalOutput")

    with tile.TileContext(nc) as tc, contextlib.ExitStack() as ctx:
        const = ctx.enter_context(tc.tile_pool(name="const", bufs=1))
        pers = ctx.enter_context(tc.tile_pool(name="pers", bufs=1))
        work = ctx.enter_context(tc.tile_pool(name="work", bufs=3))
        wst = ctx.enter_context(tc.tile_pool(name="wst", bufs=2))
        dram = ctx.enter_context(tc.tile_pool(name="dram", bufs=1, space="DRAM"))
        ps_a = ctx.enter_context(tc.tile_pool(name="ps_a", bufs=2, space="PSUM"))
        ps_c = ctx.enter_context(tc.tile_pool(name="ps_c", bufs=2, space="PSUM"))

        def pa():
            return ps_a.tile([P, 1024], F32, tag="A", name="pa")

        def pc():
            return ps_c.tile([72, 1024], F32, tag="C", name="pc")

        # ---------------- constants
        ident = const.tile([P, P], F32)
        nc.sync.dma_start(ident[:], ident_in.ap())
        ident_b = const.tile([P, P], BF16)
        nc.sync.dma_start(ident_b[:], ident_b_in.ap())
        ones_col_b = const.tile([P, 1], BF16)
        nc.sync.dma_start(ones_col_b[:], ones_col_b_in.ap())
        ones_row_b = const.tile([1, P], BF16)
        nc.sync.dma_start(ones_row_b[:], ones_row_b_in.ap())
        ones64_row = const.tile([1, 64], BF16)
        nc.sync.dma_start(ones64_row[:], ones64_row_in.ap())
        b_qk_sb = const.tile([P, 4], F32)
        nc.sync.dma_start(b_qk_sb[:], b_qk_s.ap())
        b_v_sb = const.tile([P, 256], BF16)
        nc.sync.dma_start(b_v_sb[:], b_v_bcast.ap())
        b_out_sb = const.tile([P, KC], F32)
        nc.sync.dma_start(b_out_sb[:], b_out_r.ap())
        b_mlp1_sb = const.tile([P, MLPH // P], F32)
        nc.sync.dma_start(b_mlp1_sb[:], b_mlp1_r.ap())
        b_mlp2_sb = const.tile([P, KC], F32)
        nc.sync.dma_start(b_mlp2_sb[:], b_mlp2_r.ap())
        b_ada_sb = const.tile([P, 12], F32)
        nc.sync.dma_start(b_ada_sb[:], b_ada_s.ap())
        bt_sb = const.tile([P, 12, P], BF16)
        nc.sync.dma_start(bt_sb[:], bt_in.ap().rearrange("j p m -> p j m"))
        delta_row8 = const.tile([1, 4, P], BF16)
        nc.sync.dma_start(delta_row8[:], delta_row8_in.ap())
        delta_col = const.tile([P, 4], F32)
        nc.sync.dma_start(delta_col[:], delta_col_in.ap())
        wqk_sb = const.tile([P, 4, 4, 2, P], FP8)
        nc.sync.dma_start(wqk_sb[:], w_qk_q.ap())
        wv_sb = const.tile([P, 4, 2, 256], FP8)
        nc.sync.dma_start(wv_sb[:], w_v_q.ap())
        wout_sb = const.tile([P, 2, HID], FP8)
        nc.sync.dma_start(wout_sb[:], w_out_q.ap())
        eps_sb = const.tile([1, 1], F32)
        nc.vector.memset(eps_sb[:], 1e-6)

        # ---------------- phase 0: mod shard (this core: w_ada cols 1536g..)
        cT_sb = pers.tile([P, KC], F32)
        nc.sync.dma_start(cT_sb[:], c_own.ap().rearrange("(c p) o -> p (c o)", p=P))
        silu_sb = pers.tile([P, KC], BF16)
        nc.scalar.activation(silu_sb[:], cT_sb[:], AF.Silu)
        mod_sh_sb = pers.tile([P, 12], F32)
        for mu in range(12):
            wa = wst.tile([P, KC, P], BF16, tag="wada")
            nc.sync.dma_start(wa[:], w_ada_s.ap()[mu])
            mps = pa()
            for kc in range(KC):
                nc.tensor.matmul(mps[:, 0:1], wa[:, kc, :], silu_sb[:, kc:kc + 1],
                                 start=(kc == 0), stop=(kc == KC - 1))
            nc.vector.tensor_scalar_add(
                mod_sh_sb[:, mu:mu + 1], mps[:, 0:1], b_ada_sb[:, mu:mu + 1])
        mod_bounce_in = dram.tile([P, 12], F32)
        nc.sync.dma_start(mod_bounce_in[:], mod_sh_sb[:])
        mod_bounce_out = dram.tile([4 * P, 12], F32)
        if sim:
            nc.sync.dma_start(mod_bounce_out[:][0:P, :], mod_bounce_in[:])
        else:
            nc.gpsimd.collective_compute(
                "AllGather", OP.bypass, replica_groups=RG4,
                ins=[mod_bounce_in.opt()], outs=[mod_bounce_out.opt()])
        mod_sb = pers.tile([P, 4, 12], F32)
        nc.sync.dma_start(
            mod_sb[:], mod_bounce_out[:].rearrange("(g p) j -> p g j", p=P))

        def mod_chunk(vec_idx, kc):
            gc = 8 * vec_idx + kc
            return mod_sb[:, gc // 12, gc % 12:gc % 12 + 1]

        sc1p_msa = pers.tile([P, KC], F32)
        sc1p_mlp = pers.tile([P, KC], F32)
        s1_msa = pers.tile([P, KC], F32)
        s2_msa = pers.tile([P, KC], F32)
        s1_mlp = pers.tile([P, KC], F32)
        s2_mlp = pers.tile([P, KC], F32)
        for kc in range(KC):
            nc.vector.tensor_scalar_add(sc1p_msa[:, kc:kc + 1], mod_chunk(1, kc), 1.0)
            nc.vector.tensor_scalar_add(sc1p_mlp[:, kc:kc + 1], mod_chunk(4, kc), 1.0)
            nc.vector.tensor_scalar_mul(
                s1_msa[:, kc:kc + 1], mod_chunk(2, kc), 1.0 / (WS * CXS))
            nc.vector.tensor_tensor(
                s2_msa[:, kc:kc + 1], mod_chunk(2, kc), b_out_sb[:, kc:kc + 1],
                op=OP.mult)
            nc.vector.tensor_scalar_mul(
                s1_mlp[:, kc:kc + 1], mod_chunk(5, kc), 1.0)
            nc.vector.tensor_tensor(
                s2_mlp[:, kc:kc + 1], mod_chunk(5, kc), b_mlp2_sb[:, kc:kc + 1],
                op=OP.mult)

        # ---------------- phase 1: xT (bf16) via PE transpose
        xT = pers.tile([P, KC, TT], BF16)
        for r in range(TT // P):
            x_sb = work.tile([P, HID], F32, tag="xrow", bufs=2)
            nc.sync.dma_start(x_sb[:], x_own.ap()[ts(r, P), :])
            for half in range(2):
                tps = pa()
                for k4 in range(4):
                    nc.tensor.transpose(tps[:, ts(k4, P)],
                                        x_sb[:, ts(4 * half + k4, P)], ident[:])
                nc.vector.tensor_copy(
                    xT[:, 4 * half:4 * half + 4, ts(r, P)],
                    tps[:, 0:512].rearrange("p (c t) -> p c t", c=4))

        def ln_stats(src):
            """src: [P, KC, TT] bf16 -> (m_bc, r_bc) [P, TT] f32 PSUM tiles."""
            sum_c = pc()
            for kc in range(KC):
                nc.tensor.matmul(sum_c[0:1, 0:TT], ones_col_b[:], src[:, kc, :],
                                 start=(kc == 0), stop=(kc == KC - 1))
            sumsq_c = pc()
            for kc in range(KC):
                sq = work.tile([P, TT], BF16, tag="sq", bufs=3)
                nc.gpsimd.tensor_tensor(sq[:], src[:, kc, :], src[:, kc, :],
                                        op=OP.mult)
                nc.tensor.matmul(sumsq_c[0:1, 0:TT], ones_col_b[:], sq[:],
                                 start=(kc == 0), stop=(kc == KC - 1))
            m_row = work.tile([1, TT], BF16, tag="rowtmp", bufs=4)
            nc.vector.tensor_scalar_mul(m_row[:], sum_c[0:1, 0:TT], 1.0 / HID)
            msq = work.tile([1, TT], BF16, tag="rowtmp", bufs=4)
            nc.vector.tensor_tensor(msq[:], m_row[:], m_row[:], op=OP.mult)
            var_row = work.tile([1, TT], F32, tag="rowtmp", bufs=4)
            nc.vector.scalar_tensor_tensor(
                var_row[:], sumsq_c[0:1, 0:TT], 1.0 / HID, msq[:],
                op0=OP.mult, op1=OP.subtract)
            sd_row = work.tile([1, TT], F32, tag="rowtmp", bufs=4)
            nc.scalar.activation(sd_row[:], var_row[:], AF.Sqrt, bias=eps_sb[:])
            r_row = work.tile([1, TT], BF16, tag="rowtmp", bufs=4)
            with nc.allow_low_precision(reason="rstd in bf16 is plenty for LN"):
                nc.vector.reciprocal(r_row[:], sd_row[:])
            m_ps = pa()
            nc.tensor.matmul(m_ps[:, 0:TT], ones_row_b[:], m_row[:],
                             start=True, stop=True)
            r_ps = pa()
            nc.tensor.matmul(r_ps[:, 0:TT], ones_row_b[:], r_row[:],
                             start=True, stop=True)
            return m_ps, r_ps

        # ---------------- phase 2: hT own (fp8) + AllGather
        m_ps, r_ps = ln_stats(xT)
        hT_own = pers.tile([P, KC, TT], FP8)
        for kc in range(KC):
            t0 = work.tile([P, TT], F32, tag="wf32", bufs=4)
            nc.vector.tensor_sub(t0[:], xT[:, kc, :], m_ps[:, 0:TT])
            t1 = work.tile([P, TT], F32, tag="wf32", bufs=4)
            nc.vector.tensor_tensor(t1[:], t0[:], r_ps[:, 0:TT], op=OP.mult)
            nc.vector.tensor_scalar(
                hT_own[:, kc, :], t1[:], sc1p_msa[:, kc:kc + 1], mod_chunk(0, kc),
                op0=OP.mult, op1=OP.add)
        h_bounce_in_a = dram.tile([HID // 2, TT], FP8)
        h_bounce_in_b = dram.tile([HID // 2, TT], FP8)
        nc.sync.dma_start(
            h_bounce_in_a[:].rearrange("(c p) t -> p c t", p=P), hT_own[:, 0:4, :])
        nc.sync.dma_start(
            h_bounce_in_b[:].rearrange("(c p) t -> p c t", p=P), hT_own[:, 4:8, :])
        h_bounce_out_a = dram.tile([2 * HID, TT], FP8)
        h_bounce_out_b = dram.tile([2 * HID, TT], FP8)
        if sim:
            nc.sync.dma_start(h_bounce_out_a[:][0:HID // 2, :], h_bounce_in_a[:])
            nc.sync.dma_start(h_bounce_out_b[:][0:HID // 2, :], h_bounce_in_b[:])
        else:
            nc.gpsimd.collective_compute(
                "AllGather", OP.bypass, replica_groups=RG4,
                ins=[h_bounce_in_a.opt()], outs=[h_bounce_out_a.opt()])
            nc.gpsimd.collective_compute(
                "AllGather", OP.bypass, replica_groups=RG4,
                ins=[h_bounce_in_b.opt()], outs=[h_bounce_out_b.opt()])
        hT_full = pers.tile([P, 32, TT], FP8)
        for jq in range(4):
            nc.sync.dma_start(
                hT_full[:, KC * jq:KC * jq + 4, :],
                h_bounce_out_a[:][ts(jq, HID // 2), :].rearrange("(c p) t -> p c t", p=P))
            nc.sync.dma_start(
                hT_full[:, KC * jq + 4:KC * jq + 8, :],
                h_bounce_out_b[:][ts(jq, HID // 2), :].rearrange("(c p) t -> p c t", p=P))

        # ---------------- phase 3: qkv (fp8 DoubleRow)
        qT = pers.tile([P, 2, N], BF16)
        kT = pers.tile([P, 2, N], BF16)
        v_aug = pers.tile([P, NBLK, 288], FP8)
        nc.vector.memset(v_aug[:], 0.0)
        nc.vector.memset(
            v_aug[:].rearrange("p b (h e) -> p b h e", h=4)[:, :, :, 64:65], 1.0)

        for blk in range(NBLK):
            jq, tb = blk // 4, blk % 4
            vps = pa()
            for kp in range(4):
                nc.tensor.matmul(
                    vps[:, 0:256],
                    hT_full[:, KC * jq + 2 * kp:KC * jq + 2 * kp + 2, ts(tb, P)],
                    wv_sb[:, kp, :, :],
                    start=(kp == 0), stop=(kp == 3), perf_mode=DR)
            nc.vector.scalar_tensor_tensor(
                v_aug[:, blk, :].rearrange("p (h e) -> p h e", h=4)[:, :, 0:64],
                vps[:, 0:256].rearrange("p (h e) -> p h e", h=4), 1.0 / WS,
                b_v_sb[:].rearrange("p (h e) -> p h e", h=4),
                op0=OP.mult, op1=OP.add)

        for mu in range(4):       # q chunks 0,1; k chunks 2,3
            for jq in range(4):
                qps = pa()
                for kp in range(4):
                    nc.tensor.matmul(
                        qps[:, 0:TT],
                        wqk_sb[:, mu, kp, :, :],
                        hT_full[:, KC * jq + 2 * kp:KC * jq + 2 * kp + 2, :],
                        start=(kp == 0), stop=(kp == 3), perf_mode=DR)
                dst = qT if mu < 2 else kT
                nc.vector.tensor_scalar(
                    dst[:, mu % 2, ts(jq, TT)], qps[:, 0:TT], 1.0 / WS,
                    b_qk_sb[:, mu:mu + 1], op0=OP.mult, op1=OP.add)

        # ---------------- phase 4: attention
        # heads: local h = 2a + o; scores [128 keys, 1024 queries] per
        # (a, o, qspan, blk); bias koff = blk - 8*qspan - s per 128-col
        # sub-block s.
        ctxT = pers.tile([P, 2, N], FP8)
        rs_bounce_in = dram.tile([4 * HID, TT], BF16)
        for qspan in range(2):
            for a in range(2):
                cps = [pc(), pc()]
                for bp in range(NBLK // 2):
                    esb = [
                        work.tile([P, 2, 1024], FP8, tag=f"esb{o}", bufs=2,
                                  name=f"esb{o}")
                        for o in range(2)
                    ]
                    for sub in range(2):
                        blk = 2 * bp + sub
                        koff0 = blk - 8 * qspan
                        for o in range(2):
                            h = 2 * a + o
                            sc = pa()
                            for half in range(2):
                                nc.tensor.matmul(
                                    sc[:, ts(half, TT)],
                                    kT[64 * o:64 * o + 64, a, ts(blk, P)],
                                    qT[64 * o:64 * o + 64, a,
                                       1024 * qspan + 512 * half:
                                       1024 * qspan + 512 * (half + 1)],
                                    start=True, stop=True)
                            bias_arg = 0.0
                            if koff0 >= 9:
                                bias_arg = delta_col[:, h:h + 1]
                            elif koff0 >= -1:
                                for s in range(8):
                                    koff = koff0 - s
                                    if -1 <= koff <= 1:
                                        nc.tensor.matmul(
                                            sc[:, ts(s, P)],
                                            bt_sb[:, 3 * h + koff + 1, :],
                                            ident_b[:],
                                            start=False, stop=True,
                                            skip_group_check=True)
                                    elif koff >= 2:
                                        nc.tensor.matmul(
                                            sc[:, ts(s, P)],
                                            delta_row8[0:1, h, :],
                                            ones_row_b[:],
                                            start=False, stop=True,
                                            skip_group_check=True)
                            nc.scalar.activation(
                                esb[o][:, sub, :], sc[:], AF.Exp,
                                scale=0.125, bias=bias_arg)
                    for o in range(2):
                        h = 2 * a + o
                        for half in range(2):
                            nc.tensor.matmul(
                                cps[o][:, ts(half, TT)],
                                v_aug[:, 2 * bp:2 * bp + 2, 72 * h:72 * h + 72],
                                esb[o][:, :, ts(half, TT)],
                                start=(bp == 0), stop=(bp == NBLK // 2 - 1),
                                perf_mode=DR)
                for o in range(2):
                    recip = work.tile([1, 1024], BF16, tag="recip", bufs=2)
                    with nc.allow_low_precision(reason="softmax denom recip bf16"):
                        nc.vector.reciprocal(recip[:], cps[o][64:65, :])
                    csb = work.tile([64, 1024], BF16, tag="csb", bufs=2)
                    nc.vector.tensor_copy(csb[:], cps[o][0:64, :])
                    bc = pc()
                    for half in range(2):
                        nc.tensor.matmul(
                            bc[0:64, ts(half, TT)], ones64_row[:],
                            recip[0:1, ts(half, TT)], start=True, stop=True)
                    nc.vector.tensor_tensor(
                        ctxT[64 * o:64 * o + 64, a, ts(qspan, 1024)],
                        csb[:], bc[0:64, :], op=OP.mult)
            # out-proj partials for this qspan's two token quarters
            for tau in (2 * qspan, 2 * qspan + 1):
                for mu in range(KC):
                    ops_ = pa()
                    nc.tensor.matmul(
                        ops_[:, 0:TT], wout_sb[:, :, ts(mu, P)],
                        ctxT[:, :, ts(tau, TT)],
                        start=True, stop=True, perf_mode=DR)
                    po = work.tile([P, TT], BF16, tag="po", bufs=3)
                    nc.vector.tensor_copy(po[:], ops_[:, 0:TT])
                    nc.sync.dma_start(
                        rs_bounce_in[:][tau * HID + mu * P:tau * HID + (mu + 1) * P, :],
                        po[:])

        # ---------------- phase 5: ReduceScatter(add)
        rs_bounce_out = dram.tile([HID, TT], BF16)
        if sim:
            nc.sync.dma_start(rs_bounce_out[:], rs_bounce_in[:][0:HID, :])
        else:
            nc.gpsimd.collective_compute(
                "ReduceScatter", OP.add, replica_groups=RG4,
                ins=[rs_bounce_in.opt()], outs=[rs_bounce_out.opt()])
        ao_sb = pers.tile([P, KC, TT], BF16)
        nc.sync.dma_start(
            ao_sb[:], rs_bounce_out[:].rearrange("(c p) t -> p c t", p=P))

        # ---------------- phase 6: residual + LN2
        x2T = pers.tile([P, KC, TT], BF16)
        for mu in range(KC):
            tmp = work.tile([P, TT], BF16, tag="wbf", bufs=4)
            nc.vector.tensor_scalar(
                tmp[:], ao_sb[:, mu, :], s1_msa[:, mu:mu + 1], s2_msa[:, mu:mu + 1],
                op0=OP.mult, op1=OP.add)
            nc.vector.tensor_add(x2T[:, mu, :], tmp[:], xT[:, mu, :])

        m2_ps, r2_ps = ln_stats(x2T)
        h2T = pers.tile([P, KC, TT], BF16)
        for kc in range(KC):
            t0 = work.tile([P, TT], F32, tag="wf32", bufs=4)
            nc.vector.tensor_sub(t0[:], x2T[:, kc, :], m2_ps[:, 0:TT])
            t1 = work.tile([P, TT], F32, tag="wf32", bufs=4)
            nc.vector.tensor_tensor(t1[:], t0[:], r2_ps[:, 0:TT], op=OP.mult)
            nc.vector.tensor_scalar(
                h2T[:, kc, :], t1[:], sc1p_mlp[:, kc:kc + 1], mod_chunk(3, kc),
                op0=OP.mult, op1=OP.add)

        # ---------------- phase 7: MLP (token-sharded, fp8 weights streamed)
        gT = pers.tile([P, MLPH // P, TT], BF16)
        for nu in range(MLPH // P):
            w1 = wst.tile([P, KC, P], BF16, tag="w1", bufs=3)
            nc.sync.dma_start(w1[:], w_mlp1_b.ap()[nu])
            gps = pa()
            for kc in range(KC):
                nc.tensor.matmul(gps[:, 0:TT], w1[:, kc, :],
                                 h2T[:, kc, :],
                                 start=(kc == 0), stop=(kc == KC - 1))
            nc.scalar.activation(
                gT[:, nu, :], gps[:, 0:TT], AF.Gelu_apprx_tanh,
                bias=b_mlp1_sb[:, nu:nu + 1])
        for mu in range(KC):
            mps2 = pa()
            for hf in range(2):
                w2 = wst.tile([P, 16, P], BF16, tag="w2", bufs=2)
                nc.sync.dma_start(w2[:], w_mlp2_b.ap()[mu, :, ts(hf, 16), :])
                for kc in range(16):
                    gkc = 16 * hf + kc
                    nc.tensor.matmul(mps2[:, 0:TT], w2[:, kc, :],
                                     gT[:, gkc, :],
                                     start=(gkc == 0), stop=(gkc == MLPH // P - 1))
            tmp = work.tile([P, TT], BF16, tag="wbf", bufs=4)
            nc.vector.tensor_scalar(
                tmp[:], mps2[:, 0:TT], s1_mlp[:, mu:mu + 1], s2_mlp[:, mu:mu + 1],
                op0=OP.mult, op1=OP.add)
            outT = work.tile([P, TT], F32, tag="wf32", bufs=4)
            nc.vector.tensor_add(outT[:], tmp[:], x2T[:, mu, :])
            tps = pa()
            for r in range(TT // P):
                nc.tensor.transpose(tps[:, ts(r, P)], outT[:, ts(r, P)], ident[:])
            osb = work.tile([P, 512], F32, tag="osb", bufs=2)
            nc.vector.tensor_copy(osb[:], tps[:, 0:512])
            nc.sync.dma_start(
                out_t.ap()[:, ts(mu, P)].rearrange("(r p) m -> p r m", p=P),
                osb[:].rearrange("p (r m) -> p r m", r=4))

    nc.compile()
    return nc


# ---------------------------------------------------------------- runner
class SpmdRunner:
    def __init__(self, nc, n_cores, donate=True):
        install_neuronx_cc_hook()
        self.nc = nc
        self.n_cores = n_cores
        partition_name = nc.partition_id_tensor.name if nc.partition_id_tensor else None
        in_names, out_names, out_avals = [], [], []
        for alloc in nc.m.functions[0].allocations:
            if not isinstance(alloc, mybir.MemoryLocationSet):
                continue
            name = alloc.memorylocations[0].name
            if alloc.kind == "ExternalInput":
                if name != partition_name:
                    in_names.append(name)
            elif alloc.kind == "ExternalOutput":
                out_names.append(name)
                out_avals.append(
                    jax.core.ShapedArray(tuple(alloc.tensor_shape), mybir.dt.np(alloc.dtype))
                )
        self.in_names, self.out_names, self.out_avals = in_names, out_names, out_avals
        n_params = len(in_names)
        n_outs = len(out_avals)
        all_in_names = list(in_names) + list(out_names)
        if partition_name is not None:
            all_in_names.append(partition_name)

        def _body(*args):
            operands = list(args)
            if partition_name is not None:
                operands.append(partition_id_tensor())
            return tuple(
                _bass_exec_p.bind(
                    *operands,
                    out_avals=tuple(out_avals),
                    in_names=tuple(all_in_names),
                    out_names=tuple(out_names),
                    lowering_input_output_aliases=(),
                    sim_require_finite=True,
                    sim_require_nnan=True,
                    nc=nc,
                )
            )

        devices = jax.devices()[:n_cores]
        self.mesh = Mesh(np.asarray(devices), ("core",))
        donate_idx = tuple(range(n_params, n_params + n_outs)) if donate else ()
        self.fn = jax.jit(
            shard_map(
                _body,
                mesh=self.mesh,
                in_specs=(PartitionSpec("core"),) * (n_params + n_outs),
                out_specs=(PartitionSpec("core"),) * n_outs,
                check_rep=False,
            ),
            donate_argnums=donate_idx,
            keep_unused=True,
        )
        self.n_params, self.n_outs = n_params, n_outs

    def _concat_inputs(self, in_maps):
        return [
            np.concatenate([np.asarray(in_maps[c][n]) for c in range(self.n_cores)], axis=0)
            for n in self.in_names
        ]

    def run(self, in_maps):
        sharding = jax.sharding.NamedSharding(self.mesh, PartitionSpec("core"))
        concat_in = [
            jax.device_put(x, sharding) for x in self._concat_inputs(in_maps)
        ]
        zeros = [
            jax.device_put(
                np.zeros((self.n_cores * a.shape[0], *a.shape[1:]), a.dtype), sharding)
            for a in self.out_avals
        ]
        outs = self.fn(*concat_in, *zeros)
        return self._split(outs)

    def _split(self, out_arrs):
        return [
            {
                n: np.asarray(out_arrs[i]).reshape(self.n_cores, *self.out_avals[i].shape)[c]
                for i, n in enumerate(self.out_names)
            }
            for c in range(self.n_cores)
        ]

    def bench(self, in_maps, iters=30, warmup=3):
        """Chained repeated execution: output buffers of call i are donated as
        the output operands of call i+1, serializing calls on-device."""
        sharding = jax.sharding.NamedSharding(self.mesh, PartitionSpec("core"))
        concat_in = [jax.device_put(x, sharding) for x in self._concat_inputs(in_maps)]
        outs = tuple(
            jax.device_put(
                np.zeros((self.n_cores * a.shape[0], *a.shape[1:]), a.dtype), sharding)
            for a in self.out_avals
        )
        for _ in range(warmup):
            outs = self.fn(*concat_in, *outs)
        jax.block_until_ready(outs)
        t0 = time.perf_counter()
        for _ in range(iters):
            outs = self.fn(*concat_in, *outs)
        jax.block_until_ready(outs)
        t1 = time.perf_counter()
        return (t1 - t0) / iters, self._split(outs)


_CACHE = {}


def kernel(**inputs):
    """Full-input DiT block on 8 NeuronCores; returns full [B, N, HID] f32."""
    if "nc" not in _CACHE:
        _CACHE["nc"] = build_kernel()
        _CACHE["runner"] = SpmdRunner(_CACHE["nc"], 8)
    maps = make_in_maps(inputs)
    results = _CACHE["runner"].run(maps)
    return assemble_output(results)


# revision 18
# speedup vs baseline: 1.1958x; 1.0039x over previous
"""DiT block Bass kernel for 8 TRN2 NeuronCores.

Core i -> (b = i//4, g = i%4): batch item b; head group 4g..4g+3; token
quarter [512g, 512g+512) of batch b.  Activations are hidden-major
([hidden_chunk=128, tokens]) throughout; PE transposes at entry (x) and
exit (out).  Collectives: AllGather(4) for mod + h, ReduceScatter(4)
for the out-projection partials.

Dtype strategy: residual stream bf16; weights host-quantized (w_qkv,
w_out, w_mlp1, w_mlp2 in fp8e4m3 prescaled by 32; w_ada bf16); all big
GEMMs except QK^T run in fp8 DoubleRow perf mode (2 k-subtiles of 128
per pass).  Relative attention bias: constant for |d| >= 91, so
off-band score tiles get their bias via the exp() bias operand; band
tiles get a PE matmul-add of pretransposed bias tiles into PSUM before
exp.  Softmax is computed without max-subtraction (scores provably
small); denominators come from an appended ones-row in the fp8 V tiles.
"""
import contextlib
import time
import numpy as np
import ml_dtypes
import jax
from jax.sharding import Mesh, PartitionSpec
from jax.experimental.shard_map import shard_map

import concourse.bass as bass
import concourse.mybir as mybir
import concourse.tile as tile
from concourse import bacc
from concourse.bass2jax import _bass_exec_p, install_neuronx_cc_hook, partition_id_tensor

F32 = mybir.dt.float32
BF16 = mybir.dt.bfloat16
FP8 = mybir.dt.float8e4
AF = mybir.ActivationFunctionType
OP = mybir.AluOpType
DR = mybir.MatmulPerfMode.DoubleRow
ts = bass.ts

NPBF16 = ml_dtypes.bfloat16
NPFP8 = ml_dtypes.float8_e4m3

B, N, HID = 2, 2048, 1024
NH, HD = 16, 64
MLPH = 4 * HID
NB, MAXD = 32, 128
P = 128
TT = 512
KC = HID // P          # 8
NBLK = N // P          # 16
WS = 32.0              # host weight prescale for fp8
CXS = 64.0             # ctx prescale for fp8
RG4 = [[0, 1, 2, 3], [4, 5, 6, 7]]


# ---------------------------------------------------------------- host prep
def rel_bucket_np(d):
    nb = NB // 2
    buckets = np.where(d > 0, nb, 0).astype(np.int64)
    rp = np.abs(d)
    max_exact = nb // 2
    is_small = rp < max_exact
    log_ratio = np.log(np.maximum(rp, 1).astype(np.float32) / np.float32(max_exact))
    rpl = max_exact + (
        log_ratio / np.float32(np.log(MAXD / max_exact)) * (nb - max_exact)
    ).astype(np.int32)
    rpl = np.minimum(rpl, nb - 1)
    return buckets + np.where(is_small, rp, rpl)


def make_bias_tables(rel_table, g):
    """Band bias tiles + deltas for local heads 4g..4g+3.

    bt[h, j][p, m] = 8 * badj(128*(j-1) + m - p)   (j = koff+1, koff in -1..1)
    badj(d) = bias(d) - bias_minus;  delta = bias_plus - bias_minus.
    The 8x prescale compensates the 0.125 exp scale (bias added in PSUM
    pre-scale, exp bias arg applied post-scale).
    """
    d = np.arange(-(N - 1), N)
    buck = rel_bucket_np(d)  # index by d + N-1
    bt = np.zeros((4, 3, P, P), np.float32)
    delta = np.zeros((4,), np.float32)
    for hl in range(4):
        hg = 4 * g + hl
        bvec = rel_table[:, hg].astype(np.float32)
        bmin = bvec[NB // 2 - 1]
        delta[hl] = bvec[NB - 1] - bmin
        diag = bvec[buck] - bmin  # badj over d in [-(N-1), N-1]
        p = np.arange(P)[:, None]
        m = np.arange(P)[None, :]
        for j, koff in enumerate((-1, 0, 1)):
            dd = 128 * koff + m - p
            bt[hl, j] = 8.0 * diag[dd + (N - 1)]
    return bt.astype(NPBF16), delta


def make_in_maps(inputs):
    x = np.asarray(inputs["x"], np.float32)
    c = np.asarray(inputs["c"], np.float32)
    w_ada = np.asarray(inputs["w_ada"], np.float32)
    b_ada = np.asarray(inputs["b_ada"], np.float32)
    w_qkv = np.asarray(inputs["w_qkv"], np.float32)
    b_qkv = np.asarray(inputs["b_qkv"], np.float32)
    w_out = np.asarray(inputs["w_out"], np.float32)
    b_out = np.asarray(inputs["b_out"], np.float32)
    rel_table = np.asarray(inputs["rel_table"], np.float32)
    w_mlp1 = np.asarray(inputs["w_mlp1"], np.float32)
    b_mlp1 = np.asarray(inputs["b_mlp1"], np.float32)
    w_mlp2 = np.asarray(inputs["w_mlp2"], np.float32)
    b_mlp2 = np.asarray(inputs["b_mlp2"], np.float32)

    ident = np.eye(P, dtype=np.float32)
    ident_b = np.eye(P, dtype=np.float32).astype(NPBF16)
    ones_col_b = np.ones((P, 1), np.float32).astype(NPBF16)
    ones_row_b = np.ones((1, P), np.float32).astype(NPBF16)
    ones64_row = np.full((1, 64), CXS, np.float32).astype(NPBF16)

    w_mlp1_b = np.ascontiguousarray(
        w_mlp1.reshape(KC, P, MLPH // P, P).transpose(2, 1, 0, 3)
        .astype(NPBF16))                      # [32, P, kc8, P]
    w_mlp2_b = np.ascontiguousarray(
        w_mlp2.reshape(MLPH // P, P, KC, P).transpose(2, 1, 0, 3)
        .astype(NPBF16))                      # [8, P, kc32, P]

    maps = []
    for i in range(8):
        b, g = divmod(i, 4)
        qs, ks, vs = 256 * g, HID + 256 * g, 2 * HID + 256 * g
        w_qk = np.concatenate([w_qkv[:, qs:qs + 256], w_qkv[:, ks:ks + 256]], 1)
        w_v = w_qkv[:, vs:vs + 256]
        b_qk = np.concatenate([b_qkv[qs:qs + 256], b_qkv[ks:ks + 256]])
        bv = b_qkv[vs:vs + 256]

        # [P, mu4, kp4, 2, P]: global k = 128*(2*kp+sub) + p, out chunk mu
        w_qk_q = (w_qk * WS).reshape(4, 2, P, 4, P).transpose(
            2, 3, 0, 1, 4).astype(NPFP8)
        # [P, kp4, 2, 256]
        w_v_q = (w_v * WS).reshape(4, 2, P, 256).transpose(2, 0, 1, 3).astype(NPFP8)
        # [P, 2, HID]: ctx chunk-major (2 chunks of own 256 ctx dims)
        w_out_q = (w_out[256 * g:256 * (g + 1), :] * WS).reshape(
            2, P, HID).transpose(1, 0, 2).astype(NPFP8)

        bt, delta = make_bias_tables(rel_table, g)
        maps.append({
            "x_own": np.ascontiguousarray(x[b, 512 * g:512 * (g + 1), :]),
            "c_own": np.ascontiguousarray(c[b][:, None]),
            "w_ada_s": np.ascontiguousarray(
                w_ada[:, 1536 * g:1536 * (g + 1)].reshape(KC, P, 12, P)
                .transpose(2, 1, 0, 3).astype(NPBF16)),
            "b_ada_s": np.ascontiguousarray(
                b_ada[1536 * g:1536 * (g + 1)].reshape(12, P).T),
            "w_qk_q": np.ascontiguousarray(w_qk_q),
            "w_v_q": np.ascontiguousarray(w_v_q),
            "b_qk_s": np.ascontiguousarray(b_qk.reshape(4, P).T),
            "b_v_bcast": np.ascontiguousarray(
                np.broadcast_to(bv[None, :], (P, 256)).astype(NPBF16)),
            "w_out_q": np.ascontiguousarray(w_out_q),
            "b_out_r": np.ascontiguousarray(b_out.reshape(KC, P).T),
            "w_mlp1_b": w_mlp1_b,
            "b_mlp1_r": np.ascontiguousarray(b_mlp1.reshape(MLPH // P, P).T),
            "w_mlp2_b": w_mlp2_b,
            "b_mlp2_r": np.ascontiguousarray(b_mlp2.reshape(KC, P).T),
            "bt": np.ascontiguousarray(bt.reshape(12, P, P)),
            "delta_row8": np.ascontiguousarray(
                np.broadcast_to((8.0 * delta)[None, :, None], (1, 4, P))
                .astype(NPBF16)),
            "delta_col": np.ascontiguousarray(
                np.broadcast_to(delta[None, :], (P, 4)).astype(np.float32)),
            "ident": ident,
            "ident_b": ident_b,
            "ones_col_b": ones_col_b,
            "ones_row_b": ones_row_b,
            "ones64_row": ones64_row,
        })
    return maps


def assemble_output(results):
    out = np.zeros((B, N, HID), np.float32)
    for i in range(8):
        b, g = divmod(i, 4)
        out[b, 512 * g:512 * (g + 1), :] = results[i]["out"]
    return out


# ---------------------------------------------------------------- builder
def build_kernel(sim=False):
    nc = bacc.Bacc("TRN2", target_bir_lowering=False, debug=False, num_devices=8)

    din = lambda nm, sh, dt=F32: nc.dram_tensor(nm, sh, dt, kind="ExternalInput")
    x_own = din("x_own", [TT, HID])
    c_own = din("c_own", [HID, 1])
    w_ada_s = din("w_ada_s", [12, P, KC, P], BF16)
    b_ada_s = din("b_ada_s", [P, 12])
    w_qk_q = din("w_qk_q", [P, 4, 4, 2, P], FP8)
    w_v_q = din("w_v_q", [P, 4, 2, 256], FP8)
    b_qk_s = din("b_qk_s", [P, 4])
    b_v_bcast = din("b_v_bcast", [P, 256], BF16)
    w_out_q = din("w_out_q", [P, 2, HID], FP8)
    b_out_r = din("b_out_r", [P, KC])
    w_mlp1_b = din("w_mlp1_b", [MLPH // P, P, KC, P], BF16)
    b_mlp1_r = din("b_mlp1_r", [P, MLPH // P])
    w_mlp2_b = din("w_mlp2_b", [KC, P, MLPH // P, P], BF16)
    b_mlp2_r = din("b_mlp2_r", [P, KC])
    bt_in = din("bt", [12, P, P], BF16)
    delta_row8_in = din("delta_row8", [1, 4, P], BF16)
    delta_col_in = din("delta_col", [P, 4])
    ident_in = din("ident", [P, P])
    ident_b_in = din("ident_b", [P, P], BF16)
    ones_col_b_in = din("ones_col_b", [P, 1], BF16)
    ones_row_b_in = din("ones_row_b", [1, P], BF16)
    ones64_row_in = din("ones64_row", [1, 64], BF16)

    out_t = nc.dram_tensor("out", [TT, HID], F32, kind="ExternalOutput")

    with tile.TileContext(nc) as tc, contextlib.ExitStack() as ctx:
        const = ctx.enter_context(tc.tile_pool(name="const", bufs=1))
        pers = ctx.enter_context(tc.tile_pool(name="pers", bufs=1))
        work = ctx.enter_context(tc.tile_pool(name="work", bufs=3))
        wst = ctx.enter_context(tc.tile_pool(name="wst", bufs=2))
        dram = ctx.enter_context(tc.tile_pool(name="dram", bufs=1, space="DRAM"))
        ps_a = ctx.enter_context(tc.tile_pool(name="ps_a", bufs=2, space="PSUM"))
        ps_c = ctx.enter_context(tc.tile_pool(name="ps_c", bufs=2, space="PSUM"))

        def pa():
            return ps_a.tile([P, 1024], F32, tag="A", name="pa")

        def pc():
            return ps_c.tile([72, 1024], F32, tag="C", name="pc")

        # ---------------- constants
        ident = const.tile([P, P], F32)
        nc.sync.dma_start(ident[:], ident_in.ap())
        ident_b = const.tile([P, P], BF16)
        nc.sync.dma_start(ident_b[:], ident_b_in.ap())
        ones_col_b = const.tile([P, 1], BF16)
        nc.sync.dma_start(ones_col_b[:], ones_col_b_in.ap())
        ones_row_b = const.tile([1, P], BF16)
        nc.sync.dma_start(ones_row_b[:], ones_row_b_in.ap())
        ones64_row = const.tile([1, 64], BF16)
        nc.sync.dma_start(ones64_row[:], ones64_row_in.ap())
        b_qk_sb = const.tile([P, 4], F32)
        nc.sync.dma_start(b_qk_sb[:], b_qk_s.ap())
        b_v_sb = const.tile([P, 256], BF16)
        nc.sync.dma_start(b_v_sb[:], b_v_bcast.ap())
        b_out_sb = const.tile([P, KC], F32)
        nc.sync.dma_start(b_out_sb[:], b_out_r.ap())
        b_mlp1_sb = const.tile([P, MLPH // P], F32)
        nc.sync.dma_start(b_mlp1_sb[:], b_mlp1_r.ap())
        b_mlp2_sb = const.tile([P, KC], F32)
        nc.sync.dma_start(b_mlp2_sb[:], b_mlp2_r.ap())
        b_ada_sb = const.tile([P, 12], F32)
        nc.sync.dma_start(b_ada_sb[:], b_ada_s.ap())
        bt_sb = const.tile([P, 12, P], BF16)
        delta_row8 = const.tile([1, 4, P], BF16)
        nc.sync.dma_start(delta_row8[:], delta_row8_in.ap())
        delta_col = const.tile([P, 4], F32)
        nc.sync.dma_start(delta_col[:], delta_col_in.ap())
        wqk_sb = const.tile([P, 4, 4, 2, P], FP8)
        wv_sb = const.tile([P, 4, 2, 256], FP8)
        wout_sb = const.tile([P, 2, HID], FP8)
        eps_sb = const.tile([1, 1], F32)
        nc.vector.memset(eps_sb[:], 1e-6)

        # ---------------- phase 0: mod shard (this core: w_ada cols 1536g..)
        cT_sb = pers.tile([P, KC], F32)
        nc.sync.dma_start(cT_sb[:], c_own.ap().rearrange("(c p) o -> p (c o)", p=P))
        silu_sb = pers.tile([P, KC], BF16)
        nc.scalar.activation(silu_sb[:], cT_sb[:], AF.Silu)
        mod_sh_sb = pers.tile([P, 12], F32)
        for mu in range(12):
            wa = wst.tile([P, KC, P], BF16, tag="wada")
            nc.sync.dma_start(wa[:], w_ada_s.ap()[mu])
            mps = pa()
            for kc in range(KC):
                nc.tensor.matmul(mps[:, 0:1], wa[:, kc, :], silu_sb[:, kc:kc + 1],
                                 start=(kc == 0), stop=(kc == KC - 1))
            nc.vector.tensor_scalar_add(
                mod_sh_sb[:, mu:mu + 1], mps[:, 0:1], b_ada_sb[:, mu:mu + 1])
        mod_bounce_in = dram.tile([P, 12], F32)
        nc.sync.dma_start(mod_bounce_in[:], mod_sh_sb[:])
        mod_bounce_out = dram.tile([4 * P, 12], F32)
        if sim:
            nc.sync.dma_start(mod_bounce_out[:][0:P, :], mod_bounce_in[:])
        else:
            nc.gpsimd.collective_compute(
                "AllGather", OP.bypass, replica_groups=RG4,
                ins=[mod_bounce_in.opt()], outs=[mod_bounce_out.opt()])
        mod_sb = pers.tile([P, 4, 12], F32)
        nc.sync.dma_start(
            mod_sb[:], mod_bounce_out[:].rearrange("(g p) j -> p g j", p=P))

        def mod_chunk(vec_idx, kc):
            gc = 8 * vec_idx + kc
            return mod_sb[:, gc // 12, gc % 12:gc % 12 + 1]

        sc1p_msa = pers.tile([P, KC], F32)
        sc1p_mlp = pers.tile([P, KC], F32)
        s1_msa = pers.tile([P, KC], F32)
        s2_msa = pers.tile([P, KC], F32)
        s1_mlp = pers.tile([P, KC], F32)
        s2_mlp = pers.tile([P, KC], F32)
        for kc in range(KC):
            nc.vector.tensor_scalar_add(sc1p_msa[:, kc:kc + 1], mod_chunk(1, kc), 1.0)
            nc.vector.tensor_scalar_add(sc1p_mlp[:, kc:kc + 1], mod_chunk(4, kc), 1.0)
            nc.vector.tensor_scalar_mul(
                s1_msa[:, kc:kc + 1], mod_chunk(2, kc), 1.0 / (WS * CXS))
            nc.vector.tensor_tensor(
                s2_msa[:, kc:kc + 1], mod_chunk(2, kc), b_out_sb[:, kc:kc + 1],
                op=OP.mult)
            nc.vector.tensor_scalar_mul(
                s1_mlp[:, kc:kc + 1], mod_chunk(5, kc), 1.0)
            nc.vector.tensor_tensor(
                s2_mlp[:, kc:kc + 1], mod_chunk(5, kc), b_mlp2_sb[:, kc:kc + 1],
                op=OP.mult)

        # ---------------- phase 1: xT (bf16) via PE transpose
        xT = pers.tile([P, KC, TT], BF16)
        for r in range(TT // P):
            x_sb = work.tile([P, HID], F32, tag="xrow", bufs=2)
            nc.sync.dma_start(x_sb[:], x_own.ap()[ts(r, P), :])
            for half in range(2):
                tps = pa()
                for k4 in range(4):
                    nc.tensor.transpose(tps[:, ts(k4, P)],
                                        x_sb[:, ts(4 * half + k4, P)], ident[:])
                nc.vector.tensor_copy(
                    xT[:, 4 * half:4 * half + 4, ts(r, P)],
                    tps[:, 0:512].rearrange("p (c t) -> p c t", c=4))

        def ln_stats(src):
            """src: [P, KC, TT] bf16 -> (m_bc, r_bc) [P, TT] f32 PSUM tiles."""
            sum_c = pc()
            for kc in range(KC):
                nc.tensor.matmul(sum_c[0:1, 0:TT], ones_col_b[:], src[:, kc, :],
                                 start=(kc == 0), stop=(kc == KC - 1))
            sumsq_c = pc()
            for kc in range(KC):
                sq = work.tile([P, TT], BF16, tag="sq", bufs=3)
                nc.gpsimd.tensor_tensor(sq[:], src[:, kc, :], src[:, kc, :],
                                        op=OP.mult)
                nc.tensor.matmul(sumsq_c[0:1, 0:TT], ones_col_b[:], sq[:],
                                 start=(kc == 0), stop=(kc == KC - 1))
            m_row = work.tile([1, TT], BF16, tag="rowtmp", bufs=4)
            nc.vector.tensor_scalar_mul(m_row[:], sum_c[0:1, 0:TT], 1.0 / HID)
            msq = work.tile([1, TT], BF16, tag="rowtmp", bufs=4)
            nc.vector.tensor_tensor(msq[:], m_row[:], m_row[:], op=OP.mult)
            var_row = work.tile([1, TT], F32, tag="rowtmp", bufs=4)
            nc.vector.scalar_tensor_tensor(
                var_row[:], sumsq_c[0:1, 0:TT], 1.0 / HID, msq[:],
                op0=OP.mult, op1=OP.subtract)
            sd_row = work.tile([1, TT], F32, tag="rowtmp", bufs=4)
            nc.scalar.activation(sd_row[:], var_row[:], AF.Sqrt, bias=eps_sb[:])
            r_row = work.tile([1, TT], BF16, tag="rowtmp", bufs=4)
            with nc.allow_low_precision(reason="rstd in bf16 is plenty for LN"):
                nc.vector.reciprocal(r_row[:], sd_row[:])
            m_ps = pa()
            nc.tensor.matmul(m_ps[:, 0:TT], ones_row_b[:], m_row[:],
                             start=True, stop=True)
            r_ps = pa()
            nc.tensor.matmul(r_ps[:, 0:TT], ones_row_b[:], r_row[:],
                             start=True, stop=True)
            return m_ps, r_ps

        # ---------------- phase 2: hT own (fp8) + AllGather
        m_ps, r_ps = ln_stats(xT)
        hT_own = pers.tile([P, KC, TT], FP8)
        for kc in range(KC):
            t0 = work.tile([P, TT], F32, tag="wf32", bufs=4)
            nc.vector.tensor_sub(t0[:], xT[:, kc, :], m_ps[:, 0:TT])
            t1 = work.tile([P, TT], F32, tag="wf32", bufs=4)
            nc.vector.tensor_tensor(t1[:], t0[:], r_ps[:, 0:TT], op=OP.mult)
            nc.vector.tensor_scalar(
                hT_own[:, kc, :], t1[:], sc1p_msa[:, kc:kc + 1], mod_chunk(0, kc),
                op0=OP.mult, op1=OP.add)
        h_bounce_in_a = dram.tile([HID // 2, TT], FP8)
        h_bounce_in_b = dram.tile([HID // 2, TT], FP8)
        nc.sync.dma_start(
            h_bounce_in_a[:].rearrange("(c p) t -> p c t", p=P), hT_own[:, 0:4, :])
        nc.sync.dma_start(
            h_bounce_in_b[:].rearrange("(c p) t -> p c t", p=P), hT_own[:, 4:8, :])
        h_bounce_out_a = dram.tile([2 * HID, TT], FP8)
        h_bounce_out_b = dram.tile([2 * HID, TT], FP8)
        if sim:
            nc.sync.dma_start(h_bounce_out_a[:][0:HID // 2, :], h_bounce_in_a[:])
            nc.sync.dma_start(h_bounce_out_b[:][0:HID // 2, :], h_bounce_in_b[:])
        else:
            nc.gpsimd.collective_compute(
                "AllGather", OP.bypass, replica_groups=RG4,
                ins=[h_bounce_in_a.opt()], outs=[h_bounce_out_a.opt()])
            nc.gpsimd.collective_compute(
                "AllGather", OP.bypass, replica_groups=RG4,
                ins=[h_bounce_in_b.opt()], outs=[h_bounce_out_b.opt()])
        nc.sync.dma_start(wv_sb[:], w_v_q.ap())
        nc.sync.dma_start(wqk_sb[:], w_qk_q.ap())
        nc.sync.dma_start(wout_sb[:], w_out_q.ap())
        nc.sync.dma_start(bt_sb[:], bt_in.ap().rearrange("j p m -> p j m"))
        hT_full = pers.tile([P, 32, TT], FP8)
        for jq in range(4):
            nc.sync.dma_start(
                hT_full[:, KC * jq:KC * jq + 4, :],
                h_bounce_out_a[:][ts(jq, HID // 2), :].rearrange("(c p) t -> p c t", p=P))
            nc.sync.dma_start(
                hT_full[:, KC * jq + 4:KC * jq + 8, :],
                h_bounce_out_b[:][ts(jq, HID // 2), :].rearrange("(c p) t -> p c t", p=P))

        # ---------------- phase 3: qkv (fp8 DoubleRow)
        qT = pers.tile([P, 2, N], BF16)
        kT = pers.tile([P, 2, N], BF16)
        v_aug = pers.tile([P, NBLK, 288], FP8)
        nc.vector.memset(v_aug[:], 0.0)
        nc.vector.memset(
            v_aug[:].rearrange("p b (h e) -> p b h e", h=4)[:, :, :, 64:65], 1.0)

        for blk in range(NBLK):
            jq, tb = blk // 4, blk % 4
            vps = pa()
            for kp in range(4):
                nc.tensor.matmul(
                    vps[:, 0:256],
                    hT_full[:, KC * jq + 2 * kp:KC * jq + 2 * kp + 2, ts(tb, P)],
                    wv_sb[:, kp, :, :],
                    start=(kp == 0), stop=(kp == 3), perf_mode=DR)
            nc.vector.scalar_tensor_tensor(
                v_aug[:, blk, :].rearrange("p (h e) -> p h e", h=4)[:, :, 0:64],
                vps[:, 0:256].rearrange("p (h e) -> p h e", h=4), 1.0 / WS,
                b_v_sb[:].rearrange("p (h e) -> p h e", h=4),
                op0=OP.mult, op1=OP.add)

        for mu in range(4):       # q chunks 0,1; k chunks 2,3
            for jq in range(4):
                qps = pa()
                for kp in range(4):
                    nc.tensor.matmul(
                        qps[:, 0:TT],
                        wqk_sb[:, mu, kp, :, :],
                        hT_full[:, KC * jq + 2 * kp:KC * jq + 2 * kp + 2, :],
                        start=(kp == 0), stop=(kp == 3), perf_mode=DR)
                dst = qT if mu < 2 else kT
                nc.vector.tensor_scalar(
                    dst[:, mu % 2, ts(jq, TT)], qps[:, 0:TT], 1.0 / WS,
                    b_qk_sb[:, mu:mu + 1], op0=OP.mult, op1=OP.add)

        # ---------------- phase 4: attention
        # heads: local h = 2a + o; scores [128 keys, 1024 queries] per
        # (a, o, qspan, blk); bias koff = blk - 8*qspan - s per 128-col
        # sub-block s.
        ctxT = pers.tile([P, 2, N], FP8)
        rs_bounce_in = dram.tile([4 * HID, TT], BF16)
        for qspan in range(2):
            for a in range(2):
                cps = [pc(), pc()]
                for bp in range(NBLK // 2):
                    esb = [
                        work.tile([P, 2, 1024], FP8, tag=f"esb{o}", bufs=2,
                                  name=f"esb{o}")
                        for o in range(2)
                    ]
                    for sub in range(2):
                        blk = 2 * bp + sub
                        koff0 = blk - 8 * qspan
                        for o in range(2):
                            h = 2 * a + o
                            sc = pa()
                            for half in range(2):
                                nc.tensor.matmul(
                                    sc[:, ts(half, TT)],
                                    kT[64 * o:64 * o + 64, a, ts(blk, P)],
                                    qT[64 * o:64 * o + 64, a,
                                       1024 * qspan + 512 * half:
                                       1024 * qspan + 512 * (half + 1)],
                                    start=True, stop=True)
                            bias_arg = 0.0
                            if koff0 >= 9:
                                bias_arg = delta_col[:, h:h + 1]
                            elif koff0 >= -1:
                                for s in range(8):
                                    koff = koff0 - s
                                    if -1 <= koff <= 1:
                                        nc.tensor.matmul(
                                            sc[:, ts(s, P)],
                                            bt_sb[:, 3 * h + koff + 1, :],
                                            ident_b[:],
                                            start=False, stop=True,
                                            skip_group_check=True)
                                    elif koff >= 2:
                                        nc.tensor.matmul(
                                            sc[:, ts(s, P)],
                                            delta_row8[0:1, h, :],
                                            ones_row_b[:],
                                            start=False, stop=True,
                                            skip_group_check=True)
                            nc.scalar.activation(
                                esb[o][:, sub, :], sc[:], AF.Exp,
                                scale=0.125, bias=bias_arg)
                    for o in range(2):
                        h = 2 * a + o
                        for half in range(2):
                            nc.tensor.matmul(
                                cps[o][:, ts(half, TT)],
                                v_aug[:, 2 * bp:2 * bp + 2, 72 * h:72 * h + 72],
                                esb[o][:, :, ts(half, TT)],
                                start=(bp == 0), stop=(bp == NBLK // 2 - 1),
                                perf_mode=DR)
                for o in range(2):
                    recip = work.tile([1, 1024], BF16, tag="recip", bufs=2)
                    with nc.allow_low_precision(reason="softmax denom recip bf16"):
                        nc.vector.reciprocal(recip[:], cps[o][64:65, :])
                    csb = work.tile([64, 1024], BF16, tag="csb", bufs=2)
                    nc.vector.tensor_copy(csb[:], cps[o][0:64, :])
                    bc = pc()
                    for half in range(2):
                        nc.tensor.matmul(
                            bc[0:64, ts(half, TT)], ones64_row[:],
                            recip[0:1, ts(half, TT)], start=True, stop=True)
                    nc.vector.tensor_tensor(
                        ctxT[64 * o:64 * o + 64, a, ts(qspan, 1024)],
                        csb[:], bc[0:64, :], op=OP.mult)
            # out-proj partials for this qspan's two token quarters
            for tau in (2 * qspan, 2 * qspan + 1):
                for mu in range(KC):
                    ops_ = pa()
                    nc.tensor.matmul(
                        ops_[:, 0:TT], wout_sb[:, :, ts(mu, P)],
                        ctxT[:, :, ts(tau, TT)],
                        start=True, stop=True, perf_mode=DR)
                    po = work.tile([P, TT], BF16, tag="po", bufs=3)
                    nc.vector.tensor_copy(po[:], ops_[:, 0:TT])
                    nc.sync.dma_start(
                        rs_bounce_in[:][tau * HID + mu * P:tau * HID + (mu + 1) * P, :],
                        po[:])

        # ---------------- phase 5: ReduceScatter(add)
        rs_bounce_out = dram.tile([HID, TT], BF16)
        if sim:
            nc.sync.dma_start(rs_bounce_out[:], rs_bounce_in[:][0:HID, :])
        else:
            nc.gpsimd.collective_compute(
                "ReduceScatter", OP.add, replica_groups=RG4,
                ins=[rs_bounce_in.opt()], outs=[rs_bounce_out.opt()])
        ao_sb = pers.tile([P, KC, TT], BF16)
        nc.sync.dma_start(
            ao_sb[:], rs_bounce_out[:].rearrange("(c p) t -> p c t", p=P))

        # ---------------- phase 6: residual + LN2
        x2T = pers.tile([P, KC, TT], BF16)
        for mu in range(KC):
            tmp = work.tile([P, TT], BF16, tag="wbf", bufs=4)
            nc.vector.tensor_scalar(
                tmp[:], ao_sb[:, mu, :], s1_msa[:, mu:mu + 1], s2_msa[:, mu:mu + 1],
                op0=OP.mult, op1=OP.add)
            nc.vector.tensor_add(x2T[:, mu, :], tmp[:], xT[:, mu, :])

        m2_ps, r2_ps = ln_stats(x2T)
        h2T = pers.tile([P, KC, TT], BF16)
        for kc in range(KC):
            t0 = work.tile([P, TT], F32, tag="wf32", bufs=4)
            nc.vector.tensor_sub(t0[:], x2T[:, kc, :], m2_ps[:, 0:TT])
            t1 = work.tile([P, TT], F32, tag="wf32", bufs=4)
            nc.vector.tensor_tensor(t1[:], t0[:], r2_ps[:, 0:TT], op=OP.mult)
            nc.vector.tensor_scalar(
                h2T[:, kc, :], t1[:], sc1p_mlp[:, kc:kc + 1], mod_chunk(3, kc),
                op0=OP.mult, op1=OP.add)

        # ---------------- phase 7: MLP (token-sharded, fp8 weights streamed)
        gT = pers.tile([P, MLPH // P, TT], BF16)
        for nu in range(MLPH // P):
            w1 = wst.tile([P, KC, P], BF16, tag="w1", bufs=3)
            nc.sync.dma_start(w1[:], w_mlp1_b.ap()[nu])
            gps = pa()
            for kc in range(KC):
                nc.tensor.matmul(gps[:, 0:TT], w1[:, kc, :],
                                 h2T[:, kc, :],
                                 start=(kc == 0), stop=(kc == KC - 1))
            nc.scalar.activation(
                gT[:, nu, :], gps[:, 0:TT], AF.Gelu_apprx_tanh,
                bias=b_mlp1_sb[:, nu:nu + 1])
        for mu in range(KC):
            mps2 = pa()
            for hf in range(2):
                w2 = wst.tile([P, 16, P], BF16, tag="w2", bufs=2)
                nc.sync.dma_start(w2[:], w_mlp2_b.ap()[mu, :, ts(hf, 16), :])
                for kc in range(16):
                    gkc = 16 * hf + kc
                    nc.tensor.matmul(mps2[:, 0:TT], w2[:, kc, :],
                                     gT[:, gkc, :],
                                     start=(gkc == 0), stop=(gkc == MLPH // P - 1))
            tmp = work.tile([P, TT], BF16, tag="wbf", bufs=4)
            nc.vector.tensor_scalar(
                tmp[:], mps2[:, 0:TT], s1_mlp[:, mu:mu + 1], s2_mlp[:, mu:mu + 1],
                op0=OP.mult, op1=OP.add)
            outT = work.tile([P, TT], F32, tag="wf32", bufs=4)
            nc.vector.tensor_add(outT[:], tmp[:], x2T[:, mu, :])
            tps = pa()
            for r in range(TT // P):
                nc.tensor.transpose(tps[:, ts(r, P)], outT[:, ts(r, P)], ident[:])
            osb = work.tile([P, 512], F32, tag="osb", bufs=2)
            nc.vector.tensor_copy(osb[:], tps[:, 0:512])
            nc.sync.dma_start(
                out_t.ap()[:, ts(mu, P)].rearrange("(r p) m -> p r m", p=P),
                osb[:].rearrange("p (r m) -> p r m", r=4))

    nc.compile()
    return nc


# ---------------------------------------------------------------- runner
class SpmdRunner:
    def __init__(self, nc, n_cores, donate=True):
        install_neuronx_cc_hook()
        self.nc = nc
        self.n_cores = n_cores
        partition_name = nc.partition_id_tensor.name if nc.partition_id_tensor else None
        in_names, out_names, out_avals = [], [], []
        for alloc in nc.m.functions[0].allocations:
            if not isinstance(alloc, mybir.MemoryLocationSet):
                continue
            name = alloc.memorylocations[0].name
            if alloc.kind == "ExternalInput":
                if name != partition_name:
                    in_names.append(name)
            elif alloc.kind == "ExternalOutput":
                out_names.append(name)
                out_avals.append(
                    jax.core.ShapedArray(tuple(alloc.tensor_shape), mybir.dt.np(alloc.dtype))
                )
        self.in_names, self.out_names, self.out_avals = in_names, out_names, out_avals
        n_params = len(in_names)
        n_outs = len(out_avals)
        all_in_names = list(in_names) + list(out_names)
        if partition_name is not None:
            all_in_names.append(partition_name)

        def _body(*args):
            operands = list(args)
            if partition_name is not None:
                operands.append(partition_id_tensor())
            return tuple(
                _bass_exec_p.bind(
                    *operands,
                    out_avals=tuple(out_avals),
                    in_names=tuple(all_in_names),
                    out_names=tuple(out_names),
                    lowering_input_output_aliases=(),
                    sim_require_finite=True,
                    sim_require_nnan=True,
                    nc=nc,
                )
            )

        devices = jax.devices()[:n_cores]
        self.mesh = Mesh(np.asarray(devices), ("core",))
        donate_idx = tuple(range(n_params, n_params + n_outs)) if donate else ()
        self.fn = jax.jit(
            shard_map(
                _body,
                mesh=self.mesh,
                in_specs=(PartitionSpec("core"),) * (n_params + n_outs),
                out_specs=(PartitionSpec("core"),) * n_outs,
                check_rep=False,
            ),
            donate_argnums=donate_idx,
            keep_unused=True,
        )
        self.n_params, self.n_outs = n_params, n_outs

    def _concat_inputs(self, in_maps):
        return [
            np.concatenate([np.asarray(in_maps[c][n]) for c in range(self.n_cores)], axis=0)
            for n in self.in_names
        ]

    def run(self, in_maps):
        sharding = jax.sharding.NamedSharding(self.mesh, PartitionSpec("core"))
        concat_in = [
            jax.device_put(x, sharding) for x in self._concat_inputs(in_maps)
        ]
        zeros = [
            jax.device_put(
                np.zeros((self.n_cores * a.shape[0], *a.shape[1:]), a.dtype), sharding)
            for a in self.out_avals
        ]
        outs = self.fn(*concat_in, *zeros)
        return self._split(outs)

    def _split(self, out_arrs):
        return [
            {
                n: np.asarray(out_arrs[i]).reshape(self.n_cores, *self.out_avals[i].shape)[c]
                for i, n in enumerate(self.out_names)
            }
            for c in range(self.n_cores)
        ]

    def bench(self, in_maps, iters=30, warmup=3):
        """Chained repeated execution: output buffers of call i are donated as
        the output operands of call i+1, serializing calls on-device."""
        sharding = jax.sharding.NamedSharding(self.mesh, PartitionSpec("core"))
        concat_in = [jax.device_put(x, sharding) for x in self._concat_inputs(in_maps)]
        outs = tuple(
            jax.device_put(
                np.zeros((self.n_cores * a.shape[0], *a.shape[1:]), a.dtype), sharding)
            for a in self.out_avals
        )
        for _ in range(warmup):
            outs = self.fn(*concat_in, *outs)
        jax.block_until_ready(outs)
        t0 = time.perf_counter()
        for _ in range(iters):
            outs = self.fn(*concat_in, *outs)
        jax.block_until_ready(outs)
        t1 = time.perf_counter()
        return (t1 - t0) / iters, self._split(outs)


_CACHE = {}


def kernel(**inputs):
    """Full-input DiT block on 8 NeuronCores; returns full [B, N, HID] f32."""
    if "nc" not in _CACHE:
        _CACHE["nc"] = build_kernel()
        _CACHE["runner"] = SpmdRunner(_CACHE["nc"], 8)
    maps = make_in_maps(inputs)
    results = _CACHE["runner"].run(maps)
    return assemble_output(results)


# revision 19
# speedup vs baseline: 1.2239x; 1.0236x over previous
"""DiT block Bass kernel for 8 TRN2 NeuronCores.

Core i -> (b = i//4, g = i%4): batch item b; head group 4g..4g+3; token
quarter [512g, 512g+512) of batch b.  Activations are hidden-major
([hidden_chunk=128, tokens]) throughout; PE transposes at entry (x) and
exit (out).  Collectives: AllGather(4) for mod + h, ReduceScatter(4)
for the out-projection partials.

Dtype strategy: residual stream bf16; weights host-quantized (w_qkv,
w_out, w_mlp1, w_mlp2 in fp8e4m3 prescaled by 32; w_ada bf16); all big
GEMMs except QK^T run in fp8 DoubleRow perf mode (2 k-subtiles of 128
per pass).  Relative attention bias: constant for |d| >= 91, so
off-band score tiles get their bias via the exp() bias operand; band
tiles get a PE matmul-add of pretransposed bias tiles into PSUM before
exp.  Softmax is computed without max-subtraction (scores provably
small); denominators come from an appended ones-row in the fp8 V tiles.
"""
import contextlib
import time
import numpy as np
import ml_dtypes
import jax
from jax.sharding import Mesh, PartitionSpec
from jax.experimental.shard_map import shard_map

import concourse.bass as bass
import concourse.mybir as mybir
import concourse.tile as tile
from concourse import bacc
from concourse.bass2jax import _bass_exec_p, install_neuronx_cc_hook, partition_id_tensor

F32 = mybir.dt.float32
BF16 = mybir.dt.bfloat16
FP8 = mybir.dt.float8e4
AF = mybir.ActivationFunctionType
OP = mybir.AluOpType
DR = mybir.MatmulPerfMode.DoubleRow
ts = bass.ts

NPBF16 = ml_dtypes.bfloat16
NPFP8 = ml_dtypes.float8_e4m3

B, N, HID = 2, 2048, 1024
NH, HD = 16, 64
MLPH = 4 * HID
NB, MAXD = 32, 128
P = 128
TT = 512
KC = HID // P          # 8
NBLK = N // P          # 16
WS = 32.0              # host weight prescale for fp8
CXS = 64.0             # ctx prescale for fp8
RG4 = [[0, 1, 2, 3], [4, 5, 6, 7]]


# ---------------------------------------------------------------- host prep
def rel_bucket_np(d):
    nb = NB // 2
    buckets = np.where(d > 0, nb, 0).astype(np.int64)
    rp = np.abs(d)
    max_exact = nb // 2
    is_small = rp < max_exact
    log_ratio = np.log(np.maximum(rp, 1).astype(np.float32) / np.float32(max_exact))
    rpl = max_exact + (
        log_ratio / np.float32(np.log(MAXD / max_exact)) * (nb - max_exact)
    ).astype(np.int32)
    rpl = np.minimum(rpl, nb - 1)
    return buckets + np.where(is_small, rp, rpl)


def make_bias_tables(rel_table, g):
    """Band bias tiles + deltas for local heads 4g..4g+3.

    bt[h, j][p, m] = 8 * badj(128*(j-1) + m - p)   (j = koff+1, koff in -1..1)
    badj(d) = bias(d) - bias_minus;  delta = bias_plus - bias_minus.
    The 8x prescale compensates the 0.125 exp scale (bias added in PSUM
    pre-scale, exp bias arg applied post-scale).
    """
    d = np.arange(-(N - 1), N)
    buck = rel_bucket_np(d)  # index by d + N-1
    bt = np.zeros((4, 3, P, P), np.float32)
    delta = np.zeros((4,), np.float32)
    for hl in range(4):
        hg = 4 * g + hl
        bvec = rel_table[:, hg].astype(np.float32)
        bmin = bvec[NB // 2 - 1]
        delta[hl] = bvec[NB - 1] - bmin
        diag = bvec[buck] - bmin  # badj over d in [-(N-1), N-1]
        p = np.arange(P)[:, None]
        m = np.arange(P)[None, :]
        for j, koff in enumerate((-1, 0, 1)):
            dd = 128 * koff + m - p
            bt[hl, j] = 8.0 * diag[dd + (N - 1)]
    return bt.astype(NPBF16), delta


def make_in_maps(inputs):
    x = np.asarray(inputs["x"], np.float32)
    c = np.asarray(inputs["c"], np.float32)
    w_ada = np.asarray(inputs["w_ada"], np.float32)
    b_ada = np.asarray(inputs["b_ada"], np.float32)
    w_qkv = np.asarray(inputs["w_qkv"], np.float32)
    b_qkv = np.asarray(inputs["b_qkv"], np.float32)
    w_out = np.asarray(inputs["w_out"], np.float32)
    b_out = np.asarray(inputs["b_out"], np.float32)
    rel_table = np.asarray(inputs["rel_table"], np.float32)
    w_mlp1 = np.asarray(inputs["w_mlp1"], np.float32)
    b_mlp1 = np.asarray(inputs["b_mlp1"], np.float32)
    w_mlp2 = np.asarray(inputs["w_mlp2"], np.float32)
    b_mlp2 = np.asarray(inputs["b_mlp2"], np.float32)

    ident = np.eye(P, dtype=np.float32)
    ident_b = np.eye(P, dtype=np.float32).astype(NPBF16)
    ones_col_b = np.ones((P, 1), np.float32).astype(NPBF16)
    ones_row_b = np.ones((1, P), np.float32).astype(NPBF16)
    ones64_row = np.full((1, 64), CXS, np.float32).astype(NPBF16)

    w_mlp1_b = np.ascontiguousarray(
        w_mlp1.reshape(KC, P, MLPH // P, P).transpose(2, 1, 0, 3)
        .astype(NPBF16))                      # [32, P, kc8, P]
    w_mlp2_b = np.ascontiguousarray(
        w_mlp2.reshape(MLPH // P, P, KC, P).transpose(2, 1, 0, 3)
        .astype(NPBF16))                      # [8, P, kc32, P]

    maps = []
    for i in range(8):
        b, g = divmod(i, 4)
        qs, ks, vs = 256 * g, HID + 256 * g, 2 * HID + 256 * g
        w_qk = np.concatenate([w_qkv[:, qs:qs + 256], w_qkv[:, ks:ks + 256]], 1)
        w_v = w_qkv[:, vs:vs + 256]
        b_qk = np.concatenate([b_qkv[qs:qs + 256], b_qkv[ks:ks + 256]])
        bv = b_qkv[vs:vs + 256]

        # [P, mu4, kp4, 2, P]: global k = 128*(2*kp+sub) + p, out chunk mu
        w_qk_q = (w_qk * WS).reshape(4, 2, P, 4, P).transpose(
            2, 3, 0, 1, 4).astype(NPFP8)
        # [P, kp4, 2, 256]
        w_v_q = (w_v * WS).reshape(4, 2, P, 256).transpose(2, 0, 1, 3).astype(NPFP8)
        # [P, 2, HID]: ctx chunk-major (2 chunks of own 256 ctx dims)
        w_out_q = (w_out[256 * g:256 * (g + 1), :] * WS).reshape(
            2, P, HID).transpose(1, 0, 2).astype(NPFP8)

        bt, delta = make_bias_tables(rel_table, g)
        maps.append({
            "x_own": np.ascontiguousarray(x[b, 512 * g:512 * (g + 1), :]),
            "c_own": np.ascontiguousarray(c[b][:, None]),
            "w_ada_s": np.ascontiguousarray(
                w_ada[:, 1536 * g:1536 * (g + 1)].reshape(KC, P, 12, P)
                .transpose(2, 1, 0, 3).astype(NPBF16)),
            "b_ada_s": np.ascontiguousarray(
                b_ada[1536 * g:1536 * (g + 1)].reshape(12, P).T),
            "w_qk_q": np.ascontiguousarray(w_qk_q),
            "w_v_q": np.ascontiguousarray(w_v_q),
            "b_qk_s": np.ascontiguousarray(b_qk.reshape(4, P).T),
            "b_v_bcast": np.ascontiguousarray(
                np.broadcast_to(bv[None, :], (P, 256)).astype(NPBF16)),
            "w_out_q": np.ascontiguousarray(w_out_q),
            "b_out_r": np.ascontiguousarray(b_out.reshape(KC, P).T),
            "w_mlp1_b": w_mlp1_b,
            "b_mlp1_r": np.ascontiguousarray(b_mlp1.reshape(MLPH // P, P).T),
            "w_mlp2_b": w_mlp2_b,
            "b_mlp2_r": np.ascontiguousarray(b_mlp2.reshape(KC, P).T),
            "bt": np.ascontiguousarray(bt.reshape(12, P, P)),
            "delta_row8": np.ascontiguousarray(
                np.broadcast_to((8.0 * delta)[None, :, None], (1, 4, P))
                .astype(NPBF16)),
            "delta_col": np.ascontiguousarray(
                np.broadcast_to(delta[None, :], (P, 4)).astype(np.float32)),
            "ident": ident,
            "ident_b": ident_b,
            "ones_col_b": ones_col_b,
            "ones_row_b": ones_row_b,
            "ones64_row": ones64_row,
        })
    return maps


def assemble_output(results):
    out = np.zeros((B, N, HID), np.float32)
    for i in range(8):
        b, g = divmod(i, 4)
        out[b, 512 * g:512 * (g + 1), :] = results[i]["out"]
    return out


# ---------------------------------------------------------------- builder
def build_kernel(sim=False):
    nc = bacc.Bacc("TRN2", target_bir_lowering=False, debug=False, num_devices=8)

    din = lambda nm, sh, dt=F32: nc.dram_tensor(nm, sh, dt, kind="ExternalInput")
    x_own = din("x_own", [TT, HID])
    c_own = din("c_own", [HID, 1])
    w_ada_s = din("w_ada_s", [12, P, KC, P], BF16)
    b_ada_s = din("b_ada_s", [P, 12])
    w_qk_q = din("w_qk_q", [P, 4, 4, 2, P], FP8)
    w_v_q = din("w_v_q", [P, 4, 2, 256], FP8)
    b_qk_s = din("b_qk_s", [P, 4])
    b_v_bcast = din("b_v_bcast", [P, 256], BF16)
    w_out_q = din("w_out_q", [P, 2, HID], FP8)
    b_out_r = din("b_out_r", [P, KC])
    w_mlp1_b = din("w_mlp1_b", [MLPH // P, P, KC, P], BF16)
    b_mlp1_r = din("b_mlp1_r", [P, MLPH // P])
    w_mlp2_b = din("w_mlp2_b", [KC, P, MLPH // P, P], BF16)
    b_mlp2_r = din("b_mlp2_r", [P, KC])
    bt_in = din("bt", [12, P, P], BF16)
    delta_row8_in = din("delta_row8", [1, 4, P], BF16)
    delta_col_in = din("delta_col", [P, 4])
    ident_in = din("ident", [P, P])
    ident_b_in = din("ident_b", [P, P], BF16)
    ones_col_b_in = din("ones_col_b", [P, 1], BF16)
    ones_row_b_in = din("ones_row_b", [1, P], BF16)
    ones64_row_in = din("ones64_row", [1, 64], BF16)

    out_t = nc.dram_tensor("out", [TT, HID], F32, kind="ExternalOutput")

    with tile.TileContext(nc) as tc, contextlib.ExitStack() as ctx:
        const = ctx.enter_context(tc.tile_pool(name="const", bufs=1))
        pers = ctx.enter_context(tc.tile_pool(name="pers", bufs=1))
        work = ctx.enter_context(tc.tile_pool(name="work", bufs=3))
        wst = ctx.enter_context(tc.tile_pool(name="wst", bufs=2))
        dram = ctx.enter_context(tc.tile_pool(name="dram", bufs=1, space="DRAM"))
        ps_a = ctx.enter_context(tc.tile_pool(name="ps_a", bufs=2, space="PSUM"))
        ps_c = ctx.enter_context(tc.tile_pool(name="ps_c", bufs=2, space="PSUM"))

        def pa():
            return ps_a.tile([P, 1024], F32, tag="A", name="pa")

        def pc():
            return ps_c.tile([72, 1024], F32, tag="C", name="pc")

        # ---------------- constants
        ident = const.tile([P, P], F32)
        nc.sync.dma_start(ident[:], ident_in.ap())
        ident_b = const.tile([P, P], BF16)
        nc.sync.dma_start(ident_b[:], ident_b_in.ap())
        ones_col_b = const.tile([P, 1], BF16)
        nc.sync.dma_start(ones_col_b[:], ones_col_b_in.ap())
        ones_row_b = const.tile([1, P], BF16)
        nc.sync.dma_start(ones_row_b[:], ones_row_b_in.ap())
        ones64_row = const.tile([1, 64], BF16)
        nc.sync.dma_start(ones64_row[:], ones64_row_in.ap())
        b_qk_sb = const.tile([P, 4], F32)
        nc.sync.dma_start(b_qk_sb[:], b_qk_s.ap())
        b_v_sb = const.tile([P, 256], BF16)
        nc.sync.dma_start(b_v_sb[:], b_v_bcast.ap())
        b_out_sb = const.tile([P, KC], F32)
        nc.sync.dma_start(b_out_sb[:], b_out_r.ap())
        b_mlp1_sb = const.tile([P, MLPH // P], F32)
        nc.sync.dma_start(b_mlp1_sb[:], b_mlp1_r.ap())
        b_mlp2_sb = const.tile([P, KC], F32)
        nc.sync.dma_start(b_mlp2_sb[:], b_mlp2_r.ap())
        b_ada_sb = const.tile([P, 12], F32)
        nc.sync.dma_start(b_ada_sb[:], b_ada_s.ap())
        bt_sb = const.tile([P, 12, P], BF16)
        delta_row8 = const.tile([1, 4, P], BF16)
        nc.sync.dma_start(delta_row8[:], delta_row8_in.ap())
        delta_col = const.tile([P, 4], F32)
        nc.sync.dma_start(delta_col[:], delta_col_in.ap())
        wqk_sb = const.tile([P, 4, 4, 2, P], FP8)
        wv_sb = const.tile([P, 4, 2, 256], FP8)
        wout_sb = const.tile([P, 2, HID], FP8)
        eps_sb = const.tile([1, 1], F32)
        nc.vector.memset(eps_sb[:], 1e-6)

        # ---------------- phase 0: mod shard (this core: w_ada cols 1536g..)
        cT_sb = pers.tile([P, KC], F32)
        nc.sync.dma_start(cT_sb[:], c_own.ap().rearrange("(c p) o -> p (c o)", p=P))
        silu_sb = pers.tile([P, KC], BF16)
        nc.scalar.activation(silu_sb[:], cT_sb[:], AF.Silu)
        mod_sh_sb = pers.tile([P, 12], F32)
        for mu in range(12):
            wa = wst.tile([P, KC, P], BF16, tag="wada", bufs=4)
            nc.sync.dma_start(wa[:], w_ada_s.ap()[mu])
            mps = pa()
            for kc in range(KC):
                nc.tensor.matmul(mps[:, 0:1], wa[:, kc, :], silu_sb[:, kc:kc + 1],
                                 start=(kc == 0), stop=(kc == KC - 1))
            nc.vector.tensor_scalar_add(
                mod_sh_sb[:, mu:mu + 1], mps[:, 0:1], b_ada_sb[:, mu:mu + 1])
        mod_bounce_in = dram.tile([P, 12], F32)
        nc.sync.dma_start(mod_bounce_in[:], mod_sh_sb[:])
        mod_bounce_out = dram.tile([4 * P, 12], F32)
        if sim:
            nc.sync.dma_start(mod_bounce_out[:][0:P, :], mod_bounce_in[:])
        else:
            nc.gpsimd.collective_compute(
                "AllGather", OP.bypass, replica_groups=RG4,
                ins=[mod_bounce_in.opt()], outs=[mod_bounce_out.opt()])
        mod_sb = pers.tile([P, 4, 12], F32)
        nc.sync.dma_start(
            mod_sb[:], mod_bounce_out[:].rearrange("(g p) j -> p g j", p=P))

        def mod_chunk(vec_idx, kc):
            gc = 8 * vec_idx + kc
            return mod_sb[:, gc // 12, gc % 12:gc % 12 + 1]

        sc1p_msa = pers.tile([P, KC], F32)
        sc1p_mlp = pers.tile([P, KC], F32)
        s1_msa = pers.tile([P, KC], F32)
        s2_msa = pers.tile([P, KC], F32)
        s1_mlp = pers.tile([P, KC], F32)
        s2_mlp = pers.tile([P, KC], F32)
        for kc in range(KC):
            nc.vector.tensor_scalar_add(sc1p_msa[:, kc:kc + 1], mod_chunk(1, kc), 1.0)
            nc.vector.tensor_scalar_add(sc1p_mlp[:, kc:kc + 1], mod_chunk(4, kc), 1.0)
            nc.vector.tensor_scalar_mul(
                s1_msa[:, kc:kc + 1], mod_chunk(2, kc), 1.0 / (WS * CXS))
            nc.vector.tensor_tensor(
                s2_msa[:, kc:kc + 1], mod_chunk(2, kc), b_out_sb[:, kc:kc + 1],
                op=OP.mult)
            nc.vector.tensor_scalar_mul(
                s1_mlp[:, kc:kc + 1], mod_chunk(5, kc), 1.0)
            nc.vector.tensor_tensor(
                s2_mlp[:, kc:kc + 1], mod_chunk(5, kc), b_mlp2_sb[:, kc:kc + 1],
                op=OP.mult)

        # ---------------- phase 1: xT (bf16) via PE transpose
        xT = pers.tile([P, KC, TT], BF16)
        for r in range(TT // P):
            x_sb = work.tile([P, HID], F32, tag="xrow", bufs=3)
            nc.sync.dma_start(x_sb[:], x_own.ap()[ts(r, P), :])
            for half in range(2):
                tps = pa()
                for k4 in range(4):
                    nc.tensor.transpose(tps[:, ts(k4, P)],
                                        x_sb[:, ts(4 * half + k4, P)], ident[:])
                nc.vector.tensor_copy(
                    xT[:, 4 * half:4 * half + 4, ts(r, P)],
                    tps[:, 0:512].rearrange("p (c t) -> p c t", c=4))

        def ln_stats(src):
            """src: [P, KC, TT] bf16 -> (m_bc, r_bc) [P, TT] f32 PSUM tiles."""
            sum_c = pc()
            for kc in range(KC):
                nc.tensor.matmul(sum_c[0:1, 0:TT], ones_col_b[:], src[:, kc, :],
                                 start=(kc == 0), stop=(kc == KC - 1))
            sumsq_c = pc()
            for kc in range(KC):
                sq = work.tile([P, TT], BF16, tag="sq", bufs=3)
                nc.gpsimd.tensor_tensor(sq[:], src[:, kc, :], src[:, kc, :],
                                        op=OP.mult)
                nc.tensor.matmul(sumsq_c[0:1, 0:TT], ones_col_b[:], sq[:],
                                 start=(kc == 0), stop=(kc == KC - 1))
            m_row = work.tile([1, TT], BF16, tag="rowtmp", bufs=4)
            nc.vector.tensor_scalar_mul(m_row[:], sum_c[0:1, 0:TT], 1.0 / HID)
            msq = work.tile([1, TT], BF16, tag="rowtmp", bufs=4)
            nc.vector.tensor_tensor(msq[:], m_row[:], m_row[:], op=OP.mult)
            var_row = work.tile([1, TT], F32, tag="rowtmp", bufs=4)
            nc.vector.scalar_tensor_tensor(
                var_row[:], sumsq_c[0:1, 0:TT], 1.0 / HID, msq[:],
                op0=OP.mult, op1=OP.subtract)
            sd_row = work.tile([1, TT], F32, tag="rowtmp", bufs=4)
            nc.scalar.activation(sd_row[:], var_row[:], AF.Sqrt, bias=eps_sb[:])
            r_row = work.tile([1, TT], BF16, tag="rowtmp", bufs=4)
            with nc.allow_low_precision(reason="rstd in bf16 is plenty for LN"):
                nc.vector.reciprocal(r_row[:], sd_row[:])
            m_ps = pa()
            nc.tensor.matmul(m_ps[:, 0:TT], ones_row_b[:], m_row[:],
                             start=True, stop=True)
            r_ps = pa()
            nc.tensor.matmul(r_ps[:, 0:TT], ones_row_b[:], r_row[:],
                             start=True, stop=True)
            return m_ps, r_ps

        # ---------------- phase 2: hT own (fp8) + AllGather
        m_ps, r_ps = ln_stats(xT)
        hT_own = pers.tile([P, KC, TT], FP8)
        for kc in range(KC):
            t0 = work.tile([P, TT], F32, tag="wf32", bufs=4)
            nc.vector.tensor_sub(t0[:], xT[:, kc, :], m_ps[:, 0:TT])
            t1 = work.tile([P, TT], F32, tag="wf32", bufs=4)
            nc.vector.tensor_tensor(t1[:], t0[:], r_ps[:, 0:TT], op=OP.mult)
            nc.vector.tensor_scalar(
                hT_own[:, kc, :], t1[:], sc1p_msa[:, kc:kc + 1], mod_chunk(0, kc),
                op0=OP.mult, op1=OP.add)
        h_bounce_in_a = dram.tile([HID // 2, TT], FP8)
        h_bounce_in_b = dram.tile([HID // 2, TT], FP8)
        nc.sync.dma_start(
            h_bounce_in_a[:].rearrange("(c p) t -> p c t", p=P), hT_own[:, 0:4, :])
        nc.sync.dma_start(
            h_bounce_in_b[:].rearrange("(c p) t -> p c t", p=P), hT_own[:, 4:8, :])
        h_bounce_out_a = dram.tile([2 * HID, TT], FP8)
        h_bounce_out_b = dram.tile([2 * HID, TT], FP8)
        if sim:
            nc.sync.dma_start(h_bounce_out_a[:][0:HID // 2, :], h_bounce_in_a[:])
            nc.sync.dma_start(h_bounce_out_b[:][0:HID // 2, :], h_bounce_in_b[:])
        else:
            nc.gpsimd.collective_compute(
                "AllGather", OP.bypass, replica_groups=RG4,
                ins=[h_bounce_in_a.opt()], outs=[h_bounce_out_a.opt()])
            nc.gpsimd.collective_compute(
                "AllGather", OP.bypass, replica_groups=RG4,
                ins=[h_bounce_in_b.opt()], outs=[h_bounce_out_b.opt()])
        nc.sync.dma_start(wv_sb[:], w_v_q.ap())
        nc.sync.dma_start(wqk_sb[:], w_qk_q.ap())
        nc.sync.dma_start(wout_sb[:], w_out_q.ap())
        nc.sync.dma_start(bt_sb[:], bt_in.ap().rearrange("j p m -> p j m"))
        hT_full = pers.tile([P, 32, TT], FP8)
        for jq in range(4):
            nc.sync.dma_start(
                hT_full[:, KC * jq:KC * jq + 4, :],
                h_bounce_out_a[:][ts(jq, HID // 2), :].rearrange("(c p) t -> p c t", p=P))
            nc.sync.dma_start(
                hT_full[:, KC * jq + 4:KC * jq + 8, :],
                h_bounce_out_b[:][ts(jq, HID // 2), :].rearrange("(c p) t -> p c t", p=P))

        # ---------------- phase 3: qkv (fp8 DoubleRow)
        qT = pers.tile([P, 2, N], BF16)
        kT = pers.tile([P, 2, N], BF16)
        v_aug = pers.tile([P, NBLK, 288], FP8)
        nc.vector.memset(v_aug[:], 0.0)
        nc.vector.memset(
            v_aug[:].rearrange("p b (h e) -> p b h e", h=4)[:, :, :, 64:65], 1.0)

        for blk in range(NBLK):
            jq, tb = blk // 4, blk % 4
            vps = pa()
            for kp in range(4):
                nc.tensor.matmul(
                    vps[:, 0:256],
                    hT_full[:, KC * jq + 2 * kp:KC * jq + 2 * kp + 2, ts(tb, P)],
                    wv_sb[:, kp, :, :],
                    start=(kp == 0), stop=(kp == 3), perf_mode=DR)
            nc.vector.scalar_tensor_tensor(
                v_aug[:, blk, :].rearrange("p (h e) -> p h e", h=4)[:, :, 0:64],
                vps[:, 0:256].rearrange("p (h e) -> p h e", h=4), 1.0 / WS,
                b_v_sb[:].rearrange("p (h e) -> p h e", h=4),
                op0=OP.mult, op1=OP.add)

        for mu in range(4):       # q chunks 0,1; k chunks 2,3
            for jq in range(4):
                qps = pa()
                for kp in range(4):
                    nc.tensor.matmul(
                        qps[:, 0:TT],
                        wqk_sb[:, mu, kp, :, :],
                        hT_full[:, KC * jq + 2 * kp:KC * jq + 2 * kp + 2, :],
                        start=(kp == 0), stop=(kp == 3), perf_mode=DR)
                dst = qT if mu < 2 else kT
                nc.vector.tensor_scalar(
                    dst[:, mu % 2, ts(jq, TT)], qps[:, 0:TT], 1.0 / WS,
                    b_qk_sb[:, mu:mu + 1], op0=OP.mult, op1=OP.add)

        # ---------------- phase 4: attention
        # heads: local h = 2a + o; scores [128 keys, 1024 queries] per
        # (a, o, qspan, blk); bias koff = blk - 8*qspan - s per 128-col
        # sub-block s.
        ctxT = pers.tile([P, 2, N], FP8)
        rs_bounce_in = dram.tile([4 * HID, TT], BF16)
        for qspan in range(2):
            for a in range(2):
                cps = [pc(), pc()]
                for bp in range(NBLK // 2):
                    esb = [
                        work.tile([P, 2, 1024], FP8, tag=f"esb{o}", bufs=2,
                                  name=f"esb{o}")
                        for o in range(2)
                    ]
                    for sub in range(2):
                        blk = 2 * bp + sub
                        koff0 = blk - 8 * qspan
                        for o in range(2):
                            h = 2 * a + o
                            sc = pa()
                            for half in range(2):
                                nc.tensor.matmul(
                                    sc[:, ts(half, TT)],
                                    kT[64 * o:64 * o + 64, a, ts(blk, P)],
                                    qT[64 * o:64 * o + 64, a,
                                       1024 * qspan + 512 * half:
                                       1024 * qspan + 512 * (half + 1)],
                                    start=True, stop=True)
                            bias_arg = 0.0
                            if koff0 >= 9:
                                bias_arg = delta_col[:, h:h + 1]
                            elif koff0 >= -1:
                                for s in range(8):
                                    koff = koff0 - s
                                    if -1 <= koff <= 1:
                                        nc.tensor.matmul(
                                            sc[:, ts(s, P)],
                                            bt_sb[:, 3 * h + koff + 1, :],
                                            ident_b[:],
                                            start=False, stop=True,
                                            skip_group_check=True)
                                    elif koff >= 2:
                                        nc.tensor.matmul(
                                            sc[:, ts(s, P)],
                                            delta_row8[0:1, h, :],
                                            ones_row_b[:],
                                            start=False, stop=True,
                                            skip_group_check=True)
                            nc.scalar.activation(
                                esb[o][:, sub, :], sc[:], AF.Exp,
                                scale=0.125, bias=bias_arg)
                    for o in range(2):
                        h = 2 * a + o
                        for half in range(2):
                            nc.tensor.matmul(
                                cps[o][:, ts(half, TT)],
                                v_aug[:, 2 * bp:2 * bp + 2, 72 * h:72 * h + 72],
                                esb[o][:, :, ts(half, TT)],
                                start=(bp == 0), stop=(bp == NBLK // 2 - 1),
                                perf_mode=DR)
                for o in range(2):
                    recip = work.tile([1, 1024], BF16, tag="recip", bufs=2)
                    with nc.allow_low_precision(reason="softmax denom recip bf16"):
                        nc.vector.reciprocal(recip[:], cps[o][64:65, :])
                    csb = work.tile([64, 1024], BF16, tag="csb", bufs=2)
                    nc.vector.tensor_copy(csb[:], cps[o][0:64, :])
                    bc = pc()
                    for half in range(2):
                        nc.tensor.matmul(
                            bc[0:64, ts(half, TT)], ones64_row[:],
                            recip[0:1, ts(half, TT)], start=True, stop=True)
                    nc.vector.tensor_tensor(
                        ctxT[64 * o:64 * o + 64, a, ts(qspan, 1024)],
                        csb[:], bc[0:64, :], op=OP.mult)
            # out-proj partials for this qspan's two token quarters
            for tau in (2 * qspan, 2 * qspan + 1):
                for mu in range(KC):
                    ops_ = pa()
                    nc.tensor.matmul(
                        ops_[:, 0:TT], wout_sb[:, :, ts(mu, P)],
                        ctxT[:, :, ts(tau, TT)],
                        start=True, stop=True, perf_mode=DR)
                    po = work.tile([P, TT], BF16, tag="po", bufs=3)
                    nc.vector.tensor_copy(po[:], ops_[:, 0:TT])
                    nc.sync.dma_start(
                        rs_bounce_in[:][tau * HID + mu * P:tau * HID + (mu + 1) * P, :],
                        po[:])

        # ---------------- phase 5: ReduceScatter(add)
        rs_bounce_out = dram.tile([HID, TT], BF16)
        if sim:
            nc.sync.dma_start(rs_bounce_out[:], rs_bounce_in[:][0:HID, :])
        else:
            nc.gpsimd.collective_compute(
                "ReduceScatter", OP.add, replica_groups=RG4,
                ins=[rs_bounce_in.opt()], outs=[rs_bounce_out.opt()])
        ao_sb = pers.tile([P, KC, TT], BF16)
        nc.sync.dma_start(
            ao_sb[:], rs_bounce_out[:].rearrange("(c p) t -> p c t", p=P))

        # ---------------- phase 6: residual + LN2
        x2T = pers.tile([P, KC, TT], BF16)
        for mu in range(KC):
            tmp = work.tile([P, TT], BF16, tag="wbf", bufs=4)
            nc.vector.tensor_scalar(
                tmp[:], ao_sb[:, mu, :], s1_msa[:, mu:mu + 1], s2_msa[:, mu:mu + 1],
                op0=OP.mult, op1=OP.add)
            nc.vector.tensor_add(x2T[:, mu, :], tmp[:], xT[:, mu, :])

        m2_ps, r2_ps = ln_stats(x2T)
        h2T = pers.tile([P, KC, TT], BF16)
        for kc in range(KC):
            t0 = work.tile([P, TT], F32, tag="wf32", bufs=4)
            nc.vector.tensor_sub(t0[:], x2T[:, kc, :], m2_ps[:, 0:TT])
            t1 = work.tile([P, TT], F32, tag="wf32", bufs=4)
            nc.vector.tensor_tensor(t1[:], t0[:], r2_ps[:, 0:TT], op=OP.mult)
            nc.vector.tensor_scalar(
                h2T[:, kc, :], t1[:], sc1p_mlp[:, kc:kc + 1], mod_chunk(3, kc),
                op0=OP.mult, op1=OP.add)

        # ---------------- phase 7: MLP (token-sharded, fp8 weights streamed)
        gT = pers.tile([P, MLPH // P, TT], BF16)
        for nu in range(MLPH // P):
            w1 = wst.tile([P, KC, P], BF16, tag="w1", bufs=4)
            nc.sync.dma_start(w1[:], w_mlp1_b.ap()[nu])
            gps = pa()
            for kc in range(KC):
                nc.tensor.matmul(gps[:, 0:TT], w1[:, kc, :],
                                 h2T[:, kc, :],
                                 start=(kc == 0), stop=(kc == KC - 1))
            nc.scalar.activation(
                gT[:, nu, :], gps[:, 0:TT], AF.Gelu_apprx_tanh,
                bias=b_mlp1_sb[:, nu:nu + 1])
        for mu in range(KC):
            mps2 = pa()
            for hf in range(2):
                w2 = wst.tile([P, 16, P], BF16, tag="w2", bufs=2)
                nc.sync.dma_start(w2[:], w_mlp2_b.ap()[mu, :, ts(hf, 16), :])
                for kc in range(16):
                    gkc = 16 * hf + kc
                    nc.tensor.matmul(mps2[:, 0:TT], w2[:, kc, :],
                                     gT[:, gkc, :],
                                     start=(gkc == 0), stop=(gkc == MLPH // P - 1))
            tmp = work.tile([P, TT], BF16, tag="wbf", bufs=4)
            nc.vector.tensor_scalar(
                tmp[:], mps2[:, 0:TT], s1_mlp[:, mu:mu + 1], s2_mlp[:, mu:mu + 1],
                op0=OP.mult, op1=OP.add)
            outT = work.tile([P, TT], F32, tag="wf32", bufs=4)
            nc.vector.tensor_add(outT[:], tmp[:], x2T[:, mu, :])
            tps = pa()
            for r in range(TT // P):
                nc.tensor.transpose(tps[:, ts(r, P)], outT[:, ts(r, P)], ident[:])
            osb = work.tile([P, 512], F32, tag="osb", bufs=2)
            nc.vector.tensor_copy(osb[:], tps[:, 0:512])
            nc.sync.dma_start(
                out_t.ap()[:, ts(mu, P)].rearrange("(r p) m -> p r m", p=P),
                osb[:].rearrange("p (r m) -> p r m", r=4))

    nc.compile()
    return nc


# ---------------------------------------------------------------- runner
class SpmdRunner:
    def __init__(self, nc, n_cores, donate=True):
        install_neuronx_cc_hook()
        self.nc = nc
        self.n_cores = n_cores
        partition_name = nc.partition_id_tensor.name if nc.partition_id_tensor else None
        in_names, out_names, out_avals = [], [], []
        for alloc in nc.m.functions[0].allocations:
            if not isinstance(alloc, mybir.MemoryLocationSet):
                continue
            name = alloc.memorylocations[0].name
            if alloc.kind == "ExternalInput":
                if name != partition_name:
                    in_names.append(name)
            elif alloc.kind == "ExternalOutput":
                out_names.append(name)
                out_avals.append(
                    jax.core.ShapedArray(tuple(alloc.tensor_shape), mybir.dt.np(alloc.dtype))
                )
        self.in_names, self.out_names, self.out_avals = in_names, out_names, out_avals
        n_params = len(in_names)
        n_outs = len(out_avals)
        all_in_names = list(in_names) + list(out_names)
        if partition_name is not None:
            all_in_names.append(partition_name)

        def _body(*args):
            operands = list(args)
            if partition_name is not None:
                operands.append(partition_id_tensor())
            return tuple(
                _bass_exec_p.bind(
                    *operands,
                    out_avals=tuple(out_avals),
                    in_names=tuple(all_in_names),
                    out_names=tuple(out_names),
                    lowering_input_output_aliases=(),
                    sim_require_finite=True,
                    sim_require_nnan=True,
                    nc=nc,
                )
            )

        devices = jax.devices()[:n_cores]
        self.mesh = Mesh(np.asarray(devices), ("core",))
        donate_idx = tuple(range(n_params, n_params + n_outs)) if donate else ()
        self.fn = jax.jit(
            shard_map(
                _body,
                mesh=self.mesh,
                in_specs=(PartitionSpec("core"),) * (n_params + n_outs),
                out_specs=(PartitionSpec("core"),) * n_outs,
                check_rep=False,
            ),
            donate_argnums=donate_idx,
            keep_unused=True,
        )
        self.n_params, self.n_outs = n_params, n_outs

    def _concat_inputs(self, in_maps):
        return [
            np.concatenate([np.asarray(in_maps[c][n]) for c in range(self.n_cores)], axis=0)
            for n in self.in_names
        ]

    def run(self, in_maps):
        sharding = jax.sharding.NamedSharding(self.mesh, PartitionSpec("core"))
        concat_in = [
            jax.device_put(x, sharding) for x in self._concat_inputs(in_maps)
        ]
        zeros = [
            jax.device_put(
                np.zeros((self.n_cores * a.shape[0], *a.shape[1:]), a.dtype), sharding)
            for a in self.out_avals
        ]
        outs = self.fn(*concat_in, *zeros)
        return self._split(outs)

    def _split(self, out_arrs):
        return [
            {
                n: np.asarray(out_arrs[i]).reshape(self.n_cores, *self.out_avals[i].shape)[c]
                for i, n in enumerate(self.out_names)
            }
            for c in range(self.n_cores)
        ]

    def bench(self, in_maps, iters=30, warmup=3):
        """Chained repeated execution: output buffers of call i are donated as
        the output operands of call i+1, serializing calls on-device."""
        sharding = jax.sharding.NamedSharding(self.mesh, PartitionSpec("core"))
        concat_in = [jax.device_put(x, sharding) for x in self._concat_inputs(in_maps)]
        outs = tuple(
            jax.device_put(
                np.zeros((self.n_cores * a.shape[0], *a.shape[1:]), a.dtype), sharding)
            for a in self.out_avals
        )
        for _ in range(warmup):
            outs = self.fn(*concat_in, *outs)
        jax.block_until_ready(outs)
        t0 = time.perf_counter()
        for _ in range(iters):
            outs = self.fn(*concat_in, *outs)
        jax.block_until_ready(outs)
        t1 = time.perf_counter()
        return (t1 - t0) / iters, self._split(outs)


_CACHE = {}


def kernel(**inputs):
    """Full-input DiT block on 8 NeuronCores; returns full [B, N, HID] f32."""
    if "nc" not in _CACHE:
        _CACHE["nc"] = build_kernel()
        _CACHE["runner"] = SpmdRunner(_CACHE["nc"], 8)
    maps = make_in_maps(inputs)
    results = _CACHE["runner"].run(maps)
    return assemble_output(results)


# revision 20
# speedup vs baseline: 1.2453x; 1.0175x over previous
"""DiT block Bass kernel for 8 TRN2 NeuronCores.

Core i -> (b = i//4, g = i%4): batch item b; head group 4g..4g+3; token
quarter [512g, 512g+512) of batch b.  Activations are hidden-major
([hidden_chunk=128, tokens]) throughout; PE transposes at entry (x) and
exit (out).  Collectives: AllGather(4) for mod + h, ReduceScatter(4)
for the out-projection partials.

Dtype strategy: residual stream bf16; weights host-quantized (w_qkv,
w_out, w_mlp1, w_mlp2 in fp8e4m3 prescaled by 32; w_ada bf16); all big
GEMMs except QK^T run in fp8 DoubleRow perf mode (2 k-subtiles of 128
per pass).  Relative attention bias: constant for |d| >= 91, so
off-band score tiles get their bias via the exp() bias operand; band
tiles get a PE matmul-add of pretransposed bias tiles into PSUM before
exp.  Softmax is computed without max-subtraction (scores provably
small); denominators come from an appended ones-row in the fp8 V tiles.
"""
import contextlib
import time
import numpy as np
import ml_dtypes
import jax
from jax.sharding import Mesh, PartitionSpec
from jax.experimental.shard_map import shard_map

import concourse.bass as bass
import concourse.mybir as mybir
import concourse.tile as tile
from concourse import bacc
from concourse.bass2jax import _bass_exec_p, install_neuronx_cc_hook, partition_id_tensor

F32 = mybir.dt.float32
BF16 = mybir.dt.bfloat16
FP8 = mybir.dt.float8e4
AF = mybir.ActivationFunctionType
OP = mybir.AluOpType
DR = mybir.MatmulPerfMode.DoubleRow
ts = bass.ts

NPBF16 = ml_dtypes.bfloat16
NPFP8 = ml_dtypes.float8_e4m3

B, N, HID = 2, 2048, 1024
NH, HD = 16, 64
MLPH = 4 * HID
NB, MAXD = 32, 128
P = 128
TT = 512
KC = HID // P          # 8
NBLK = N // P          # 16
WS = 32.0              # host weight prescale for fp8
CXS = 64.0             # ctx prescale for fp8
RG4 = [[0, 1, 2, 3], [4, 5, 6, 7]]


# ---------------------------------------------------------------- host prep
def rel_bucket_np(d):
    nb = NB // 2
    buckets = np.where(d > 0, nb, 0).astype(np.int64)
    rp = np.abs(d)
    max_exact = nb // 2
    is_small = rp < max_exact
    log_ratio = np.log(np.maximum(rp, 1).astype(np.float32) / np.float32(max_exact))
    rpl = max_exact + (
        log_ratio / np.float32(np.log(MAXD / max_exact)) * (nb - max_exact)
    ).astype(np.int32)
    rpl = np.minimum(rpl, nb - 1)
    return buckets + np.where(is_small, rp, rpl)


def make_bias_tables(rel_table, g):
    """Band bias tiles + deltas for local heads 4g..4g+3.

    bt[h, j][p, m] = 8 * badj(128*(j-1) + m - p)   (j = koff+1, koff in -1..1)
    badj(d) = bias(d) - bias_minus;  delta = bias_plus - bias_minus.
    The 8x prescale compensates the 0.125 exp scale (bias added in PSUM
    pre-scale, exp bias arg applied post-scale).
    """
    d = np.arange(-(N - 1), N)
    buck = rel_bucket_np(d)  # index by d + N-1
    bt = np.zeros((4, 3, P, P), np.float32)
    delta = np.zeros((4,), np.float32)
    for hl in range(4):
        hg = 4 * g + hl
        bvec = rel_table[:, hg].astype(np.float32)
        bmin = bvec[NB // 2 - 1]
        delta[hl] = bvec[NB - 1] - bmin
        diag = bvec[buck] - bmin  # badj over d in [-(N-1), N-1]
        p = np.arange(P)[:, None]
        m = np.arange(P)[None, :]
        for j, koff in enumerate((-1, 0, 1)):
            dd = 128 * koff + m - p
            bt[hl, j] = 8.0 * diag[dd + (N - 1)]
    return bt.astype(NPBF16), delta


def make_in_maps(inputs):
    x = np.asarray(inputs["x"], np.float32)
    c = np.asarray(inputs["c"], np.float32)
    w_ada = np.asarray(inputs["w_ada"], np.float32)
    b_ada = np.asarray(inputs["b_ada"], np.float32)
    w_qkv = np.asarray(inputs["w_qkv"], np.float32)
    b_qkv = np.asarray(inputs["b_qkv"], np.float32)
    w_out = np.asarray(inputs["w_out"], np.float32)
    b_out = np.asarray(inputs["b_out"], np.float32)
    rel_table = np.asarray(inputs["rel_table"], np.float32)
    w_mlp1 = np.asarray(inputs["w_mlp1"], np.float32)
    b_mlp1 = np.asarray(inputs["b_mlp1"], np.float32)
    w_mlp2 = np.asarray(inputs["w_mlp2"], np.float32)
    b_mlp2 = np.asarray(inputs["b_mlp2"], np.float32)

    ident = np.eye(P, dtype=np.float32)
    ident_b = np.eye(P, dtype=np.float32).astype(NPBF16)
    ones_col_b = np.ones((P, 1), np.float32).astype(NPBF16)
    ones_row_b = np.ones((1, P), np.float32).astype(NPBF16)
    ones64_row = np.full((1, 64), CXS, np.float32).astype(NPBF16)

    w_mlp1_b = np.ascontiguousarray(
        w_mlp1.reshape(KC, P, MLPH // P, P).transpose(2, 1, 0, 3)
        .astype(NPBF16))                      # [32, P, kc8, P]
    w_mlp2_b = np.ascontiguousarray(
        w_mlp2.reshape(MLPH // P, P, KC, P).transpose(2, 1, 0, 3)
        .astype(NPBF16))                      # [8, P, kc32, P]

    maps = []
    for i in range(8):
        b, g = divmod(i, 4)
        qs, ks, vs = 256 * g, HID + 256 * g, 2 * HID + 256 * g
        w_qk = np.concatenate([w_qkv[:, qs:qs + 256], w_qkv[:, ks:ks + 256]], 1)
        w_v = w_qkv[:, vs:vs + 256]
        b_qk = np.concatenate([b_qkv[qs:qs + 256], b_qkv[ks:ks + 256]])
        bv = b_qkv[vs:vs + 256]

        # [P, mu4, kp4, 2, P]: global k = 128*(2*kp+sub) + p, out chunk mu
        w_qk_q = (w_qk * WS).reshape(4, 2, P, 4, P).transpose(
            2, 3, 0, 1, 4).astype(NPFP8)
        # [P, kp4, 2, 256]
        w_v_q = (w_v * WS).reshape(4, 2, P, 256).transpose(2, 0, 1, 3).astype(NPFP8)
        # [P, 2, HID]: ctx chunk-major (2 chunks of own 256 ctx dims)
        w_out_q = (w_out[256 * g:256 * (g + 1), :] * WS).reshape(
            2, P, HID).transpose(1, 0, 2).astype(NPFP8)

        bt, delta = make_bias_tables(rel_table, g)
        maps.append({
            "x_own": np.ascontiguousarray(x[b, 512 * g:512 * (g + 1), :]),
            "c_own": np.ascontiguousarray(c[b][:, None]),
            "w_ada_s": np.ascontiguousarray(
                w_ada[:, 1536 * g:1536 * (g + 1)].reshape(KC, P, 12, P)
                .transpose(2, 1, 0, 3).astype(NPBF16)),
            "b_ada_s": np.ascontiguousarray(
                b_ada[1536 * g:1536 * (g + 1)].reshape(12, P).T),
            "w_qk_q": np.ascontiguousarray(w_qk_q),
            "w_v_q": np.ascontiguousarray(w_v_q),
            "b_qk_s": np.ascontiguousarray(b_qk.reshape(4, P).T),
            "b_v_bcast": np.ascontiguousarray(
                np.broadcast_to(bv[None, :], (P, 256)).astype(NPBF16)),
            "w_out_q": np.ascontiguousarray(w_out_q),
            "b_out_r": np.ascontiguousarray(b_out.reshape(KC, P).T),
            "w_mlp1_b": w_mlp1_b,
            "b_mlp1_r": np.ascontiguousarray(b_mlp1.reshape(MLPH // P, P).T),
            "w_mlp2_b": w_mlp2_b,
            "b_mlp2_r": np.ascontiguousarray(b_mlp2.reshape(KC, P).T),
            "bt": np.ascontiguousarray(bt.reshape(12, P, P)),
            "delta_row8": np.ascontiguousarray(
                np.broadcast_to((8.0 * delta)[None, :, None], (1, 4, P))
                .astype(NPBF16)),
            "delta_col": np.ascontiguousarray(
                np.broadcast_to(delta[None, :], (P, 4)).astype(np.float32)),
            "ident": ident,
            "ident_b": ident_b,
            "ones_col_b": ones_col_b,
            "ones_row_b": ones_row_b,
            "ones64_row": ones64_row,
        })
    return maps


def assemble_output(results):
    out = np.zeros((B, N, HID), np.float32)
    for i in range(8):
        b, g = divmod(i, 4)
        out[b, 512 * g:512 * (g + 1), :] = results[i]["out"]
    return out


# ---------------------------------------------------------------- builder
def build_kernel(sim=False):
    nc = bacc.Bacc("TRN2", target_bir_lowering=False, debug=False, num_devices=8)

    din = lambda nm, sh, dt=F32: nc.dram_tensor(nm, sh, dt, kind="ExternalInput")
    x_own = din("x_own", [TT, HID])
    c_own = din("c_own", [HID, 1])
    w_ada_s = din("w_ada_s", [12, P, KC, P], BF16)
    b_ada_s = din("b_ada_s", [P, 12])
    w_qk_q = din("w_qk_q", [P, 4, 4, 2, P], FP8)
    w_v_q = din("w_v_q", [P, 4, 2, 256], FP8)
    b_qk_s = din("b_qk_s", [P, 4])
    b_v_bcast = din("b_v_bcast", [P, 256], BF16)
    w_out_q = din("w_out_q", [P, 2, HID], FP8)
    b_out_r = din("b_out_r", [P, KC])
    w_mlp1_b = din("w_mlp1_b", [MLPH // P, P, KC, P], BF16)
    b_mlp1_r = din("b_mlp1_r", [P, MLPH // P])
    w_mlp2_b = din("w_mlp2_b", [KC, P, MLPH // P, P], BF16)
    b_mlp2_r = din("b_mlp2_r", [P, KC])
    bt_in = din("bt", [12, P, P], BF16)
    delta_row8_in = din("delta_row8", [1, 4, P], BF16)
    delta_col_in = din("delta_col", [P, 4])
    ident_in = din("ident", [P, P])
    ident_b_in = din("ident_b", [P, P], BF16)
    ones_col_b_in = din("ones_col_b", [P, 1], BF16)
    ones_row_b_in = din("ones_row_b", [1, P], BF16)
    ones64_row_in = din("ones64_row", [1, 64], BF16)

    out_t = nc.dram_tensor("out", [TT, HID], F32, kind="ExternalOutput")

    with tile.TileContext(nc) as tc, contextlib.ExitStack() as ctx:
        const = ctx.enter_context(tc.tile_pool(name="const", bufs=1))
        pers = ctx.enter_context(tc.tile_pool(name="pers", bufs=1))
        work = ctx.enter_context(tc.tile_pool(name="work", bufs=3))
        wst = ctx.enter_context(tc.tile_pool(name="wst", bufs=2))
        dram = ctx.enter_context(tc.tile_pool(name="dram", bufs=1, space="DRAM"))
        ps_a = ctx.enter_context(tc.tile_pool(name="ps_a", bufs=2, space="PSUM"))
        ps_c = ctx.enter_context(tc.tile_pool(name="ps_c", bufs=2, space="PSUM"))

        def pa():
            return ps_a.tile([P, 1024], F32, tag="A", name="pa")

        def pc():
            return ps_c.tile([72, 1024], F32, tag="C", name="pc")

        # ---------------- constants
        ident = const.tile([P, P], F32)
        nc.sync.dma_start(ident[:], ident_in.ap())
        ident_b = const.tile([P, P], BF16)
        nc.sync.dma_start(ident_b[:], ident_b_in.ap())
        ones_col_b = const.tile([P, 1], BF16)
        nc.sync.dma_start(ones_col_b[:], ones_col_b_in.ap())
        ones_row_b = const.tile([1, P], BF16)
        nc.sync.dma_start(ones_row_b[:], ones_row_b_in.ap())
        ones64_row = const.tile([1, 64], BF16)
        nc.sync.dma_start(ones64_row[:], ones64_row_in.ap())
        b_qk_sb = const.tile([P, 4], F32)
        nc.sync.dma_start(b_qk_sb[:], b_qk_s.ap())
        b_v_sb = const.tile([P, 256], BF16)
        nc.sync.dma_start(b_v_sb[:], b_v_bcast.ap())
        b_out_sb = const.tile([P, KC], F32)
        nc.sync.dma_start(b_out_sb[:], b_out_r.ap())
        b_mlp1_sb = const.tile([P, MLPH // P], F32)
        nc.sync.dma_start(b_mlp1_sb[:], b_mlp1_r.ap())
        b_mlp2_sb = const.tile([P, KC], F32)
        nc.sync.dma_start(b_mlp2_sb[:], b_mlp2_r.ap())
        b_ada_sb = const.tile([P, 12], F32)
        nc.sync.dma_start(b_ada_sb[:], b_ada_s.ap())
        bt_sb = const.tile([P, 12, P], BF16)
        delta_row8 = const.tile([1, 4, P], BF16)
        nc.sync.dma_start(delta_row8[:], delta_row8_in.ap())
        delta_col = const.tile([P, 4], F32)
        nc.sync.dma_start(delta_col[:], delta_col_in.ap())
        wqk_sb = const.tile([P, 4, 4, 2, P], FP8)
        wv_sb = const.tile([P, 4, 2, 256], FP8)
        wout_sb = const.tile([P, 2, HID], FP8)
        eps_sb = const.tile([1, 1], F32)
        nc.vector.memset(eps_sb[:], 1e-6)

        # ---------------- phase 0: mod shard (this core: w_ada cols 1536g..)
        cT_sb = pers.tile([P, KC], F32)
        nc.sync.dma_start(cT_sb[:], c_own.ap().rearrange("(c p) o -> p (c o)", p=P))
        silu_sb = pers.tile([P, KC], BF16)
        nc.scalar.activation(silu_sb[:], cT_sb[:], AF.Silu)
        mod_sh_sb = pers.tile([P, 12], F32)
        for mu in range(12):
            wa = wst.tile([P, KC, P], BF16, tag="wada", bufs=4)
            nc.sync.dma_start(wa[:], w_ada_s.ap()[mu])
            mps = pa()
            for kc in range(KC):
                nc.tensor.matmul(mps[:, 0:1], wa[:, kc, :], silu_sb[:, kc:kc + 1],
                                 start=(kc == 0), stop=(kc == KC - 1))
            nc.vector.tensor_scalar_add(
                mod_sh_sb[:, mu:mu + 1], mps[:, 0:1], b_ada_sb[:, mu:mu + 1])
        mod_bounce_in = dram.tile([P, 12], F32)
        nc.sync.dma_start(mod_bounce_in[:], mod_sh_sb[:])
        mod_bounce_out = dram.tile([4 * P, 12], F32)
        if sim:
            nc.sync.dma_start(mod_bounce_out[:][0:P, :], mod_bounce_in[:])
        else:
            nc.gpsimd.collective_compute(
                "AllGather", OP.bypass, replica_groups=RG4,
                ins=[mod_bounce_in.opt()], outs=[mod_bounce_out.opt()])
        mod_sb = pers.tile([P, 4, 12], F32)
        nc.sync.dma_start(
            mod_sb[:], mod_bounce_out[:].rearrange("(g p) j -> p g j", p=P))

        def mod_chunk(vec_idx, kc):
            gc = 8 * vec_idx + kc
            return mod_sb[:, gc // 12, gc % 12:gc % 12 + 1]

        sc1p_msa = pers.tile([P, KC], F32)
        sc1p_mlp = pers.tile([P, KC], F32)
        s1_msa = pers.tile([P, KC], F32)
        s2_msa = pers.tile([P, KC], F32)
        s1_mlp = pers.tile([P, KC], F32)
        s2_mlp = pers.tile([P, KC], F32)
        for kc in range(KC):
            nc.vector.tensor_scalar_add(sc1p_msa[:, kc:kc + 1], mod_chunk(1, kc), 1.0)
            nc.vector.tensor_scalar_add(sc1p_mlp[:, kc:kc + 1], mod_chunk(4, kc), 1.0)
            nc.vector.tensor_scalar_mul(
                s1_msa[:, kc:kc + 1], mod_chunk(2, kc), 1.0 / (WS * CXS))
            nc.vector.tensor_tensor(
                s2_msa[:, kc:kc + 1], mod_chunk(2, kc), b_out_sb[:, kc:kc + 1],
                op=OP.mult)
            nc.vector.tensor_scalar_mul(
                s1_mlp[:, kc:kc + 1], mod_chunk(5, kc), 1.0)
            nc.vector.tensor_tensor(
                s2_mlp[:, kc:kc + 1], mod_chunk(5, kc), b_mlp2_sb[:, kc:kc + 1],
                op=OP.mult)

        # ---------------- phase 1: xT (bf16) via PE transpose
        xT = pers.tile([P, KC, TT], BF16)
        for r in range(TT // P):
            x_sb = work.tile([P, HID], F32, tag="xrow", bufs=3)
            nc.sync.dma_start(x_sb[:], x_own.ap()[ts(r, P), :])
            for half in range(2):
                tps = pa()
                for k4 in range(4):
                    nc.tensor.transpose(tps[:, ts(k4, P)],
                                        x_sb[:, ts(4 * half + k4, P)], ident[:])
                nc.vector.tensor_copy(
                    xT[:, 4 * half:4 * half + 4, ts(r, P)],
                    tps[:, 0:512].rearrange("p (c t) -> p c t", c=4))

        def ln_stats(src):
            """src: [P, KC, TT] bf16 -> (m_bc, r_bc) [P, TT] f32 PSUM tiles."""
            sum_c = pc()
            for kc in range(KC):
                nc.tensor.matmul(sum_c[0:1, 0:TT], ones_col_b[:], src[:, kc, :],
                                 start=(kc == 0), stop=(kc == KC - 1))
            sumsq_c = pc()
            for kc in range(KC):
                sq = work.tile([P, TT], BF16, tag="sq", bufs=3)
                nc.gpsimd.tensor_tensor(sq[:], src[:, kc, :], src[:, kc, :],
                                        op=OP.mult)
                nc.tensor.matmul(sumsq_c[0:1, 0:TT], ones_col_b[:], sq[:],
                                 start=(kc == 0), stop=(kc == KC - 1))
            m_row = work.tile([1, TT], BF16, tag="rowtmp", bufs=4)
            nc.vector.tensor_scalar_mul(m_row[:], sum_c[0:1, 0:TT], 1.0 / HID)
            msq = work.tile([1, TT], BF16, tag="rowtmp", bufs=4)
            nc.vector.tensor_tensor(msq[:], m_row[:], m_row[:], op=OP.mult)
            var_row = work.tile([1, TT], F32, tag="rowtmp", bufs=4)
            nc.vector.scalar_tensor_tensor(
                var_row[:], sumsq_c[0:1, 0:TT], 1.0 / HID, msq[:],
                op0=OP.mult, op1=OP.subtract)
            sd_row = work.tile([1, TT], F32, tag="rowtmp", bufs=4)
            nc.scalar.activation(sd_row[:], var_row[:], AF.Sqrt, bias=eps_sb[:])
            r_row = work.tile([1, TT], BF16, tag="rowtmp", bufs=4)
            with nc.allow_low_precision(reason="rstd in bf16 is plenty for LN"):
                nc.vector.reciprocal(r_row[:], sd_row[:])
            m_ps = pa()
            nc.tensor.matmul(m_ps[:, 0:TT], ones_row_b[:], m_row[:],
                             start=True, stop=True)
            r_ps = pa()
            nc.tensor.matmul(r_ps[:, 0:TT], ones_row_b[:], r_row[:],
                             start=True, stop=True)
            return m_ps, r_ps

        # ---------------- phase 2: hT own (fp8) + AllGather
        m_ps, r_ps = ln_stats(xT)
        hT_own = pers.tile([P, KC, TT], FP8)
        for kc in range(KC):
            t0 = work.tile([P, TT], F32, tag="wf32", bufs=4)
            nc.vector.tensor_sub(t0[:], xT[:, kc, :], m_ps[:, 0:TT])
            t1 = work.tile([P, TT], F32, tag="wf32", bufs=4)
            nc.vector.tensor_tensor(t1[:], t0[:], r_ps[:, 0:TT], op=OP.mult)
            nc.vector.tensor_scalar(
                hT_own[:, kc, :], t1[:], sc1p_msa[:, kc:kc + 1], mod_chunk(0, kc),
                op0=OP.mult, op1=OP.add)
        h_bounce_in_a = dram.tile([HID // 2, TT], FP8)
        h_bounce_in_b = dram.tile([HID // 2, TT], FP8)
        nc.sync.dma_start(
            h_bounce_in_a[:].rearrange("(c p) t -> p c t", p=P), hT_own[:, 0:4, :])
        nc.sync.dma_start(
            h_bounce_in_b[:].rearrange("(c p) t -> p c t", p=P), hT_own[:, 4:8, :])
        h_bounce_out_a = dram.tile([2 * HID, TT], FP8)
        h_bounce_out_b = dram.tile([2 * HID, TT], FP8)
        if sim:
            nc.sync.dma_start(h_bounce_out_a[:][0:HID // 2, :], h_bounce_in_a[:])
            nc.sync.dma_start(h_bounce_out_b[:][0:HID // 2, :], h_bounce_in_b[:])
        else:
            nc.gpsimd.collective_compute(
                "AllGather", OP.bypass, replica_groups=RG4,
                ins=[h_bounce_in_a.opt()], outs=[h_bounce_out_a.opt()])
            nc.gpsimd.collective_compute(
                "AllGather", OP.bypass, replica_groups=RG4,
                ins=[h_bounce_in_b.opt()], outs=[h_bounce_out_b.opt()])
        nc.sync.dma_start(wv_sb[:], w_v_q.ap())
        nc.sync.dma_start(wqk_sb[:], w_qk_q.ap())
        nc.sync.dma_start(wout_sb[:], w_out_q.ap())
        nc.sync.dma_start(bt_sb[:], bt_in.ap().rearrange("j p m -> p j m"))
        hT_full = pers.tile([P, 32, TT], FP8)
        for jq in range(4):
            nc.sync.dma_start(
                hT_full[:, KC * jq:KC * jq + 4, :],
                h_bounce_out_a[:][ts(jq, HID // 2), :].rearrange("(c p) t -> p c t", p=P))
            nc.sync.dma_start(
                hT_full[:, KC * jq + 4:KC * jq + 8, :],
                h_bounce_out_b[:][ts(jq, HID // 2), :].rearrange("(c p) t -> p c t", p=P))

        # ---------------- phase 3: qkv (fp8 DoubleRow)
        qT = pers.tile([P, 2, N], BF16)
        kT = pers.tile([P, 2, N], BF16)
        v_aug = pers.tile([P, NBLK, 288], FP8)
        nc.vector.memset(v_aug[:], 0.0)
        nc.vector.memset(
            v_aug[:].rearrange("p b (h e) -> p b h e", h=4)[:, :, :, 64:65], 1.0)

        for blk in range(NBLK):
            jq, tb = blk // 4, blk % 4
            vps = pa()
            for kp in range(4):
                nc.tensor.matmul(
                    vps[:, 0:256],
                    hT_full[:, KC * jq + 2 * kp:KC * jq + 2 * kp + 2, ts(tb, P)],
                    wv_sb[:, kp, :, :],
                    start=(kp == 0), stop=(kp == 3), perf_mode=DR)
            nc.vector.scalar_tensor_tensor(
                v_aug[:, blk, :].rearrange("p (h e) -> p h e", h=4)[:, :, 0:64],
                vps[:, 0:256].rearrange("p (h e) -> p h e", h=4), 1.0 / WS,
                b_v_sb[:].rearrange("p (h e) -> p h e", h=4),
                op0=OP.mult, op1=OP.add)

        for mu in range(4):       # q chunks 0,1; k chunks 2,3
            for jq in range(4):
                qps = pa()
                for kp in range(4):
                    nc.tensor.matmul(
                        qps[:, 0:TT],
                        wqk_sb[:, mu, kp, :, :],
                        hT_full[:, KC * jq + 2 * kp:KC * jq + 2 * kp + 2, :],
                        start=(kp == 0), stop=(kp == 3), perf_mode=DR)
                dst = qT if mu < 2 else kT
                nc.vector.tensor_scalar(
                    dst[:, mu % 2, ts(jq, TT)], qps[:, 0:TT], 1.0 / WS,
                    b_qk_sb[:, mu:mu + 1], op0=OP.mult, op1=OP.add)

        # ---------------- phase 4: attention
        # heads: local h = 2a + o; scores [128 keys, 1024 queries] per
        # (a, o, qspan, blk); bias koff = blk - 8*qspan - s per 128-col
        # sub-block s.
        ctxT = pers.tile([P, 2, N], FP8)
        rs_bounce_in = dram.tile([4 * HID, TT], BF16)
        pending = []

        def emit_outproj(tau, mu):
            ops_ = pa()
            nc.tensor.matmul(
                ops_[:, 0:TT], wout_sb[:, :, ts(mu, P)],
                ctxT[:, :, ts(tau, TT)],
                start=True, stop=True, perf_mode=DR)
            po = work.tile([P, TT], BF16, tag="po", bufs=3)
            nc.vector.tensor_copy(po[:], ops_[:, 0:TT])
            nc.sync.dma_start(
                rs_bounce_in[:][tau * HID + mu * P:tau * HID + (mu + 1) * P, :],
                po[:])

        for qspan in range(2):
            for a in range(2):
                cps = [pc(), pc()]
                for bp in range(NBLK // 2):
                    esb = [
                        work.tile([P, 2, 1024], FP8, tag=f"esb{o}", bufs=2,
                                  name=f"esb{o}")
                        for o in range(2)
                    ]
                    for sub in range(2):
                        blk = 2 * bp + sub
                        koff0 = blk - 8 * qspan
                        for o in range(2):
                            h = 2 * a + o
                            sc = pa()
                            for half in range(2):
                                nc.tensor.matmul(
                                    sc[:, ts(half, TT)],
                                    kT[64 * o:64 * o + 64, a, ts(blk, P)],
                                    qT[64 * o:64 * o + 64, a,
                                       1024 * qspan + 512 * half:
                                       1024 * qspan + 512 * (half + 1)],
                                    start=True, stop=True)
                            bias_arg = 0.0
                            if koff0 >= 9:
                                bias_arg = delta_col[:, h:h + 1]
                            elif koff0 >= -1:
                                for s in range(8):
                                    koff = koff0 - s
                                    if -1 <= koff <= 1:
                                        nc.tensor.matmul(
                                            sc[:, ts(s, P)],
                                            bt_sb[:, 3 * h + koff + 1, :],
                                            ident_b[:],
                                            start=False, stop=True,
                                            skip_group_check=True)
                                    elif koff >= 2:
                                        nc.tensor.matmul(
                                            sc[:, ts(s, P)],
                                            delta_row8[0:1, h, :],
                                            ones_row_b[:],
                                            start=False, stop=True,
                                            skip_group_check=True)
                            nc.scalar.activation(
                                esb[o][:, sub, :], sc[:], AF.Exp,
                                scale=0.125, bias=bias_arg)
                    for o in range(2):
                        h = 2 * a + o
                        for half in range(2):
                            nc.tensor.matmul(
                                cps[o][:, ts(half, TT)],
                                v_aug[:, 2 * bp:2 * bp + 2, 72 * h:72 * h + 72],
                                esb[o][:, :, ts(half, TT)],
                                start=(bp == 0), stop=(bp == NBLK // 2 - 1),
                                perf_mode=DR)
                    if pending:
                        emit_outproj(*pending.pop(0))
                for o in range(2):
                    recip = work.tile([1, 1024], BF16, tag="recip", bufs=2)
                    with nc.allow_low_precision(reason="softmax denom recip bf16"):
                        nc.vector.reciprocal(recip[:], cps[o][64:65, :])
                    csb = work.tile([64, 1024], BF16, tag="csb", bufs=2)
                    nc.vector.tensor_copy(csb[:], cps[o][0:64, :])
                    bc = pc()
                    for half in range(2):
                        nc.tensor.matmul(
                            bc[0:64, ts(half, TT)], ones64_row[:],
                            recip[0:1, ts(half, TT)], start=True, stop=True)
                    nc.vector.tensor_tensor(
                        ctxT[64 * o:64 * o + 64, a, ts(qspan, 1024)],
                        csb[:], bc[0:64, :], op=OP.mult)
            # queue this qspan's out-proj partials; they drain inside the
            # next qspan's Act-bound block loop (16 items, 16 iterations)
            pending += [(tau, mu) for tau in (2 * qspan, 2 * qspan + 1)
                        for mu in range(KC)]

        for tau, mu in pending:
            emit_outproj(tau, mu)

        # ---------------- phase 5: ReduceScatter(add)
        rs_bounce_out = dram.tile([HID, TT], BF16)
        if sim:
            nc.sync.dma_start(rs_bounce_out[:], rs_bounce_in[:][0:HID, :])
        else:
            nc.gpsimd.collective_compute(
                "ReduceScatter", OP.add, replica_groups=RG4,
                ins=[rs_bounce_in.opt()], outs=[rs_bounce_out.opt()])
        ao_sb = pers.tile([P, KC, TT], BF16)
        nc.sync.dma_start(
            ao_sb[:], rs_bounce_out[:].rearrange("(c p) t -> p c t", p=P))

        # ---------------- phase 6: residual + LN2
        x2T = pers.tile([P, KC, TT], BF16)
        for mu in range(KC):
            tmp = work.tile([P, TT], BF16, tag="wbf", bufs=4)
            nc.vector.tensor_scalar(
                tmp[:], ao_sb[:, mu, :], s1_msa[:, mu:mu + 1], s2_msa[:, mu:mu + 1],
                op0=OP.mult, op1=OP.add)
            nc.vector.tensor_add(x2T[:, mu, :], tmp[:], xT[:, mu, :])

        m2_ps, r2_ps = ln_stats(x2T)
        h2T = pers.tile([P, KC, TT], BF16)
        for kc in range(KC):
            t0 = work.tile([P, TT], F32, tag="wf32", bufs=4)
            nc.vector.tensor_sub(t0[:], x2T[:, kc, :], m2_ps[:, 0:TT])
            t1 = work.tile([P, TT], F32, tag="wf32", bufs=4)
            nc.vector.tensor_tensor(t1[:], t0[:], r2_ps[:, 0:TT], op=OP.mult)
            nc.vector.tensor_scalar(
                h2T[:, kc, :], t1[:], sc1p_mlp[:, kc:kc + 1], mod_chunk(3, kc),
                op0=OP.mult, op1=OP.add)

        # ---------------- phase 7: MLP (token-sharded, fp8 weights streamed)
        gT = pers.tile([P, MLPH // P, TT], BF16)
        for nu in range(MLPH // P):
            w1 = wst.tile([P, KC, P], BF16, tag="w1", bufs=4)
            nc.sync.dma_start(w1[:], w_mlp1_b.ap()[nu])
            gps = pa()
            for kc in range(KC):
                nc.tensor.matmul(gps[:, 0:TT], w1[:, kc, :],
                                 h2T[:, kc, :],
                                 start=(kc == 0), stop=(kc == KC - 1))
            nc.scalar.activation(
                gT[:, nu, :], gps[:, 0:TT], AF.Gelu_apprx_tanh,
                bias=b_mlp1_sb[:, nu:nu + 1])
        for mu in range(KC):
            mps2 = pa()
            for hf in range(2):
                w2 = wst.tile([P, 16, P], BF16, tag="w2", bufs=2)
                nc.sync.dma_start(w2[:], w_mlp2_b.ap()[mu, :, ts(hf, 16), :])
                for kc in range(16):
                    gkc = 16 * hf + kc
                    nc.tensor.matmul(mps2[:, 0:TT], w2[:, kc, :],
                                     gT[:, gkc, :],
                                     start=(gkc == 0), stop=(gkc == MLPH // P - 1))
            tmp = work.tile([P, TT], BF16, tag="wbf", bufs=4)
            nc.vector.tensor_scalar(
                tmp[:], mps2[:, 0:TT], s1_mlp[:, mu:mu + 1], s2_mlp[:, mu:mu + 1],
                op0=OP.mult, op1=OP.add)
            outT = work.tile([P, TT], F32, tag="wf32", bufs=4)
            nc.vector.tensor_add(outT[:], tmp[:], x2T[:, mu, :])
            tps = pa()
            for r in range(TT // P):
                nc.tensor.transpose(tps[:, ts(r, P)], outT[:, ts(r, P)], ident[:])
            osb = work.tile([P, 512], F32, tag="osb", bufs=2)
            nc.vector.tensor_copy(osb[:], tps[:, 0:512])
            nc.sync.dma_start(
                out_t.ap()[:, ts(mu, P)].rearrange("(r p) m -> p r m", p=P),
                osb[:].rearrange("p (r m) -> p r m", r=4))

    nc.compile()
    return nc


# ---------------------------------------------------------------- runner
class SpmdRunner:
    def __init__(self, nc, n_cores, donate=True):
        install_neuronx_cc_hook()
        self.nc = nc
        self.n_cores = n_cores
        partition_name = nc.partition_id_tensor.name if nc.partition_id_tensor else None
        in_names, out_names, out_avals = [], [], []
        for alloc in nc.m.functions[0].allocations:
            if not isinstance(alloc, mybir.MemoryLocationSet):
                continue
            name = alloc.memorylocations[0].name
            if alloc.kind == "ExternalInput":
                if name != partition_name:
                    in_names.append(name)
            elif alloc.kind == "ExternalOutput":
                out_names.append(name)
                out_avals.append(
                    jax.core.ShapedArray(tuple(alloc.tensor_shape), mybir.dt.np(alloc.dtype))
                )
        self.in_names, self.out_names, self.out_avals = in_names, out_names, out_avals
        n_params = len(in_names)
        n_outs = len(out_avals)
        all_in_names = list(in_names) + list(out_names)
        if partition_name is not None:
            all_in_names.append(partition_name)

        def _body(*args):
            operands = list(args)
            if partition_name is not None:
                operands.append(partition_id_tensor())
            return tuple(
                _bass_exec_p.bind(
                    *operands,
                    out_avals=tuple(out_avals),
                    in_names=tuple(all_in_names),
                    out_names=tuple(out_names),
                    lowering_input_output_aliases=(),
                    sim_require_finite=True,
                    sim_require_nnan=True,
                    nc=nc,
                )
            )

        devices = jax.devices()[:n_cores]
        self.mesh = Mesh(np.asarray(devices), ("core",))
        donate_idx = tuple(range(n_params, n_params + n_outs)) if donate else ()
        self.fn = jax.jit(
            shard_map(
                _body,
                mesh=self.mesh,
                in_specs=(PartitionSpec("core"),) * (n_params + n_outs),
                out_specs=(PartitionSpec("core"),) * n_outs,
                check_rep=False,
            ),
            donate_argnums=donate_idx,
            keep_unused=True,
        )
        self.n_params, self.n_outs = n_params, n_outs

    def _concat_inputs(self, in_maps):
        return [
            np.concatenate([np.asarray(in_maps[c][n]) for c in range(self.n_cores)], axis=0)
            for n in self.in_names
        ]

    def run(self, in_maps):
        sharding = jax.sharding.NamedSharding(self.mesh, PartitionSpec("core"))
        concat_in = [
            jax.device_put(x, sharding) for x in self._concat_inputs(in_maps)
        ]
        zeros = [
            jax.device_put(
                np.zeros((self.n_cores * a.shape[0], *a.shape[1:]), a.dtype), sharding)
            for a in self.out_avals
        ]
        outs = self.fn(*concat_in, *zeros)
        return self._split(outs)

    def _split(self, out_arrs):
        return [
            {
                n: np.asarray(out_arrs[i]).reshape(self.n_cores, *self.out_avals[i].shape)[c]
                for i, n in enumerate(self.out_names)
            }
            for c in range(self.n_cores)
        ]

    def bench(self, in_maps, iters=30, warmup=3):
        """Chained repeated execution: output buffers of call i are donated as
        the output operands of call i+1, serializing calls on-device."""
        sharding = jax.sharding.NamedSharding(self.mesh, PartitionSpec("core"))
        concat_in = [jax.device_put(x, sharding) for x in self._concat_inputs(in_maps)]
        outs = tuple(
            jax.device_put(
                np.zeros((self.n_cores * a.shape[0], *a.shape[1:]), a.dtype), sharding)
            for a in self.out_avals
        )
        for _ in range(warmup):
            outs = self.fn(*concat_in, *outs)
        jax.block_until_ready(outs)
        t0 = time.perf_counter()
        for _ in range(iters):
            outs = self.fn(*concat_in, *outs)
        jax.block_until_ready(outs)
        t1 = time.perf_counter()
        return (t1 - t0) / iters, self._split(outs)


_CACHE = {}


def kernel(**inputs):
    """Full-input DiT block on 8 NeuronCores; returns full [B, N, HID] f32."""
    if "nc" not in _CACHE:
        _CACHE["nc"] = build_kernel()
        _CACHE["runner"] = SpmdRunner(_CACHE["nc"], 8)
    maps = make_in_maps(inputs)
    results = _CACHE["runner"].run(maps)
    return assemble_output(results)
